# revision 33
# baseline (speedup 1.0000x reference)
"""Arcee decoder layer on 8 TRN2 NeuronCores — TP8, fp8 hi/lo DoubleRow.

Sharding (8-way TP, transposed activation layout [hidden, seq] on device):
  - core c owns: q heads 4c..4c+3 + kv head c, residual rows 512c..512c+511,
    intermediate cols 2048c..2048c+2047.
  - Big GEMMs (qkv/o/up/down) run as 3-term hi/lo fp8e4m3 DoubleRow:
    W.x ~= Whi.xhi + Whi.xlo + Wlo.xhi, each term contracting 256 rows per
    0.5-cycle/row matmul. Weights pre-quantized on host (ln1/ln2 and
    per-tensor scales folded); activations split hi/lo on device.
  - RMSNorm: un-normalized residual stream AllGathered with per-core partial
    sum-of-squares row embedded; rsqrt scale folded into PSUM eviction.
  - AG payload [520, 1024] bf16 per S-half: rows 0-511 carry x rows as fp8
    bytes (hi in bf16 cols 0-511, lo in 512-1023), row 512 = bf16 ssq row.
  - o_proj is COLUMN-sharded: per-chunk fp8 hi/lo attention outputs are
    AllGathered (0.5 MiB/rank, much cheaper than the 4 MiB ReduceScatter of
    o partials), then each core computes the full contraction for its own
    512 hid rows. residual2 is fused into the o eviction via an extra
    DoubleRow matmul with a 128*I fp8 identity against the (xhi, xlo) pair,
    so res2 never round-trips through a collective.
  - down_proj partials reduce via bf16 ReduceScatter split into uneven
    pieces so the exposed tail is small.
  - attention (scores/softmax/PV) stays bf16. DMAs are batched into fat
    tiles and spread across queues to keep dispatch off the critical path.
"""
import sys

sys.path.insert(0, "/opt/trn_rl_repo")

import contextlib
import math
import numpy as np
import ml_dtypes

import concourse.bass as bass
import concourse.mybir as mybir
import concourse.tile as tile
from concourse import bacc
from concourse.bass_isa import ReduceOp
from concourse.masks import make_identity

F32 = mybir.dt.float32
BF16 = mybir.dt.bfloat16
F8 = mybir.dt.float8e4
F8E5 = mybir.dt.float8e5
I32 = mybir.dt.int32
AF = mybir.ActivationFunctionType
ALU = mybir.AluOpType
DR = mybir.MatmulPerfMode.DoubleRow
F8NP = ml_dtypes.float8_e4m3

N_CORES = 8
S = 2048
HID = 4096
N_HEADS = 32
N_KV = 8
DHEAD = 128
INTER = 16384
EPS = 1e-5
THETA = 10000.0

HQ = N_HEADS // N_CORES          # 4 q heads per core
HID_SH = HID // N_CORES          # 512 residual rows per core
INT_SH = INTER // N_CORES        # 2048 intermediate per core
NJ = HQ + 2                      # qkv col tiles per core (4q + k + v)
QKV_COLS = NJ * DHEAD            # 768
P = 128
SC = 512                         # seq chunk
NSC = S // SC                    # 4
SH = S // 2                      # 1024 (half)
NT_HID = HID // P                # 32
NT_HSH = HID_SH // P             # 4
NT_INT = INT_SH // P             # 16
NPH = NT_HID // 2                # 16 k-pairs over HID
NPI = NT_INT // 2                # 8 k-pairs over INT_SH
BLK = HID_SH + 8                 # 520 payload rows
TWO_PI = 2.0 * math.pi

# fp8 scales (activations unscaled; weights scaled on host)
SW1 = 1024.0
SWO = 128.0                      # must stay fp8-representable (identity add)
SWU = 1024.0
SWD = 1024.0
SQU = 0.25                       # scale on u = relu(z)^2
CE1 = 1.0 / SW1                  # qkv evict const (with rsqrt row)
CO = 1.0 / SWO                   # o evict const
ALPHA_UP = math.sqrt(SQU) / SWU  # relu evict scale
CD = 1.0 / (SWD * SQU)           # down evict const (with 1/var row)

# down RS pieces (m-tile counts; sum = 32); tapered so the tail is short.
PIECES = [8, 8, 8, 4, 2, 1, 1]

# softmax bias: probs stored as e5m2 exp(s*inv_sqrt_d - XC); max masked
# score*inv_sqrt_d is 12.47 and the min row-max is -4.03 for this input
# distribution, so XC=4 keeps exp in [3e-4, 4.8e3] — inside e5m2 range.
XC = 4.0


def build_graph():
    nc = bacc.Bacc(None, target_bir_lowering=False, debug=False)

    hT = nc.declare_dram_parameter("hT", [HID_SH, S], BF16, isOutput=False)
    rT = nc.declare_dram_parameter("rT", [HID_SH, S], BF16, isOutput=False)
    cos_in = nc.declare_dram_parameter("cos2", [P, S], BF16, isOutput=False)
    sin_in = nc.declare_dram_parameter("sin_neg", [P, S], BF16, isOutput=False)
    # weights arrive pre-laid-out in SBUF tile order (see _lay_tiles):
    # row = mtile*128 + p, cols = (t, two, m) flattened — every per-tile DMA
    # is a contiguous [128, X] block (full-width descriptors).
    wq_hi = nc.declare_dram_parameter("wq_hi", [NJ * P, NPH * 2 * P], F8, isOutput=False)
    wq_lo = nc.declare_dram_parameter("wq_lo", [NJ * P, NPH * 2 * P], F8, isOutput=False)
    wo_hi = nc.declare_dram_parameter("wo_hi", [P, NPH * 2 * HID_SH], F8, isOutput=False)
    wo_lo = nc.declare_dram_parameter("wo_lo", [P, NPH * 2 * HID_SH], F8, isOutput=False)
    wu_hi = nc.declare_dram_parameter("wu_hi", [NT_INT * P, NPH * 2 * P], F8, isOutput=False)
    wu_lo = nc.declare_dram_parameter("wu_lo", [NT_INT * P, NPH * 2 * P], F8, isOutput=False)
    wd_hi = nc.declare_dram_parameter("wd_hi", [NT_HID * P, NPI * 2 * P], F8, isOutput=False)
    wd_lo = nc.declare_dram_parameter("wd_lo", [NT_HID * P, NPI * 2 * P], F8, isOutput=False)
    out_res2 = nc.declare_dram_parameter("res2T", [HID_SH, S], F32, isOutput=True)
    out_mlp = nc.declare_dram_parameter("mlpT", [HID_SH, S], F32, isOutput=True)

    RG = [list(range(N_CORES))]
    inv_sqrt_d = 1.0 / math.sqrt(DHEAD)

    # per-tile contiguous weight slices -> [p, t(pair), two, m]
    def wslice(w, i, m=P):
        return w[i * P:(i + 1) * P, :].rearrange("p (t two m) -> p t two m",
                                                 two=2, m=m)

    hT_v = hT[:].rearrange("(i p) s -> p i s", p=P)
    rT_v = rT[:].rearrange("(i p) s -> p i s", p=P)
    res2_v = out_res2[:].rearrange("(i p) s -> p i s", p=P)

    with tile.TileContext(nc) as tc:
        with contextlib.ExitStack() as ctx:
            const = ctx.enter_context(tc.tile_pool(name="const", bufs=1))
            acc = ctx.enter_context(tc.tile_pool(name="acc", bufs=6, space="PSUM"))
            rowps = ctx.enter_context(tc.tile_pool(name="rowps", bufs=1, space="PSUM"))
            tpps = ctx.enter_context(tc.tile_pool(name="tpps", bufs=1, space="PSUM"))
            dram = ctx.enter_context(tc.tile_pool(name="dram", bufs=1, space="DRAM"))

            ones_bf = const.tile([P, 1], BF16)
            nc.vector.memset(ones_bf[:], 1.0)
            # DR lhsT needs pair-step %16==0, so the rowsum ones tile is
            # [P, 2, 16] (rows 0-15 of the result all carry the same sum)
            ones_f5p = const.tile([P, 2, 16], F8E5)
            nc.vector.memset(ones_f5p[:], 1.0)
            negxc = const.tile([P, 1], F32)
            nc.vector.memset(negxc[:], -XC)

            # DRAM scratch
            ag1_in = [dram.tile([BLK, SC], BF16, name=f"ag1_in{s_}")
                      for s_ in range(NSC)]
            ag1_out = [dram.tile([N_CORES * BLK, SC], BF16, name=f"ag1_out{s_}",
                                 addr_space="Shared") for s_ in range(NSC)]
            ag2_in = [dram.tile([BLK, SH], BF16, name=f"ag2_in{h}") for h in range(2)]
            ag2_out = [dram.tile([N_CORES * BLK, SH], BF16, name=f"ag2_out{h}",
                                 addr_space="Shared") for h in range(2)]
            # per-chunk attention-out AG: fp8 hi (rows 0-511) + lo (512-1023)
            aag_in = [dram.tile([2 * HID_SH, SC // 2], BF16, name=f"aag_in{sc}")
                      for sc in range(NSC)]
            aag_out = [dram.tile([N_CORES * 2 * HID_SH, SC // 2], BF16,
                                 name=f"aag_out{sc}", addr_space="Shared")
                       for sc in range(NSC)]
            rs2_in = [dram.tile([mc * P, S], BF16, name=f"rs2_in{pi}")
                      for pi, mc in enumerate(PIECES)]
            rs2_out = [dram.tile([mc * P // N_CORES, S], BF16,
                                 name=f"rs2_out{pi}")
                       for pi, mc in enumerate(PIECES)]

            ag1_v = [t[:].rearrange("(c r) s -> c r s", r=BLK) for t in ag1_out]
            ag2_v = [t[:].rearrange("(c r) s -> c r s", r=BLK) for t in ag2_out]
            ag1_8 = [t[:].bitcast(F8).rearrange("(c r) s -> c r s", r=BLK)
                     for t in ag1_out]
            ag2_8 = [t[:].bitcast(F8).rearrange("(c r) s -> c r s", r=BLK)
                     for t in ag2_out]

            # per-chunk ag1 payload regions (hi fp8 | lo fp8, + ssq row)
            def pay1_hi(t):
                return t[0:HID_SH, 0:SC // 2].bitcast(F8) \
                    .rearrange("(i p) s -> p i s", p=P)

            def pay1_lo(t):
                return t[0:HID_SH, SC // 2:SC].bitcast(F8) \
                    .rearrange("(i p) s -> p i s", p=P)
            aag_in8 = [t[:].bitcast(F8) for t in aag_in]           # [1024, SC]
            aag_out8 = [t[:].bitcast(F8).rearrange("(c r) s -> c r s",
                                                   r=2 * HID_SH)
                        for t in aag_out]                          # [8,1024,SC]

            # payload hi/lo region views as [p, i, s] fp8
            def pay_hi(t, cb0):
                return t[0:HID_SH, cb0:cb0 + SC // 2].bitcast(F8) \
                    .rearrange("(i p) s -> p i s", p=P)

            def pay_lo(t, cb0):
                return t[0:HID_SH, SH // 2 + cb0:SH // 2 + cb0 + SC // 2] \
                    .bitcast(F8).rearrange("(i p) s -> p i s", p=P)

            # =========== era A pools (attention + residual stream) ===========
            a_es = contextlib.ExitStack()
            apers = a_es.enter_context(tc.tile_pool(name="apers", bufs=1))
            awork = a_es.enter_context(tc.tile_pool(name="awork", bufs=1))
            owork = a_es.enter_context(tc.tile_pool(name="owork", bufs=1))

            _cnt = [0]

            def wtile(pool, shape, dt, tag, bufs):
                _cnt[0] += 1
                return pool.tile(shape, dt, tag=tag, bufs=bufs,
                                 name=f"t_{_cnt[0]}")

            # ---- phase 1: x = h + r; hi/lo fp8 + ssq into payload ----
            with tc.tile_pool(name="p1", bufs=1) as p1:
                for sc in range(NSC):
                    cs = slice(sc * SC, (sc + 1) * SC)
                    ps = rowps.tile([1, SC], F32, tag="row", name=f"ssq1p{sc}")
                    hf = wtile(p1, [P, NT_HSH, SC], BF16, "hf", 2)
                    rf = wtile(p1, [P, NT_HSH, SC], BF16, "rf", 2)
                    nc.sync.dma_start(hf[:], hT_v[:, :, cs])
                    nc.sync.dma_start(rf[:], rT_v[:, :, cs])
                    xt = wtile(p1, [P, NT_HSH, SC], F32, "xt", 2)
                    nc.vector.tensor_tensor(xt[:], hf[:], rf[:], ALU.add)
                    xhi = wtile(p1, [P, NT_HSH, SC], F8, "xhi", 2)
                    nc.vector.tensor_copy(xhi[:], xt[:])
                    xlo = wtile(p1, [P, NT_HSH, SC], F8, "xlo", 2)
                    nc.vector.tensor_tensor(xlo[:], xt[:], xhi[:], ALU.subtract)
                    nc.scalar.dma_start(pay1_hi(ag1_in[sc]), xhi[:])
                    nc.sync.dma_start(pay1_lo(ag1_in[sc]), xlo[:])
                    sq = wtile(p1, [P, NT_HSH, SC], BF16, "sq", 2)
                    nc.scalar.activation(sq[:], xt[:], AF.Square)
                    for i in range(NT_HSH):
                        nc.tensor.matmul(ps[:], ones_bf[:], sq[:, i, :],
                                         start=(i == 0), stop=(i == NT_HSH - 1))
                    ssq1b = awork.tile([1, SC], BF16, tag="ssq1b", bufs=2,
                                       name=f"ssq1b{sc}")
                    nc.vector.tensor_copy(ssq1b[:], ps[:])
                    nc.sync.dma_start(
                        ag1_in[sc][HID_SH:HID_SH + 1, 0:SC], ssq1b[:])
                    nc.gpsimd.collective_compute(
                        "AllGather", ALU.bypass, replica_groups=RG,
                        ins=[ag1_in[sc][:].opt()], outs=[ag1_out[sc][:].opt()])

            # ---- rope tables + masks (after AGs so phase-1 wins queues) ----
            ident = apers.tile([P, P], BF16)
            make_identity(nc, ident[:])
            cos2 = apers.tile([P, S], BF16)
            sin_neg = apers.tile([P, S], BF16)
            cmask = []
            for j in range(SC // P):
                mk = apers.tile([P, SC], BF16, name=f"cmask{j}")
                nc.vector.memset(mk[:], 1.0)
                nc.gpsimd.affine_select(mk[:], mk[:], pattern=[[1, SC]],
                                        base=-j * P, channel_multiplier=-1,
                                        compare_op=ALU.is_ge, fill=0.0)
                cmask.append(mk)

            nc.sync.dma_start(cos2[:], cos_in[:])
            nc.sync.dma_start(sin_neg[:], sin_in[:])

            # fp8 identity * SWO for the fused residual add in o_proj
            sw_id = apers.tile([P, 2, P], F8, name="sw_id")
            nc.scalar.activation(sw_id[:, 0, :], ident[:], AF.Copy, scale=SWO)
            nc.scalar.activation(sw_id[:, 1, :], ident[:], AF.Copy, scale=SWO)

            # ---- persistent attention-era tiles ----
            kT = apers.tile([P, S], BF16, name="kT")
            vT = apers.tile([P, S], BF16, name="vT")
            s1b = apers.tile([P, S], BF16, name="s1b")

            # wo SBUF cache: created late, in the region wqp frees
            wo_es = contextlib.ExitStack()

            # wq SBUF cache + gathered-x tiles (freed after qkv3)
            wq_es = contextlib.ExitStack()
            wqp = wq_es.enter_context(tc.tile_pool(name="wqp", bufs=1))
            wq_sb_hi = [wqp.tile([P, NPH, 2, P], F8, name=f"wqh{j}") for j in range(NJ)]
            wq_sb_lo = [wqp.tile([P, NPH, 2, P], F8, name=f"wql{j}") for j in range(NJ)]
            for j in range(NJ):
                nc.scalar.dma_start(wq_sb_hi[j][:], wslice(wq_hi, j))
            for j in range(NJ):
                nc.scalar.dma_start(wq_sb_lo[j][:], wslice(wq_lo, j))

            def emit_s1b(sc):
                cs = slice(sc * SC, (sc + 1) * SC)
                srows_b = awork.tile([8, SC], BF16, tag="srb", bufs=1,
                                     name=f"sr1b{sc}")
                nc.gpsimd.dma_start(srows_b[:], ag1_v[sc][:, HID_SH, 0:SC])
                srows = awork.tile([8, SC], F32, tag="srf", bufs=1,
                                   name=f"sr1f{sc}")
                nc.vector.tensor_copy(srows[:], srows_b[:])
                ssum = awork.tile([8, SC], F32, tag="ssum", bufs=1,
                                  name=f"ss1{sc}")
                nc.gpsimd.partition_all_reduce(ssum[:], srows[:], channels=8,
                                               reduce_op=ReduceOp.add)
                var = awork.tile([1, SC], F32, tag="var", bufs=1, name=f"v1{sc}")
                nc.scalar.activation(var[:], ssum[:1, :], AF.Copy,
                                     scale=1.0 / HID, bias=EPS)
                nc.vector.reciprocal(var[:], var[:])
                varb = awork.tile([1, SC], BF16, tag="varb", bufs=1,
                                  name=f"v1b{sc}")
                nc.scalar.activation(varb[:], var[:], AF.Sqrt, scale=CE1 * CE1)
                nc.gpsimd.partition_broadcast(s1b[:, cs], varb[:])

            qcs = {}
            gqs = {}

            def load_gq(sc):
                ghi, glo = [], []
                for cb in range(N_CORES):
                    g = wqp.tile([P, 2, 2, SC], F8, tag="ghi", bufs=9,
                                 name=f"gh{cb}_{sc}")
                    nc.gpsimd.dma_start(
                        g[:], ag1_8[sc][cb, 0:HID_SH, 0:SC]
                        .rearrange("(tp two p) n -> p tp two n", tp=2, two=2))
                    ghi.append(g)
                    g = wqp.tile([P, 2, 2, SC], F8, tag="glo", bufs=9,
                                 name=f"gl{cb}_{sc}")
                    nc.sync.dma_start(
                        g[:], ag1_8[sc][cb, 0:HID_SH, SC:2 * SC]
                        .rearrange("(tp two p) n -> p tp two n", tp=2, two=2))
                    glo.append(g)
                gqs[sc] = (ghi, glo)

            def emit_qkv(sc):
                cs = slice(sc * SC, (sc + 1) * SC)
                if sc not in gqs:
                    load_gq(sc)
                ghi, glo = gqs[sc]

                def rhs(lst, g):
                    return lst[g // 2][:, g % 2]

                qc = {}
                pss = [acc.tile([P, SC], F32, tag="acc", name=f"qk{j}_{sc}")
                       for j in range(NJ)]
                for g in range(NPH):
                    for j in range(NJ):
                        nc.tensor.matmul(pss[j][:], wq_sb_hi[j][:, g],
                                         rhs(ghi, g), start=(g == 0),
                                         stop=False, perf_mode=DR)
                for g in range(NPH):
                    for j in range(NJ):
                        nc.tensor.matmul(pss[j][:], wq_sb_hi[j][:, g],
                                         rhs(glo, g), start=False, stop=False,
                                         perf_mode=DR)
                for g in range(NPH):
                    for j in range(NJ):
                        nc.tensor.matmul(pss[j][:], wq_sb_lo[j][:, g],
                                         rhs(ghi, g), start=False,
                                         stop=(g == NPH - 1), perf_mode=DR)
                for j in range(NJ):
                    if j < HQ:
                        dst = awork.tile([P, SC], BF16, tag="qc", bufs=8,
                                         name=f"qc{j}_{sc}")
                        qc[j] = dst
                        nc.vector.tensor_tensor(dst[:], pss[j][:], s1b[:, cs],
                                                ALU.mult)
                    else:
                        dst = kT if j == HQ else vT
                        nc.vector.tensor_tensor(dst[:, cs], pss[j][:],
                                                s1b[:, cs], ALU.mult)
                qcs[sc] = qc

            def emit_attn(sc):
                cs = slice(sc * SC, (sc + 1) * SC)
                qc = qcs[sc]
                # rope on q tiles + k chunk
                for j in range(HQ + 1):
                    tv = qc[j][:] if j < HQ else kT[:, cs]
                    swp = wtile(awork, [P, SC], BF16, "t1k", 3)
                    nc.sync.dma_start(swp[:64, :], tv[64:, :])
                    nc.sync.dma_start(swp[64:, :], tv[:64, :])
                    m1 = wtile(awork, [P, SC], BF16, "t1k", 3)
                    nc.vector.tensor_tensor(m1[:], tv, cos2[:, cs], ALU.mult)
                    m2 = wtile(awork, [P, SC], BF16, "t1k", 3)
                    nc.vector.tensor_tensor(m2[:], swp[:], sin_neg[:, cs], ALU.mult)
                    nc.vector.tensor_tensor(tv, m1[:], m2[:], ALU.add)

                # v transpose in place
                for t in range(sc * (SC // P), (sc + 1) * (SC // P)):
                    pst = tpps.tile([P, P], BF16, tag="tp", name=f"tp{t}")
                    nc.tensor.transpose(pst[:], vT[:, t * P:(t + 1) * P], ident[:])
                    nc.vector.tensor_copy(vT[:, t * P:(t + 1) * P], pst[:])

                # attention: 4 heads x this chunk; fp8 hi/lo attn output
                nsk = (sc + 1) * (SC // P)
                ahi = awork.tile([P, HQ, SC], F8, tag="ahi", bufs=1,
                                 name=f"ahi{sc}")
                alo = awork.tile([P, HQ, SC], F8, tag="alo", bufs=1,
                                 name=f"alo{sc}")
                for h in range(HQ):
                    pv = acc.tile([P, SC], F32, tag="acc", name=f"pv{h}_{sc}")
                    rs = rowps.tile([1, SC], F32, tag="row", name=f"rs{h}_{sc}")
                    for skt in range(nsk):
                        sps = acc.tile([P, SC], F32, tag="acc",
                                       name=f"s{h}_{sc}_{skt}")
                        nc.tensor.matmul(sps[:], kT[:, skt * P:(skt + 1) * P],
                                         qc[h][:], start=True, stop=True)
                        ex = wtile(awork, [P, SC], BF16, "ex", 3)
                        nc.scalar.activation(ex[:], sps[:], AF.Exp,
                                             scale=inv_sqrt_d)
                        if skt >= 4 * sc:
                            nc.vector.tensor_tensor(ex[:], ex[:],
                                                    cmask[skt - 4 * sc][:],
                                                    ALU.mult)
                        nc.tensor.matmul(rs[:], ones_bf[:], ex[:],
                                         start=(skt == 0), stop=(skt == nsk - 1))
                        nc.tensor.matmul(pv[:], vT[:, skt * P:(skt + 1) * P],
                                         ex[:], start=(skt == 0),
                                         stop=(skt == nsk - 1))
                    rcp = awork.tile([1, SC], F32, tag="rcp", bufs=1,
                                     name=f"rcp{h}_{sc}")
                    nc.vector.reciprocal(rcp[:], rs[:])
                    rcpb = wtile(awork, [P, SC], F32, "rcpb", 1)
                    nc.gpsimd.partition_broadcast(rcpb[:], rcp[:])
                    a32 = wtile(awork, [P, SC], F32, "a32", 1)
                    nc.vector.tensor_tensor(a32[:], pv[:], rcpb[:], ALU.mult)
                    nc.vector.tensor_copy(ahi[:, h, :], a32[:])
                    nc.vector.tensor_tensor(alo[:, h, :], a32[:], ahi[:, h, :],
                                            ALU.subtract)

                # publish fp8 attn out + AllGather for column-sharded o_proj
                nc.scalar.dma_start(
                    aag_in8[sc][0:HID_SH, :]
                    .rearrange("(h p) s -> p h s", p=P), ahi[:])
                nc.sync.dma_start(
                    aag_in8[sc][HID_SH:2 * HID_SH, :]
                    .rearrange("(h p) s -> p h s", p=P), alo[:])
                nc.gpsimd.collective_compute(
                    "AllGather", ALU.bypass, replica_groups=RG,
                    ins=[aag_in[sc][:].opt()], outs=[aag_out[sc][:].opt()])

            def emit_o(sc):
                """Column-sharded o_proj for chunk sc with fused residual2."""
                cs = slice(sc * SC, (sc + 1) * SC)
                hh = sc // 2
                cb0 = (sc % 2) * (SC // 2)
                # x (residual stream) hi/lo pair for the fused add
                xp = owork.tile([P, NT_HSH, 2, SC], F8, tag="xp", bufs=1,
                                name=f"xp{sc}")
                nc.scalar.dma_start(xp[:, :, 0, :], pay1_hi(ag1_in[sc]))
                nc.sync.dma_start(xp[:, :, 1, :], pay1_lo(ag1_in[sc]))

                r2f = owork.tile([P, NT_HSH, SC], F32, tag="r2f", bufs=1,
                                 name=f"r2f{sc}")
                r2hi = owork.tile([P, NT_HSH, SC], F8, tag="r2hi", bufs=1,
                                  name=f"r2hi{sc}")
                r2lo = owork.tile([P, NT_HSH, SC], F8, tag="r2lo", bufs=1,
                                  name=f"r2lo{sc}")
                sq = owork.tile([P, NT_HSH, SC], BF16, tag="r2sq", bufs=1,
                                name=f"r2sq{sc}")
                ps2 = rowps.tile([1, SC], F32, tag="row", name=f"ssq2_{sc}")
                pss = [acc.tile([P, SC], F32, tag="acc", name=f"o{m}_{sc}")
                       for m in range(NT_HSH)]
                for g in range(NPH):
                    cb, j = g // 2, g % 2
                    ghi = owork.tile([P, 2, SC], F8, tag="oghi", bufs=3,
                                     name=f"ogh{g}_{sc}")
                    nc.scalar.dma_start(
                        ghi[:], aag_out8[sc][cb, j * 256:(j + 1) * 256, :]
                        .rearrange("(two p) s -> p two s", p=P))
                    glo = owork.tile([P, 2, SC], F8, tag="oglo", bufs=3,
                                     name=f"ogl{g}_{sc}")
                    nc.sync.dma_start(
                        glo[:], aag_out8[sc][cb, HID_SH + j * 256:
                                             HID_SH + (j + 1) * 256, :]
                        .rearrange("(two p) s -> p two s", p=P))
                    for m in range(NT_HSH):
                        nc.tensor.matmul(pss[m][:],
                                         wo_sb_hi[:, g, :, m * P:(m + 1) * P],
                                         ghi[:], start=(g == 0), stop=False,
                                         perf_mode=DR)
                    for m in range(NT_HSH):
                        nc.tensor.matmul(pss[m][:],
                                         wo_sb_hi[:, g, :, m * P:(m + 1) * P],
                                         glo[:], start=False, stop=False,
                                         perf_mode=DR)
                    for m in range(NT_HSH):
                        nc.tensor.matmul(pss[m][:],
                                         wo_sb_lo[:, g, :, m * P:(m + 1) * P],
                                         ghi[:], start=False, stop=False,
                                         perf_mode=DR)
                for m in range(NT_HSH):
                    # += SWO * (xhi + xlo): fused residual add
                    nc.tensor.matmul(pss[m][:], sw_id[:], xp[:, m],
                                     start=False, stop=True, perf_mode=DR)
                    nc.scalar.activation(r2f[:, m, :], pss[m][:], AF.Copy,
                                         scale=CO)
                    nc.vector.tensor_copy(r2hi[:, m, :], r2f[:, m, :])
                    nc.vector.tensor_tensor(r2lo[:, m, :], r2f[:, m, :],
                                            r2hi[:, m, :], ALU.subtract)
                    nc.scalar.activation(sq[:, m, :], r2f[:, m, :], AF.Square)
                    nc.tensor.matmul(ps2[:], ones_bf[:], sq[:, m, :],
                                     start=(m == 0), stop=(m == NT_HSH - 1))
                nc.gpsimd.dma_start(res2_v[:, :, cs], r2f[:])
                nc.gpsimd.dma_start(pay_hi(ag2_in[hh], cb0), r2hi[:])
                nc.gpsimd.dma_start(pay_lo(ag2_in[hh], cb0), r2lo[:])
                ssq2 = owork.tile([1, SC], BF16, tag="ssq2", bufs=2,
                                  name=f"sq2_{sc}")
                nc.vector.tensor_copy(ssq2[:], ps2[:])
                nc.gpsimd.dma_start(
                    ag2_in[hh][HID_SH:HID_SH + 1,
                               (sc % 2) * SC:(sc % 2) * SC + SC], ssq2[:])

            def emit_ag2(hh):
                nc.gpsimd.collective_compute(
                    "AllGather", ALU.bypass, replica_groups=RG,
                    ins=[ag2_in[hh][:].opt()], outs=[ag2_out[hh][:].opt()])

            # =========== era B pools (MLP) ===========
            # created mid-schedule (pool creation reserves SBUF immediately);
            # closures below bind these names at call time.
            b_es = contextlib.ExitStack()
            mpers = mwork = mstr = None
            s2b = None

            def emit_s2b(sc):
                cs = slice(sc * SC, (sc + 1) * SC)
                hh = sc // 2
                hcs = slice((sc % 2) * SC, (sc % 2) * SC + SC)
                srows_b = mwork.tile([8, SC], BF16, tag="srb2", bufs=1,
                                     name=f"sr2b{sc}")
                nc.gpsimd.dma_start(srows_b[:], ag2_v[hh][:, HID_SH, hcs])
                srows = mwork.tile([8, SC], F32, tag="srf2", bufs=1,
                                   name=f"sr2f{sc}")
                nc.vector.tensor_copy(srows[:], srows_b[:])
                ssum = mwork.tile([8, SC], F32, tag="ssum2", bufs=1,
                                  name=f"ss2{sc}")
                nc.gpsimd.partition_all_reduce(ssum[:], srows[:], channels=8,
                                               reduce_op=ReduceOp.add)
                var = mwork.tile([1, SC], F32, tag="var2", bufs=2, name=f"v2{sc}")
                nc.scalar.activation(var[:], ssum[:1, :], AF.Copy,
                                     scale=1.0 / (HID * CD), bias=EPS / CD)
                nc.vector.reciprocal(var[:], var[:])     # = CD / var
                varb2 = mwork.tile([1, SC], BF16, tag="vb2", bufs=2,
                                   name=f"vb2{sc}")
                nc.vector.tensor_copy(varb2[:], var[:])
                nc.gpsimd.partition_broadcast(s2b[:, cs], varb2[:])

            g2pool = {}

            def load_g2(half):
                g2p = g2pool["p"]
                ghi, glo = [], []
                for cb in range(N_CORES):
                    g = g2p.tile([P, 2, 2, SH], F8, tag="g2h", bufs=8,
                                 name=f"g2h{cb}_{half}")
                    nc.gpsimd.dma_start(
                        g[:], ag2_8[half][cb, 0:HID_SH, 0:SH]
                        .rearrange("(tp two p) n -> p tp two n", tp=2, two=2))
                    ghi.append(g)
                    g = g2p.tile([P, 2, 2, SH], F8, tag="g2l", bufs=8,
                                 name=f"g2l{cb}_{half}")
                    nc.sync.dma_start(
                        g[:], ag2_8[half][cb, 0:HID_SH, SH:2 * SH]
                        .rearrange("(tp two p) n -> p tp two n", tp=2, two=2))
                    glo.append(g)
                return ghi, glo


            g2 = {}

            # ============ era A schedule ============
            load_gq(0)
            emit_s1b(0)
            emit_qkv(0)
            emit_s1b(1)
            emit_qkv(1)
            emit_attn(0)
            emit_s1b(2)
            emit_qkv(2)
            emit_attn(1)
            emit_s1b(3)
            emit_qkv(3)
            wq_es.close()
            mpers = b_es.enter_context(tc.tile_pool(name="mpers", bufs=1, side="right"))
            mwork = b_es.enter_context(tc.tile_pool(name="mwork", bufs=1, side="right"))
            wop = wo_es.enter_context(tc.tile_pool(name="wop", bufs=1))
            wo_sb_hi = wop.tile([P, NPH, 2, HID_SH], F8, name="wo_h")
            wo_sb_lo = wop.tile([P, NPH, 2, HID_SH], F8, name="wo_l")
            nc.gpsimd.dma_start(wo_sb_hi[:], wslice(wo_hi, 0, m=HID_SH))
            nc.gpsimd.dma_start(wo_sb_lo[:], wslice(wo_lo, 0, m=HID_SH))
            s2b = mpers.tile([P, S], BF16, name="s2b")
            g2pool["p"] = b_es.enter_context(tc.tile_pool(name="g2p", bufs=1, side="right"))
            emit_attn(2)
            emit_o(0)
            emit_o(1)
            emit_ag2(0)
            emit_attn(3)
            emit_s2b(0)
            emit_s2b(1)
            g2[0] = load_g2(0)
            emit_o(2)
            emit_o(3)
            emit_ag2(1)
            g2[1] = load_g2(1)
            emit_s2b(2)
            emit_s2b(3)
            wo_es.close()
            a_es.close()

            mstr = b_es.enter_context(tc.tile_pool(name="mstr", bufs=1, side="right"))
            utp = b_es.enter_context(tc.tile_pool(name="utp", bufs=1, side="right"))
            ut_hi = [utp.tile([P, 2, S], F8, name=f"uth{g}") for g in range(NPI)]
            ut_lo = [utp.tile([P, 2, S], F8, name=f"utl{g}") for g in range(NPI)]

            def emit_up(half, it_range):
                ghi, glo = g2[half]

                def rhs(lst, g, ncs):
                    return lst[g // 2][:, g % 2, :, ncs]

                for it in it_range:
                    wh = mstr.tile([P, NPH, 2, P], F8, tag="wuh", bufs=2,
                                   name=f"wuh{it}_{half}")
                    nc.scalar.dma_start(wh[:], wslice(wu_hi, it))
                    wl = mstr.tile([P, NPH, 2, P], F8, tag="wul", bufs=2,
                                   name=f"wul{it}_{half}")
                    nc.scalar.dma_start(wl[:], wslice(wu_lo, it))
                    pss = [acc.tile([P, SC], F32, tag="acc", name=f"up{it}_{2*half+ci}")
                           for ci in range(2)]
                    for g in range(NPH):
                        for ci in range(2):
                            nc.tensor.matmul(pss[ci][:], wh[:, g],
                                             rhs(ghi, g, slice(ci * SC, (ci + 1) * SC)),
                                             start=(g == 0), stop=False,
                                             perf_mode=DR)
                    for g in range(NPH):
                        for ci in range(2):
                            nc.tensor.matmul(pss[ci][:], wh[:, g],
                                             rhs(glo, g, slice(ci * SC, (ci + 1) * SC)),
                                             start=False, stop=False, perf_mode=DR)
                    for g in range(NPH):
                        for ci in range(2):
                            nc.tensor.matmul(pss[ci][:], wl[:, g],
                                             rhs(ghi, g, slice(ci * SC, (ci + 1) * SC)),
                                             start=False, stop=(g == NPH - 1),
                                             perf_mode=DR)
                    for ci in range(2):
                        sc = 2 * half + ci
                        cs = slice(sc * SC, (sc + 1) * SC)
                        rl = mwork.tile([P, SC], F32, tag="rl", bufs=3,
                                        name=f"rl{it}_{sc}")
                        nc.scalar.activation(rl[:], pss[ci][:], AF.Relu,
                                             scale=ALPHA_UP)
                        nc.vector.tensor_tensor(rl[:], rl[:], rl[:], ALU.mult)
                        nc.vector.tensor_copy(ut_hi[it // 2][:, it % 2, cs], rl[:])
                        nc.vector.tensor_tensor(ut_lo[it // 2][:, it % 2, cs],
                                                rl[:], ut_hi[it // 2][:, it % 2, cs],
                                                ALU.subtract)

            # ---- era B schedule ----
            emit_up(0, range(NT_INT))
            emit_up(1, range(NT_INT))

            # ---- down proj: 3-term DoubleRow over full S, uneven RS ----
            mstart = 0
            for pi, mc in enumerate(PIECES):
                rs2t = rs2_in[pi]
                rs2_fat = rs2t[:].rearrange("(q p) s -> p q s", p=P)
                for mq in range(mc):
                    m = mstart + mq
                    wh = mstr.tile([P, NPI, 2, P], F8, tag="wdh", bufs=3,
                                   name=f"wdh{m}")
                    nc.scalar.dma_start(wh[:], wslice(wd_hi, m))
                    wl = mstr.tile([P, NPI, 2, P], F8, tag="wdl", bufs=3,
                                   name=f"wdl{m}")
                    nc.scalar.dma_start(wl[:], wslice(wd_lo, m))
                    evf = mwork.tile([P, NSC, SC], BF16, tag="dnev", bufs=1,
                                     name=f"dev{m}")
                    for sc in range(NSC):
                        cs = slice(sc * SC, (sc + 1) * SC)
                        ps = acc.tile([P, SC], F32, tag="acc",
                                      name=f"dn{m}_{sc}")
                        for g in range(NPI):
                            nc.tensor.matmul(ps[:], wh[:, g],
                                             ut_hi[g][:, :, cs],
                                             start=(g == 0), stop=False,
                                             perf_mode=DR)
                        for g in range(NPI):
                            nc.tensor.matmul(ps[:], wh[:, g],
                                             ut_lo[g][:, :, cs],
                                             start=False, stop=False,
                                             perf_mode=DR)
                        for g in range(NPI):
                            nc.tensor.matmul(ps[:], wl[:, g],
                                             ut_hi[g][:, :, cs],
                                             start=False,
                                             stop=(g == NPI - 1),
                                             perf_mode=DR)
                        nc.vector.tensor_tensor(evf[:, sc, :], ps[:],
                                                s2b[:, cs], ALU.mult)
                    nc.sync.dma_start(rs2_fat[:, mq], evf[:])
                nc.gpsimd.collective_compute(
                    "ReduceScatter", ALU.add, replica_groups=RG,
                    ins=[rs2t[:].opt()], outs=[rs2_out[pi][:].opt()])
                orow = mstart * P // N_CORES
                nc.gpsimd.dma_start(
                    out_mlp[orow:orow + mc * P // N_CORES, :], rs2_out[pi][:])
                mstart += mc
            b_es.close()

    nc.compile()
    return nc


def _q8_pair(x):
    x32 = np.asarray(x, np.float32)
    hi = np.asarray(np.clip(x32, -240, 240), F8NP)
    lo = np.asarray(np.clip(x32 - hi.astype(np.float32), -240, 240), F8NP)
    return np.ascontiguousarray(hi), np.ascontiguousarray(lo)


def _lay_tiles(w, mt=P):
    """[K, M] fp8 -> [(M//mt)*128, (K//256)*2*mt] in SBUF tile order.

    Row = mtile*128 + p; cols = (kpair, two, m) flattened, so each per-tile
    DMA is one contiguous [128, (K//256)*2*mt] block."""
    K, M = w.shape
    a = w.reshape(K // 256, 2, P, M // mt, mt).transpose(3, 2, 0, 1, 4)
    return np.ascontiguousarray(a.reshape(M // mt * P, (K // 256) * 2 * mt))


def shard_inputs(positions, hidden_states, residual, qkv_w, o_w, up_w, down_w,
                 ln1_w, ln2_w):
    hTf = np.ascontiguousarray(
        np.asarray(hidden_states).reshape(S, HID).T.astype(ml_dtypes.bfloat16))
    rTf = np.ascontiguousarray(
        np.asarray(residual).reshape(S, HID).T.astype(ml_dtypes.bfloat16))
    pos = np.asarray(positions).reshape(S).astype(np.float64)
    inv = 1.0 / (THETA ** (np.arange(0, DHEAD, 2, dtype=np.float64) / DHEAD))
    fr = pos[:, None] * inv                      # [S, 64]
    cost = np.cos(fr).T.astype(np.float32)       # [64, S]
    sint = np.sin(fr).T.astype(np.float32)
    cos2 = np.ascontiguousarray(
        np.concatenate([cost, cost], 0).astype(ml_dtypes.bfloat16))
    sin_neg = np.ascontiguousarray(
        np.concatenate([-sint, sint], 0).astype(ml_dtypes.bfloat16))
    q_size = N_HEADS * DHEAD
    kv = N_KV * DHEAD
    w1 = np.asarray(qkv_w, np.float32) * np.asarray(ln1_w, np.float32)[:, None] * SW1
    wof = np.asarray(o_w, np.float32) * SWO
    wuf = np.asarray(up_w, np.float32) * np.asarray(ln2_w, np.float32)[:, None] * SWU
    wdf = np.asarray(down_w, np.float32) * SWD
    in_maps = []
    for c in range(N_CORES):
        wq_c = np.concatenate([
            w1[:, c * HQ * DHEAD:(c + 1) * HQ * DHEAD],
            w1[:, q_size + c * DHEAD:q_size + (c + 1) * DHEAD],
            w1[:, q_size + kv + c * DHEAD:q_size + kv + (c + 1) * DHEAD],
        ], axis=1)
        wq_h, wq_l = _q8_pair(wq_c)
        wo_h, wo_l = _q8_pair(wof[:, c * HID_SH:(c + 1) * HID_SH])
        wu_h, wu_l = _q8_pair(wuf[:, c * INT_SH:(c + 1) * INT_SH])
        wd_h, wd_l = _q8_pair(wdf[c * INT_SH:(c + 1) * INT_SH, :])
        wq_h, wq_l = _lay_tiles(wq_h), _lay_tiles(wq_l)
        wo_h, wo_l = _lay_tiles(wo_h, mt=HID_SH), _lay_tiles(wo_l, mt=HID_SH)
        wu_h, wu_l = _lay_tiles(wu_h), _lay_tiles(wu_l)
        wd_h, wd_l = _lay_tiles(wd_h), _lay_tiles(wd_l)
        in_maps.append({
            "hT": np.ascontiguousarray(hTf[c * HID_SH:(c + 1) * HID_SH]),
            "rT": np.ascontiguousarray(rTf[c * HID_SH:(c + 1) * HID_SH]),
            "cos2": cos2, "sin_neg": sin_neg,
            "wq_hi": wq_h, "wq_lo": wq_l,
            "wo_hi": wo_h, "wo_lo": wo_l,
            "wu_hi": wu_h, "wu_lo": wu_l,
            "wd_hi": wd_h, "wd_lo": wd_l,
        })
    return in_maps


_CACHE = {}


def kernel(**inputs):
    from concourse.bass_utils import run_bass_kernel_spmd
    if "nc" not in _CACHE:
        _CACHE["nc"] = build_graph()
    nc = _CACHE["nc"]
    in_maps = shard_inputs(**{k: np.asarray(v) for k, v in inputs.items()})
    res = run_bass_kernel_spmd(nc, in_maps, core_ids=list(range(N_CORES)),
                               trace=False)
    res2T = np.concatenate([res.results[c]["res2T"] for c in range(N_CORES)], axis=0)
    mlpT = np.empty((HID, S), np.float32)
    for c in range(N_CORES):
        mt = res.results[c]["mlpT"]
        mstart = 0
        for pi, mc in enumerate(PIECES):
            rows = mc * P // N_CORES          # rows per core for this piece
            orow = mstart * P // N_CORES
            g0 = mstart * P + c * rows        # global hid row start
            mlpT[g0:g0 + rows] = mt[orow:orow + rows]
            mstart += mc
    mlp_out = np.ascontiguousarray(mlpT.T).reshape(1, S, HID)
    residual2 = np.ascontiguousarray(res2T.T).reshape(1, S, HID)
    return mlp_out, residual2


# revision 35
# speedup vs baseline: 1.0135x; 1.0135x over previous
"""Arcee decoder layer on 8 TRN2 NeuronCores — TP8, fp8 hi/lo DoubleRow.

Sharding (8-way TP, transposed activation layout [hidden, seq] on device):
  - core c owns: q heads 4c..4c+3 + kv head c, residual rows 512c..512c+511,
    intermediate cols 2048c..2048c+2047.
  - Big GEMMs (qkv/o/up/down) run as 3-term hi/lo fp8e4m3 DoubleRow:
    W.x ~= Whi.xhi + Whi.xlo + Wlo.xhi, each term contracting 256 rows per
    0.5-cycle/row matmul. Weights pre-quantized on host (ln1/ln2 and
    per-tensor scales folded); activations split hi/lo on device.
  - RMSNorm: un-normalized residual stream AllGathered with per-core partial
    sum-of-squares row embedded; rsqrt scale folded into PSUM eviction.
  - AG payload [520, 1024] bf16 per S-half: rows 0-511 carry x rows as fp8
    bytes (hi in bf16 cols 0-511, lo in 512-1023), row 512 = bf16 ssq row.
  - o_proj is COLUMN-sharded: per-chunk fp8 hi/lo attention outputs are
    AllGathered (0.5 MiB/rank, much cheaper than the 4 MiB ReduceScatter of
    o partials), then each core computes the full contraction for its own
    512 hid rows. residual2 is fused into the o eviction via an extra
    DoubleRow matmul with a 128*I fp8 identity against the (xhi, xlo) pair,
    so res2 never round-trips through a collective.
  - down_proj partials reduce via bf16 ReduceScatter split into uneven
    pieces so the exposed tail is small.
  - attention (scores/softmax/PV) stays bf16. DMAs are batched into fat
    tiles and spread across queues to keep dispatch off the critical path.
"""
import sys

sys.path.insert(0, "/opt/trn_rl_repo")

import contextlib
import math
import numpy as np
import ml_dtypes

import concourse.bass as bass
import concourse.mybir as mybir
import concourse.tile as tile
from concourse import bacc
from concourse.bass_isa import ReduceOp
from concourse.masks import make_identity

F32 = mybir.dt.float32
BF16 = mybir.dt.bfloat16
F8 = mybir.dt.float8e4
F8E5 = mybir.dt.float8e5
I32 = mybir.dt.int32
AF = mybir.ActivationFunctionType
ALU = mybir.AluOpType
DR = mybir.MatmulPerfMode.DoubleRow
F8NP = ml_dtypes.float8_e4m3

N_CORES = 8
S = 2048
HID = 4096
N_HEADS = 32
N_KV = 8
DHEAD = 128
INTER = 16384
EPS = 1e-5
THETA = 10000.0

HQ = N_HEADS // N_CORES          # 4 q heads per core
HID_SH = HID // N_CORES          # 512 residual rows per core
INT_SH = INTER // N_CORES        # 2048 intermediate per core
NJ = HQ + 2                      # qkv col tiles per core (4q + k + v)
QKV_COLS = NJ * DHEAD            # 768
P = 128
SC = 512                         # seq chunk
NSC = S // SC                    # 4
SH = S // 2                      # 1024 (half)
NT_HID = HID // P                # 32
NT_HSH = HID_SH // P             # 4
NT_INT = INT_SH // P             # 16
NPH = NT_HID // 2                # 16 k-pairs over HID
NPI = NT_INT // 2                # 8 k-pairs over INT_SH
BLK = HID_SH + 8                 # 520 payload rows
TWO_PI = 2.0 * math.pi

# fp8 scales (activations unscaled; weights scaled on host)
SW1 = 1024.0
SWO = 128.0                      # must stay fp8-representable (identity add)
SWU = 1024.0
SWD = 1024.0
SQU = 0.25                       # scale on u = relu(z)^2
CE1 = 1.0 / SW1                  # qkv evict const (with rsqrt row)
CO = 1.0 / SWO                   # o evict const
ALPHA_UP = math.sqrt(SQU) / SWU  # relu evict scale
CD = 1.0 / (SWD * SQU)           # down evict const (with 1/var row)

# down RS pieces (m-tile counts; sum = 32); tapered so the tail is short.
PIECES = [8, 8, 7, 4, 2, 2, 1]

# softmax bias: probs stored as e5m2 exp(s*inv_sqrt_d - XC); max masked
# score*inv_sqrt_d is 12.47 and the min row-max is -4.03 for this input
# distribution, so XC=4 keeps exp in [3e-4, 4.8e3] — inside e5m2 range.
XC = 4.0


def build_graph():
    nc = bacc.Bacc(None, target_bir_lowering=False, debug=False)

    hT = nc.declare_dram_parameter("hT", [HID_SH, S], BF16, isOutput=False)
    rT = nc.declare_dram_parameter("rT", [HID_SH, S], BF16, isOutput=False)
    cos_in = nc.declare_dram_parameter("cos2", [P, S], BF16, isOutput=False)
    sin_in = nc.declare_dram_parameter("sin_neg", [P, S], BF16, isOutput=False)
    # weights arrive pre-laid-out in SBUF tile order (see _lay_tiles):
    # row = mtile*128 + p, cols = (t, two, m) flattened — every per-tile DMA
    # is a contiguous [128, X] block (full-width descriptors).
    wq_hi = nc.declare_dram_parameter("wq_hi", [NJ * P, NPH * 2 * P], F8, isOutput=False)
    wq_lo = nc.declare_dram_parameter("wq_lo", [NJ * P, NPH * 2 * P], F8, isOutput=False)
    wo_hi = nc.declare_dram_parameter("wo_hi", [P, NPH * 2 * HID_SH], F8, isOutput=False)
    wo_lo = nc.declare_dram_parameter("wo_lo", [P, NPH * 2 * HID_SH], F8, isOutput=False)
    wu_hi = nc.declare_dram_parameter("wu_hi", [NT_INT * P, NPH * 2 * P], F8, isOutput=False)
    wu_lo = nc.declare_dram_parameter("wu_lo", [NT_INT * P, NPH * 2 * P], F8, isOutput=False)
    wd_hi = nc.declare_dram_parameter("wd_hi", [NT_HID * P, NPI * 2 * P], F8, isOutput=False)
    wd_lo = nc.declare_dram_parameter("wd_lo", [NT_HID * P, NPI * 2 * P], F8, isOutput=False)
    out_res2 = nc.declare_dram_parameter("res2T", [HID_SH, S], F32, isOutput=True)
    out_mlp = nc.declare_dram_parameter("mlpT", [HID_SH, S], F32, isOutput=True)

    RG = [list(range(N_CORES))]
    inv_sqrt_d = 1.0 / math.sqrt(DHEAD)

    # per-tile contiguous weight slices -> [p, t(pair), two, m]
    def wslice(w, i, m=P):
        return w[i * P:(i + 1) * P, :].rearrange("p (t two m) -> p t two m",
                                                 two=2, m=m)

    hT_v = hT[:].rearrange("(i p) s -> p i s", p=P)
    rT_v = rT[:].rearrange("(i p) s -> p i s", p=P)
    res2_v = out_res2[:].rearrange("(i p) s -> p i s", p=P)

    with tile.TileContext(nc) as tc:
        with contextlib.ExitStack() as ctx:
            const = ctx.enter_context(tc.tile_pool(name="const", bufs=1))
            acc = ctx.enter_context(tc.tile_pool(name="acc", bufs=6, space="PSUM"))
            rowps = ctx.enter_context(tc.tile_pool(name="rowps", bufs=1, space="PSUM"))
            tpps = ctx.enter_context(tc.tile_pool(name="tpps", bufs=1, space="PSUM"))
            dram = ctx.enter_context(tc.tile_pool(name="dram", bufs=1, space="DRAM"))

            ones_bf = const.tile([P, 1], BF16)
            nc.vector.memset(ones_bf[:], 1.0)
            # DR lhsT needs pair-step %16==0, so the rowsum ones tile is
            # [P, 2, 16] (rows 0-15 of the result all carry the same sum)
            ones_f5p = const.tile([P, 2, 16], F8E5)
            nc.vector.memset(ones_f5p[:], 1.0)
            negxc = const.tile([P, 1], F32)
            nc.vector.memset(negxc[:], -XC)

            # DRAM scratch
            ag1_in = [dram.tile([BLK, SC], BF16, name=f"ag1_in{s_}")
                      for s_ in range(NSC)]
            ag1_out = [dram.tile([N_CORES * BLK, SC], BF16, name=f"ag1_out{s_}",
                                 addr_space="Shared") for s_ in range(NSC)]
            ag2_in = [dram.tile([BLK, SH], BF16, name=f"ag2_in{h}") for h in range(2)]
            ag2_out = [dram.tile([N_CORES * BLK, SH], BF16, name=f"ag2_out{h}",
                                 addr_space="Shared") for h in range(2)]
            # per-chunk attention-out AG: fp8 hi (rows 0-511) + lo (512-1023)
            aag_in = [dram.tile([2 * HID_SH, SC // 2], BF16, name=f"aag_in{sc}")
                      for sc in range(NSC)]
            aag_out = [dram.tile([N_CORES * 2 * HID_SH, SC // 2], BF16,
                                 name=f"aag_out{sc}", addr_space="Shared")
                       for sc in range(NSC)]
            rs2_in = [dram.tile([mc * P, S], BF16, name=f"rs2_in{pi}")
                      for pi, mc in enumerate(PIECES)]
            rs2_out = [dram.tile([mc * P // N_CORES, S], BF16,
                                 name=f"rs2_out{pi}")
                       for pi, mc in enumerate(PIECES)]

            ag1_v = [t[:].rearrange("(c r) s -> c r s", r=BLK) for t in ag1_out]
            ag2_v = [t[:].rearrange("(c r) s -> c r s", r=BLK) for t in ag2_out]
            ag1_8 = [t[:].bitcast(F8).rearrange("(c r) s -> c r s", r=BLK)
                     for t in ag1_out]
            ag2_8 = [t[:].bitcast(F8).rearrange("(c r) s -> c r s", r=BLK)
                     for t in ag2_out]

            # per-chunk ag1 payload regions (hi fp8 | lo fp8, + ssq row)
            def pay1_hi(t):
                return t[0:HID_SH, 0:SC // 2].bitcast(F8) \
                    .rearrange("(i p) s -> p i s", p=P)

            def pay1_lo(t):
                return t[0:HID_SH, SC // 2:SC].bitcast(F8) \
                    .rearrange("(i p) s -> p i s", p=P)
            aag_in8 = [t[:].bitcast(F8) for t in aag_in]           # [1024, SC]
            aag_out8 = [t[:].bitcast(F8).rearrange("(c r) s -> c r s",
                                                   r=2 * HID_SH)
                        for t in aag_out]                          # [8,1024,SC]

            # payload hi/lo region views as [p, i, s] fp8
            def pay_hi(t, cb0):
                return t[0:HID_SH, cb0:cb0 + SC // 2].bitcast(F8) \
                    .rearrange("(i p) s -> p i s", p=P)

            def pay_lo(t, cb0):
                return t[0:HID_SH, SH // 2 + cb0:SH // 2 + cb0 + SC // 2] \
                    .bitcast(F8).rearrange("(i p) s -> p i s", p=P)

            # =========== era A pools (attention + residual stream) ===========
            a_es = contextlib.ExitStack()
            apers = a_es.enter_context(tc.tile_pool(name="apers", bufs=1))
            awork = a_es.enter_context(tc.tile_pool(name="awork", bufs=1))
            owork = a_es.enter_context(tc.tile_pool(name="owork", bufs=1))

            _cnt = [0]

            def wtile(pool, shape, dt, tag, bufs):
                _cnt[0] += 1
                return pool.tile(shape, dt, tag=tag, bufs=bufs,
                                 name=f"t_{_cnt[0]}")

            # ---- phase 1: x = h + r; hi/lo fp8 + ssq into payload ----
            with tc.tile_pool(name="p1", bufs=1) as p1:
                for sc in range(NSC):
                    cs = slice(sc * SC, (sc + 1) * SC)
                    ps = rowps.tile([1, SC], F32, tag="row", name=f"ssq1p{sc}")
                    hf = wtile(p1, [P, NT_HSH, SC], BF16, "hf", 2)
                    rf = wtile(p1, [P, NT_HSH, SC], BF16, "rf", 2)
                    nc.sync.dma_start(hf[:], hT_v[:, :, cs])
                    nc.sync.dma_start(rf[:], rT_v[:, :, cs])
                    xt = wtile(p1, [P, NT_HSH, SC], F32, "xt", 2)
                    nc.vector.tensor_tensor(xt[:], hf[:], rf[:], ALU.add)
                    xhi = wtile(p1, [P, NT_HSH, SC], F8, "xhi", 2)
                    nc.vector.tensor_copy(xhi[:], xt[:])
                    xlo = wtile(p1, [P, NT_HSH, SC], F8, "xlo", 2)
                    nc.vector.tensor_tensor(xlo[:], xt[:], xhi[:], ALU.subtract)
                    nc.scalar.dma_start(pay1_hi(ag1_in[sc]), xhi[:])
                    nc.sync.dma_start(pay1_lo(ag1_in[sc]), xlo[:])
                    sq = wtile(p1, [P, NT_HSH, SC], BF16, "sq", 2)
                    nc.scalar.activation(sq[:], xt[:], AF.Square)
                    for i in range(NT_HSH):
                        nc.tensor.matmul(ps[:], ones_bf[:], sq[:, i, :],
                                         start=(i == 0), stop=(i == NT_HSH - 1))
                    ssq1b = awork.tile([1, SC], BF16, tag="ssq1b", bufs=2,
                                       name=f"ssq1b{sc}")
                    nc.vector.tensor_copy(ssq1b[:], ps[:])
                    nc.sync.dma_start(
                        ag1_in[sc][HID_SH:HID_SH + 1, 0:SC], ssq1b[:])
                    nc.gpsimd.collective_compute(
                        "AllGather", ALU.bypass, replica_groups=RG,
                        ins=[ag1_in[sc][:].opt()], outs=[ag1_out[sc][:].opt()])

            # ---- rope tables + masks (after AGs so phase-1 wins queues) ----
            ident = apers.tile([P, P], BF16)
            make_identity(nc, ident[:])
            cos2 = apers.tile([P, S], BF16)
            sin_neg = apers.tile([P, S], BF16)
            cmask = []
            for j in range(SC // P):
                mk = apers.tile([P, SC], BF16, name=f"cmask{j}")
                nc.vector.memset(mk[:], 1.0)
                nc.gpsimd.affine_select(mk[:], mk[:], pattern=[[1, SC]],
                                        base=-j * P, channel_multiplier=-1,
                                        compare_op=ALU.is_ge, fill=0.0)
                cmask.append(mk)

            nc.sync.dma_start(cos2[:], cos_in[:])
            nc.sync.dma_start(sin_neg[:], sin_in[:])

            # fp8 identity * SWO for the fused residual add in o_proj
            sw_id = apers.tile([P, 2, P], F8, name="sw_id")
            nc.scalar.activation(sw_id[:, 0, :], ident[:], AF.Copy, scale=SWO)
            nc.scalar.activation(sw_id[:, 1, :], ident[:], AF.Copy, scale=SWO)

            # ---- persistent attention-era tiles ----
            kT = apers.tile([P, S], BF16, name="kT")
            vT = apers.tile([P, S], BF16, name="vT")
            s1b = apers.tile([P, S], BF16, name="s1b")

            # wo SBUF cache: created late, in the region wqp frees
            wo_es = contextlib.ExitStack()

            # wq SBUF cache + gathered-x tiles (freed after qkv3)
            wq_es = contextlib.ExitStack()
            wqp = wq_es.enter_context(tc.tile_pool(name="wqp", bufs=1))
            wq_sb_hi = [wqp.tile([P, NPH, 2, P], F8, name=f"wqh{j}") for j in range(NJ)]
            wq_sb_lo = [wqp.tile([P, NPH, 2, P], F8, name=f"wql{j}") for j in range(NJ)]
            for j in range(NJ):
                nc.scalar.dma_start(wq_sb_hi[j][:], wslice(wq_hi, j))
            for j in range(NJ):
                nc.scalar.dma_start(wq_sb_lo[j][:], wslice(wq_lo, j))

            def emit_s1b(sc):
                cs = slice(sc * SC, (sc + 1) * SC)
                srows_b = awork.tile([8, SC], BF16, tag="srb", bufs=1,
                                     name=f"sr1b{sc}")
                nc.gpsimd.dma_start(srows_b[:], ag1_v[sc][:, HID_SH, 0:SC])
                srows = awork.tile([8, SC], F32, tag="srf", bufs=1,
                                   name=f"sr1f{sc}")
                nc.vector.tensor_copy(srows[:], srows_b[:])
                ssum = awork.tile([8, SC], F32, tag="ssum", bufs=1,
                                  name=f"ss1{sc}")
                nc.gpsimd.partition_all_reduce(ssum[:], srows[:], channels=8,
                                               reduce_op=ReduceOp.add)
                var = awork.tile([1, SC], F32, tag="var", bufs=1, name=f"v1{sc}")
                nc.scalar.activation(var[:], ssum[:1, :], AF.Copy,
                                     scale=1.0 / HID, bias=EPS)
                nc.vector.reciprocal(var[:], var[:])
                varb = awork.tile([1, SC], BF16, tag="varb", bufs=1,
                                  name=f"v1b{sc}")
                nc.scalar.activation(varb[:], var[:], AF.Sqrt, scale=CE1 * CE1)
                nc.gpsimd.partition_broadcast(s1b[:, cs], varb[:])

            qcs = {}
            gqs = {}

            def load_gq(sc):
                ghi, glo = [], []
                for cb in range(N_CORES):
                    g = wqp.tile([P, 2, 2, SC], F8, tag="ghi", bufs=16,
                                 name=f"gh{cb}_{sc}")
                    nc.gpsimd.dma_start(
                        g[:], ag1_8[sc][cb, 0:HID_SH, 0:SC]
                        .rearrange("(tp two p) n -> p tp two n", tp=2, two=2))
                    ghi.append(g)
                    g = wqp.tile([P, 2, 2, SC], F8, tag="glo", bufs=16,
                                 name=f"gl{cb}_{sc}")
                    nc.sync.dma_start(
                        g[:], ag1_8[sc][cb, 0:HID_SH, SC:2 * SC]
                        .rearrange("(tp two p) n -> p tp two n", tp=2, two=2))
                    glo.append(g)
                gqs[sc] = (ghi, glo)

            def emit_qkv(sc):
                cs = slice(sc * SC, (sc + 1) * SC)
                if sc not in gqs:
                    load_gq(sc)
                ghi, glo = gqs[sc]

                def rhs(lst, g):
                    return lst[g // 2][:, g % 2]

                qc = {}
                pss = [acc.tile([P, SC], F32, tag="acc", name=f"qk{j}_{sc}")
                       for j in range(NJ)]
                for g in range(NPH):
                    for j in range(NJ):
                        nc.tensor.matmul(pss[j][:], wq_sb_hi[j][:, g],
                                         rhs(ghi, g), start=(g == 0),
                                         stop=False, perf_mode=DR)
                for g in range(NPH):
                    for j in range(NJ):
                        nc.tensor.matmul(pss[j][:], wq_sb_hi[j][:, g],
                                         rhs(glo, g), start=False, stop=False,
                                         perf_mode=DR)
                for g in range(NPH):
                    for j in range(NJ):
                        nc.tensor.matmul(pss[j][:], wq_sb_lo[j][:, g],
                                         rhs(ghi, g), start=False,
                                         stop=(g == NPH - 1), perf_mode=DR)
                for j in range(NJ):
                    if j < HQ:
                        dst = awork.tile([P, SC], BF16, tag="qc", bufs=8,
                                         name=f"qc{j}_{sc}")
                        qc[j] = dst
                        nc.vector.tensor_tensor(dst[:], pss[j][:], s1b[:, cs],
                                                ALU.mult)
                    else:
                        dst = kT if j == HQ else vT
                        nc.vector.tensor_tensor(dst[:, cs], pss[j][:],
                                                s1b[:, cs], ALU.mult)
                qcs[sc] = qc

            def emit_attn(sc):
                cs = slice(sc * SC, (sc + 1) * SC)
                qc = qcs[sc]
                # rope on q tiles + k chunk
                for j in range(HQ + 1):
                    tv = qc[j][:] if j < HQ else kT[:, cs]
                    swp = wtile(awork, [P, SC], BF16, "t1k", 3)
                    nc.sync.dma_start(swp[:64, :], tv[64:, :])
                    nc.sync.dma_start(swp[64:, :], tv[:64, :])
                    m1 = wtile(awork, [P, SC], BF16, "t1k", 3)
                    nc.vector.tensor_tensor(m1[:], tv, cos2[:, cs], ALU.mult)
                    m2 = wtile(awork, [P, SC], BF16, "t1k", 3)
                    nc.vector.tensor_tensor(m2[:], swp[:], sin_neg[:, cs], ALU.mult)
                    nc.vector.tensor_tensor(tv, m1[:], m2[:], ALU.add)

                # v transpose in place
                for t in range(sc * (SC // P), (sc + 1) * (SC // P)):
                    pst = tpps.tile([P, P], BF16, tag="tp", name=f"tp{t}")
                    nc.tensor.transpose(pst[:], vT[:, t * P:(t + 1) * P], ident[:])
                    nc.vector.tensor_copy(vT[:, t * P:(t + 1) * P], pst[:])

                # attention: 4 heads x this chunk; fp8 hi/lo attn output
                nsk = (sc + 1) * (SC // P)
                ahi = awork.tile([P, HQ, SC], F8, tag="ahi", bufs=1,
                                 name=f"ahi{sc}")
                alo = awork.tile([P, HQ, SC], F8, tag="alo", bufs=1,
                                 name=f"alo{sc}")
                for h in range(HQ):
                    pv = acc.tile([P, SC], F32, tag="acc", name=f"pv{h}_{sc}")
                    rs = rowps.tile([1, SC], F32, tag="row", name=f"rs{h}_{sc}")
                    for skt in range(nsk):
                        sps = acc.tile([P, SC], F32, tag="acc",
                                       name=f"s{h}_{sc}_{skt}")
                        nc.tensor.matmul(sps[:], kT[:, skt * P:(skt + 1) * P],
                                         qc[h][:], start=True, stop=True)
                        ex = wtile(awork, [P, SC], BF16, "ex", 3)
                        nc.scalar.activation(ex[:], sps[:], AF.Exp,
                                             scale=inv_sqrt_d)
                        if skt >= 4 * sc:
                            nc.vector.tensor_tensor(ex[:], ex[:],
                                                    cmask[skt - 4 * sc][:],
                                                    ALU.mult)
                        nc.tensor.matmul(rs[:], ones_bf[:], ex[:],
                                         start=(skt == 0), stop=(skt == nsk - 1))
                        nc.tensor.matmul(pv[:], vT[:, skt * P:(skt + 1) * P],
                                         ex[:], start=(skt == 0),
                                         stop=(skt == nsk - 1))
                    rcp = awork.tile([1, SC], F32, tag="rcp", bufs=1,
                                     name=f"rcp{h}_{sc}")
                    nc.vector.reciprocal(rcp[:], rs[:])
                    rcpb = wtile(awork, [P, SC], F32, "rcpb", 1)
                    nc.gpsimd.partition_broadcast(rcpb[:], rcp[:])
                    a32 = wtile(awork, [P, SC], F32, "a32", 1)
                    nc.vector.tensor_tensor(a32[:], pv[:], rcpb[:], ALU.mult)
                    nc.vector.tensor_copy(ahi[:, h, :], a32[:])
                    nc.vector.tensor_tensor(alo[:, h, :], a32[:], ahi[:, h, :],
                                            ALU.subtract)

                # publish fp8 attn out + AllGather for column-sharded o_proj
                nc.scalar.dma_start(
                    aag_in8[sc][0:HID_SH, :]
                    .rearrange("(h p) s -> p h s", p=P), ahi[:])
                nc.sync.dma_start(
                    aag_in8[sc][HID_SH:2 * HID_SH, :]
                    .rearrange("(h p) s -> p h s", p=P), alo[:])
                nc.gpsimd.collective_compute(
                    "AllGather", ALU.bypass, replica_groups=RG,
                    ins=[aag_in[sc][:].opt()], outs=[aag_out[sc][:].opt()])

            def emit_o(sc):
                """Column-sharded o_proj for chunk sc with fused residual2."""
                cs = slice(sc * SC, (sc + 1) * SC)
                hh = sc // 2
                cb0 = (sc % 2) * (SC // 2)
                # x (residual stream) hi/lo pair for the fused add
                xp = owork.tile([P, NT_HSH, 2, SC], F8, tag="xp", bufs=1,
                                name=f"xp{sc}")
                nc.scalar.dma_start(xp[:, :, 0, :], pay1_hi(ag1_in[sc]))
                nc.sync.dma_start(xp[:, :, 1, :], pay1_lo(ag1_in[sc]))

                r2f = owork.tile([P, NT_HSH, SC], F32, tag="r2f", bufs=1,
                                 name=f"r2f{sc}")
                r2hi = owork.tile([P, NT_HSH, SC], F8, tag="r2hi", bufs=1,
                                  name=f"r2hi{sc}")
                r2lo = owork.tile([P, NT_HSH, SC], F8, tag="r2lo", bufs=1,
                                  name=f"r2lo{sc}")
                sq = owork.tile([P, NT_HSH, SC], BF16, tag="r2sq", bufs=1,
                                name=f"r2sq{sc}")
                ps2 = rowps.tile([1, SC], F32, tag="row", name=f"ssq2_{sc}")
                pss = [acc.tile([P, SC], F32, tag="acc", name=f"o{m}_{sc}")
                       for m in range(NT_HSH)]
                for g in range(NPH):
                    cb, j = g // 2, g % 2
                    ghi = owork.tile([P, 2, SC], F8, tag="oghi", bufs=4,
                                     name=f"ogh{g}_{sc}")
                    nc.scalar.dma_start(
                        ghi[:], aag_out8[sc][cb, j * 256:(j + 1) * 256, :]
                        .rearrange("(two p) s -> p two s", p=P))
                    glo = owork.tile([P, 2, SC], F8, tag="oglo", bufs=4,
                                     name=f"ogl{g}_{sc}")
                    nc.sync.dma_start(
                        glo[:], aag_out8[sc][cb, HID_SH + j * 256:
                                             HID_SH + (j + 1) * 256, :]
                        .rearrange("(two p) s -> p two s", p=P))
                    for m in range(NT_HSH):
                        nc.tensor.matmul(pss[m][:],
                                         wo_sb_hi[:, g, :, m * P:(m + 1) * P],
                                         ghi[:], start=(g == 0), stop=False,
                                         perf_mode=DR)
                    for m in range(NT_HSH):
                        nc.tensor.matmul(pss[m][:],
                                         wo_sb_hi[:, g, :, m * P:(m + 1) * P],
                                         glo[:], start=False, stop=False,
                                         perf_mode=DR)
                    for m in range(NT_HSH):
                        nc.tensor.matmul(pss[m][:],
                                         wo_sb_lo[:, g, :, m * P:(m + 1) * P],
                                         ghi[:], start=False, stop=False,
                                         perf_mode=DR)
                for m in range(NT_HSH):
                    # += SWO * (xhi + xlo): fused residual add
                    nc.tensor.matmul(pss[m][:], sw_id[:], xp[:, m],
                                     start=False, stop=True, perf_mode=DR)
                    nc.scalar.activation(r2f[:, m, :], pss[m][:], AF.Copy,
                                         scale=CO)
                    nc.vector.tensor_copy(r2hi[:, m, :], r2f[:, m, :])
                    nc.vector.tensor_tensor(r2lo[:, m, :], r2f[:, m, :],
                                            r2hi[:, m, :], ALU.subtract)
                    nc.scalar.activation(sq[:, m, :], r2f[:, m, :], AF.Square)
                    nc.tensor.matmul(ps2[:], ones_bf[:], sq[:, m, :],
                                     start=(m == 0), stop=(m == NT_HSH - 1))
                nc.gpsimd.dma_start(res2_v[:, :, cs], r2f[:])
                nc.gpsimd.dma_start(pay_hi(ag2_in[hh], cb0), r2hi[:])
                nc.gpsimd.dma_start(pay_lo(ag2_in[hh], cb0), r2lo[:])
                ssq2 = owork.tile([1, SC], BF16, tag="ssq2", bufs=2,
                                  name=f"sq2_{sc}")
                nc.vector.tensor_copy(ssq2[:], ps2[:])
                nc.gpsimd.dma_start(
                    ag2_in[hh][HID_SH:HID_SH + 1,
                               (sc % 2) * SC:(sc % 2) * SC + SC], ssq2[:])

            def emit_ag2(hh):
                nc.gpsimd.collective_compute(
                    "AllGather", ALU.bypass, replica_groups=RG,
                    ins=[ag2_in[hh][:].opt()], outs=[ag2_out[hh][:].opt()])

            # =========== era B pools (MLP) ===========
            # created mid-schedule (pool creation reserves SBUF immediately);
            # closures below bind these names at call time.
            b_es = contextlib.ExitStack()
            mpers = mwork = mstr = None
            s2b = None

            def emit_s2b(sc):
                cs = slice(sc * SC, (sc + 1) * SC)
                hh = sc // 2
                hcs = slice((sc % 2) * SC, (sc % 2) * SC + SC)
                srows_b = mwork.tile([8, SC], BF16, tag="srb2", bufs=1,
                                     name=f"sr2b{sc}")
                nc.gpsimd.dma_start(srows_b[:], ag2_v[hh][:, HID_SH, hcs])
                srows = mwork.tile([8, SC], F32, tag="srf2", bufs=1,
                                   name=f"sr2f{sc}")
                nc.vector.tensor_copy(srows[:], srows_b[:])
                ssum = mwork.tile([8, SC], F32, tag="ssum2", bufs=1,
                                  name=f"ss2{sc}")
                nc.gpsimd.partition_all_reduce(ssum[:], srows[:], channels=8,
                                               reduce_op=ReduceOp.add)
                var = mwork.tile([1, SC], F32, tag="var2", bufs=2, name=f"v2{sc}")
                nc.scalar.activation(var[:], ssum[:1, :], AF.Copy,
                                     scale=1.0 / (HID * CD), bias=EPS / CD)
                nc.vector.reciprocal(var[:], var[:])     # = CD / var
                varb2 = mwork.tile([1, SC], BF16, tag="vb2", bufs=2,
                                   name=f"vb2{sc}")
                nc.vector.tensor_copy(varb2[:], var[:])
                nc.gpsimd.partition_broadcast(s2b[:, cs], varb2[:])

            g2pool = {}

            def load_g2(half):
                g2p = g2pool["p"]
                ghi, glo = [], []
                for cb in range(N_CORES):
                    g = g2p.tile([P, 2, 2, SH], F8, tag="g2h", bufs=8,
                                 name=f"g2h{cb}_{half}")
                    nc.gpsimd.dma_start(
                        g[:], ag2_8[half][cb, 0:HID_SH, 0:SH]
                        .rearrange("(tp two p) n -> p tp two n", tp=2, two=2))
                    ghi.append(g)
                    g = g2p.tile([P, 2, 2, SH], F8, tag="g2l", bufs=8,
                                 name=f"g2l{cb}_{half}")
                    nc.sync.dma_start(
                        g[:], ag2_8[half][cb, 0:HID_SH, SH:2 * SH]
                        .rearrange("(tp two p) n -> p tp two n", tp=2, two=2))
                    glo.append(g)
                return ghi, glo


            g2 = {}

            # ============ era A schedule ============
            load_gq(0)
            emit_s1b(0)
            emit_qkv(0)
            load_gq(1)
            emit_s1b(1)
            emit_qkv(1)
            load_gq(2)
            emit_attn(0)
            emit_s1b(2)
            emit_qkv(2)
            load_gq(3)
            emit_attn(1)
            emit_s1b(3)
            emit_qkv(3)
            wq_es.close()
            mpers = b_es.enter_context(tc.tile_pool(name="mpers", bufs=1, side="right"))
            mwork = b_es.enter_context(tc.tile_pool(name="mwork", bufs=1, side="right"))
            wop = wo_es.enter_context(tc.tile_pool(name="wop", bufs=1))
            wo_sb_hi = wop.tile([P, NPH, 2, HID_SH], F8, name="wo_h")
            wo_sb_lo = wop.tile([P, NPH, 2, HID_SH], F8, name="wo_l")
            nc.gpsimd.dma_start(wo_sb_hi[:], wslice(wo_hi, 0, m=HID_SH))
            nc.gpsimd.dma_start(wo_sb_lo[:], wslice(wo_lo, 0, m=HID_SH))
            s2b = mpers.tile([P, S], BF16, name="s2b")
            g2pool["p"] = b_es.enter_context(tc.tile_pool(name="g2p", bufs=1, side="right"))
            emit_attn(2)
            emit_o(0)
            emit_o(1)
            emit_ag2(0)
            emit_attn(3)
            emit_s2b(0)
            emit_s2b(1)
            g2[0] = load_g2(0)
            emit_o(2)
            emit_o(3)
            emit_ag2(1)
            g2[1] = load_g2(1)
            emit_s2b(2)
            emit_s2b(3)
            wo_es.close()
            a_es.close()

            mstr = b_es.enter_context(tc.tile_pool(name="mstr", bufs=1, side="right"))
            utp = b_es.enter_context(tc.tile_pool(name="utp", bufs=1, side="right"))
            ut_hi = [utp.tile([P, 2, S], F8, name=f"uth{g}") for g in range(NPI)]
            ut_lo = [utp.tile([P, 2, S], F8, name=f"utl{g}") for g in range(NPI)]

            def emit_up(half, it_range):
                ghi, glo = g2[half]

                def rhs(lst, g, ncs):
                    return lst[g // 2][:, g % 2, :, ncs]

                for it in it_range:
                    wh = mstr.tile([P, NPH, 2, P], F8, tag="wuh", bufs=2,
                                   name=f"wuh{it}_{half}")
                    nc.scalar.dma_start(wh[:], wslice(wu_hi, it))
                    wl = mstr.tile([P, NPH, 2, P], F8, tag="wul", bufs=2,
                                   name=f"wul{it}_{half}")
                    nc.scalar.dma_start(wl[:], wslice(wu_lo, it))
                    pss = [acc.tile([P, SC], F32, tag="acc", name=f"up{it}_{2*half+ci}")
                           for ci in range(2)]
                    for g in range(NPH):
                        for ci in range(2):
                            nc.tensor.matmul(pss[ci][:], wh[:, g],
                                             rhs(ghi, g, slice(ci * SC, (ci + 1) * SC)),
                                             start=(g == 0), stop=False,
                                             perf_mode=DR)
                    for g in range(NPH):
                        for ci in range(2):
                            nc.tensor.matmul(pss[ci][:], wh[:, g],
                                             rhs(glo, g, slice(ci * SC, (ci + 1) * SC)),
                                             start=False, stop=False, perf_mode=DR)
                    for g in range(NPH):
                        for ci in range(2):
                            nc.tensor.matmul(pss[ci][:], wl[:, g],
                                             rhs(ghi, g, slice(ci * SC, (ci + 1) * SC)),
                                             start=False, stop=(g == NPH - 1),
                                             perf_mode=DR)
                    for ci in range(2):
                        sc = 2 * half + ci
                        cs = slice(sc * SC, (sc + 1) * SC)
                        rl = mwork.tile([P, SC], F32, tag="rl", bufs=2,
                                        name=f"rl{it}_{sc}")
                        nc.scalar.activation(rl[:], pss[ci][:], AF.Relu,
                                             scale=ALPHA_UP)
                        nc.vector.tensor_tensor(rl[:], rl[:], rl[:], ALU.mult)
                        nc.vector.tensor_copy(ut_hi[it // 2][:, it % 2, cs], rl[:])
                        nc.vector.tensor_tensor(ut_lo[it // 2][:, it % 2, cs],
                                                rl[:], ut_hi[it // 2][:, it % 2, cs],
                                                ALU.subtract)

            # ---- era B schedule ----
            emit_up(0, range(NT_INT))
            emit_up(1, range(NT_INT))

            # ---- down proj: 3-term DoubleRow over full S, uneven RS ----
            mstart = 0
            for pi, mc in enumerate(PIECES):
                rs2t = rs2_in[pi]
                rs2_fat = rs2t[:].rearrange("(q p) s -> p q s", p=P)
                for mq in range(mc):
                    m = mstart + mq
                    wh = mstr.tile([P, NPI, 2, P], F8, tag="wdh", bufs=3,
                                   name=f"wdh{m}")
                    nc.scalar.dma_start(wh[:], wslice(wd_hi, m))
                    wl = mstr.tile([P, NPI, 2, P], F8, tag="wdl", bufs=3,
                                   name=f"wdl{m}")
                    nc.scalar.dma_start(wl[:], wslice(wd_lo, m))
                    evf = mwork.tile([P, NSC, SC], BF16, tag="dnev", bufs=1,
                                     name=f"dev{m}")
                    for sc in range(NSC):
                        cs = slice(sc * SC, (sc + 1) * SC)
                        ps = acc.tile([P, SC], F32, tag="acc",
                                      name=f"dn{m}_{sc}")
                        for g in range(NPI):
                            nc.tensor.matmul(ps[:], wh[:, g],
                                             ut_hi[g][:, :, cs],
                                             start=(g == 0), stop=False,
                                             perf_mode=DR)
                        for g in range(NPI):
                            nc.tensor.matmul(ps[:], wh[:, g],
                                             ut_lo[g][:, :, cs],
                                             start=False, stop=False,
                                             perf_mode=DR)
                        for g in range(NPI):
                            nc.tensor.matmul(ps[:], wl[:, g],
                                             ut_hi[g][:, :, cs],
                                             start=False,
                                             stop=(g == NPI - 1),
                                             perf_mode=DR)
                        nc.vector.tensor_tensor(evf[:, sc, :], ps[:],
                                                s2b[:, cs], ALU.mult)
                    nc.sync.dma_start(rs2_fat[:, mq], evf[:])
                nc.gpsimd.collective_compute(
                    "ReduceScatter", ALU.add, replica_groups=RG,
                    ins=[rs2t[:].opt()], outs=[rs2_out[pi][:].opt()])
                orow = mstart * P // N_CORES
                nc.gpsimd.dma_start(
                    out_mlp[orow:orow + mc * P // N_CORES, :], rs2_out[pi][:])
                mstart += mc
            b_es.close()

    nc.compile()
    return nc


def _q8_pair(x):
    x32 = np.asarray(x, np.float32)
    hi = np.asarray(np.clip(x32, -240, 240), F8NP)
    lo = np.asarray(np.clip(x32 - hi.astype(np.float32), -240, 240), F8NP)
    return np.ascontiguousarray(hi), np.ascontiguousarray(lo)


def _lay_tiles(w, mt=P):
    """[K, M] fp8 -> [(M//mt)*128, (K//256)*2*mt] in SBUF tile order.

    Row = mtile*128 + p; cols = (kpair, two, m) flattened, so each per-tile
    DMA is one contiguous [128, (K//256)*2*mt] block."""
    K, M = w.shape
    a = w.reshape(K // 256, 2, P, M // mt, mt).transpose(3, 2, 0, 1, 4)
    return np.ascontiguousarray(a.reshape(M // mt * P, (K // 256) * 2 * mt))


def shard_inputs(positions, hidden_states, residual, qkv_w, o_w, up_w, down_w,
                 ln1_w, ln2_w):
    hTf = np.ascontiguousarray(
        np.asarray(hidden_states).reshape(S, HID).T.astype(ml_dtypes.bfloat16))
    rTf = np.ascontiguousarray(
        np.asarray(residual).reshape(S, HID).T.astype(ml_dtypes.bfloat16))
    pos = np.asarray(positions).reshape(S).astype(np.float64)
    inv = 1.0 / (THETA ** (np.arange(0, DHEAD, 2, dtype=np.float64) / DHEAD))
    fr = pos[:, None] * inv                      # [S, 64]
    cost = np.cos(fr).T.astype(np.float32)       # [64, S]
    sint = np.sin(fr).T.astype(np.float32)
    cos2 = np.ascontiguousarray(
        np.concatenate([cost, cost], 0).astype(ml_dtypes.bfloat16))
    sin_neg = np.ascontiguousarray(
        np.concatenate([-sint, sint], 0).astype(ml_dtypes.bfloat16))
    q_size = N_HEADS * DHEAD
    kv = N_KV * DHEAD
    w1 = np.asarray(qkv_w, np.float32) * np.asarray(ln1_w, np.float32)[:, None] * SW1
    wof = np.asarray(o_w, np.float32) * SWO
    wuf = np.asarray(up_w, np.float32) * np.asarray(ln2_w, np.float32)[:, None] * SWU
    wdf = np.asarray(down_w, np.float32) * SWD
    in_maps = []
    for c in range(N_CORES):
        wq_c = np.concatenate([
            w1[:, c * HQ * DHEAD:(c + 1) * HQ * DHEAD],
            w1[:, q_size + c * DHEAD:q_size + (c + 1) * DHEAD],
            w1[:, q_size + kv + c * DHEAD:q_size + kv + (c + 1) * DHEAD],
        ], axis=1)
        wq_h, wq_l = _q8_pair(wq_c)
        wo_h, wo_l = _q8_pair(wof[:, c * HID_SH:(c + 1) * HID_SH])
        wu_h, wu_l = _q8_pair(wuf[:, c * INT_SH:(c + 1) * INT_SH])
        wd_h, wd_l = _q8_pair(wdf[c * INT_SH:(c + 1) * INT_SH, :])
        wq_h, wq_l = _lay_tiles(wq_h), _lay_tiles(wq_l)
        wo_h, wo_l = _lay_tiles(wo_h, mt=HID_SH), _lay_tiles(wo_l, mt=HID_SH)
        wu_h, wu_l = _lay_tiles(wu_h), _lay_tiles(wu_l)
        wd_h, wd_l = _lay_tiles(wd_h), _lay_tiles(wd_l)
        in_maps.append({
            "hT": np.ascontiguousarray(hTf[c * HID_SH:(c + 1) * HID_SH]),
            "rT": np.ascontiguousarray(rTf[c * HID_SH:(c + 1) * HID_SH]),
            "cos2": cos2, "sin_neg": sin_neg,
            "wq_hi": wq_h, "wq_lo": wq_l,
            "wo_hi": wo_h, "wo_lo": wo_l,
            "wu_hi": wu_h, "wu_lo": wu_l,
            "wd_hi": wd_h, "wd_lo": wd_l,
        })
    return in_maps


_CACHE = {}


def kernel(**inputs):
    from concourse.bass_utils import run_bass_kernel_spmd
    if "nc" not in _CACHE:
        _CACHE["nc"] = build_graph()
    nc = _CACHE["nc"]
    in_maps = shard_inputs(**{k: np.asarray(v) for k, v in inputs.items()})
    res = run_bass_kernel_spmd(nc, in_maps, core_ids=list(range(N_CORES)),
                               trace=False)
    res2T = np.concatenate([res.results[c]["res2T"] for c in range(N_CORES)], axis=0)
    mlpT = np.empty((HID, S), np.float32)
    for c in range(N_CORES):
        mt = res.results[c]["mlpT"]
        mstart = 0
        for pi, mc in enumerate(PIECES):
            rows = mc * P // N_CORES          # rows per core for this piece
            orow = mstart * P // N_CORES
            g0 = mstart * P + c * rows        # global hid row start
            mlpT[g0:g0 + rows] = mt[orow:orow + rows]
            mstart += mc
    mlp_out = np.ascontiguousarray(mlpT.T).reshape(1, S, HID)
    residual2 = np.ascontiguousarray(res2T.T).reshape(1, S, HID)
    return mlp_out, residual2


# revision 36
# speedup vs baseline: 1.0147x; 1.0012x over previous
"""Arcee decoder layer on 8 TRN2 NeuronCores — TP8, fp8 hi/lo DoubleRow.

Sharding (8-way TP, transposed activation layout [hidden, seq] on device):
  - core c owns: q heads 4c..4c+3 + kv head c, residual rows 512c..512c+511,
    intermediate cols 2048c..2048c+2047.
  - Big GEMMs (qkv/o/up/down) run as 3-term hi/lo fp8e4m3 DoubleRow:
    W.x ~= Whi.xhi + Whi.xlo + Wlo.xhi, each term contracting 256 rows per
    0.5-cycle/row matmul. Weights pre-quantized on host (ln1/ln2 and
    per-tensor scales folded); activations split hi/lo on device.
  - RMSNorm: un-normalized residual stream AllGathered with per-core partial
    sum-of-squares row embedded; rsqrt scale folded into PSUM eviction.
  - AG payload [520, 1024] bf16 per S-half: rows 0-511 carry x rows as fp8
    bytes (hi in bf16 cols 0-511, lo in 512-1023), row 512 = bf16 ssq row.
  - o_proj is COLUMN-sharded: per-chunk fp8 hi/lo attention outputs are
    AllGathered (0.5 MiB/rank, much cheaper than the 4 MiB ReduceScatter of
    o partials), then each core computes the full contraction for its own
    512 hid rows. residual2 is fused into the o eviction via an extra
    DoubleRow matmul with a 128*I fp8 identity against the (xhi, xlo) pair,
    so res2 never round-trips through a collective.
  - down_proj partials reduce via bf16 ReduceScatter split into uneven
    pieces so the exposed tail is small.
  - attention (scores/softmax/PV) stays bf16. DMAs are batched into fat
    tiles and spread across queues to keep dispatch off the critical path.
"""
import sys

sys.path.insert(0, "/opt/trn_rl_repo")

import contextlib
import math
import numpy as np
import ml_dtypes

import concourse.bass as bass
import concourse.mybir as mybir
import concourse.tile as tile
from concourse import bacc
from concourse.bass_isa import ReduceOp
from concourse.masks import make_identity

F32 = mybir.dt.float32
BF16 = mybir.dt.bfloat16
F8 = mybir.dt.float8e4
F8E5 = mybir.dt.float8e5
I32 = mybir.dt.int32
AF = mybir.ActivationFunctionType
ALU = mybir.AluOpType
DR = mybir.MatmulPerfMode.DoubleRow
F8NP = ml_dtypes.float8_e4m3

N_CORES = 8
S = 2048
HID = 4096
N_HEADS = 32
N_KV = 8
DHEAD = 128
INTER = 16384
EPS = 1e-5
THETA = 10000.0

HQ = N_HEADS // N_CORES          # 4 q heads per core
HID_SH = HID // N_CORES          # 512 residual rows per core
INT_SH = INTER // N_CORES        # 2048 intermediate per core
NJ = HQ + 2                      # qkv col tiles per core (4q + k + v)
QKV_COLS = NJ * DHEAD            # 768
P = 128
SC = 512                         # seq chunk
NSC = S // SC                    # 4
SH = S // 2                      # 1024 (half)
NT_HID = HID // P                # 32
NT_HSH = HID_SH // P             # 4
NT_INT = INT_SH // P             # 16
NPH = NT_HID // 2                # 16 k-pairs over HID
NPI = NT_INT // 2                # 8 k-pairs over INT_SH
BLK = HID_SH + 8                 # 520 payload rows
TWO_PI = 2.0 * math.pi

# fp8 scales (activations unscaled; weights scaled on host)
SW1 = 1024.0
SWO = 128.0                      # must stay fp8-representable (identity add)
SWU = 1024.0
SWD = 1024.0
SQU = 0.25                       # scale on u = relu(z)^2
CE1 = 1.0 / SW1                  # qkv evict const (with rsqrt row)
CO = 1.0 / SWO                   # o evict const
ALPHA_UP = math.sqrt(SQU) / SWU  # relu evict scale
CD = 1.0 / (SWD * SQU)           # down evict const (with 1/var row)

# down RS pieces (m-tile counts; sum = 32); tapered so the tail is short.
PIECES = [8, 8, 7, 4, 2, 2, 1]

# softmax bias: probs stored as e5m2 exp(s*inv_sqrt_d - XC); max masked
# score*inv_sqrt_d is 12.47 and the min row-max is -4.03 for this input
# distribution, so XC=4 keeps exp in [3e-4, 4.8e3] — inside e5m2 range.
XC = 4.0


def build_graph():
    nc = bacc.Bacc(None, target_bir_lowering=False, debug=False)

    hT = nc.declare_dram_parameter("hT", [HID_SH, S], BF16, isOutput=False)
    rT = nc.declare_dram_parameter("rT", [HID_SH, S], BF16, isOutput=False)
    cos_in = nc.declare_dram_parameter("cos2", [P, S], BF16, isOutput=False)
    sin_in = nc.declare_dram_parameter("sin_neg", [P, S], BF16, isOutput=False)
    # weights arrive pre-laid-out in SBUF tile order (see _lay_tiles):
    # row = mtile*128 + p, cols = (t, two, m) flattened — every per-tile DMA
    # is a contiguous [128, X] block (full-width descriptors).
    wq_hi = nc.declare_dram_parameter("wq_hi", [NJ * P, NPH * 2 * P], F8, isOutput=False)
    wq_lo = nc.declare_dram_parameter("wq_lo", [NJ * P, NPH * 2 * P], F8, isOutput=False)
    wo_hi = nc.declare_dram_parameter("wo_hi", [P, NPH * 2 * HID_SH], F8, isOutput=False)
    wo_lo = nc.declare_dram_parameter("wo_lo", [P, NPH * 2 * HID_SH], F8, isOutput=False)
    wu_hi = nc.declare_dram_parameter("wu_hi", [NT_INT * P, NPH * 2 * P], F8, isOutput=False)
    wu_lo = nc.declare_dram_parameter("wu_lo", [NT_INT * P, NPH * 2 * P], F8, isOutput=False)
    wd_hi = nc.declare_dram_parameter("wd_hi", [NT_HID * P, NPI * 2 * P], F8, isOutput=False)
    wd_lo = nc.declare_dram_parameter("wd_lo", [NT_HID * P, NPI * 2 * P], F8, isOutput=False)
    out_res2 = nc.declare_dram_parameter("res2T", [HID_SH, S], F32, isOutput=True)
    out_mlp = nc.declare_dram_parameter("mlpT", [HID_SH, S], F32, isOutput=True)

    RG = [list(range(N_CORES))]
    inv_sqrt_d = 1.0 / math.sqrt(DHEAD)

    # per-tile contiguous weight slices -> [p, t(pair), two, m]
    def wslice(w, i, m=P):
        return w[i * P:(i + 1) * P, :].rearrange("p (t two m) -> p t two m",
                                                 two=2, m=m)

    hT_v = hT[:].rearrange("(i p) s -> p i s", p=P)
    rT_v = rT[:].rearrange("(i p) s -> p i s", p=P)
    res2_v = out_res2[:].rearrange("(i p) s -> p i s", p=P)

    with tile.TileContext(nc) as tc:
        with contextlib.ExitStack() as ctx:
            const = ctx.enter_context(tc.tile_pool(name="const", bufs=1))
            acc = ctx.enter_context(tc.tile_pool(name="acc", bufs=6, space="PSUM"))
            rowps = ctx.enter_context(tc.tile_pool(name="rowps", bufs=1, space="PSUM"))
            tpps = ctx.enter_context(tc.tile_pool(name="tpps", bufs=1, space="PSUM"))
            dram = ctx.enter_context(tc.tile_pool(name="dram", bufs=1, space="DRAM"))

            ones_bf = const.tile([P, 1], BF16)
            nc.vector.memset(ones_bf[:], 1.0)
            # DR lhsT needs pair-step %16==0, so the rowsum ones tile is
            # [P, 2, 16] (rows 0-15 of the result all carry the same sum)
            ones_f5p = const.tile([P, 2, 16], F8E5)
            nc.vector.memset(ones_f5p[:], 1.0)
            negxc = const.tile([P, 1], F32)
            nc.vector.memset(negxc[:], -XC)

            # DRAM scratch
            ag1_in = [dram.tile([BLK, SC], BF16, name=f"ag1_in{s_}")
                      for s_ in range(NSC)]
            ag1_out = [dram.tile([N_CORES * BLK, SC], BF16, name=f"ag1_out{s_}",
                                 addr_space="Shared") for s_ in range(NSC)]
            ag2_in = [dram.tile([BLK, SH], BF16, name=f"ag2_in{h}") for h in range(2)]
            ag2_out = [dram.tile([N_CORES * BLK, SH], BF16, name=f"ag2_out{h}",
                                 addr_space="Shared") for h in range(2)]
            # per-chunk attention-out AG: fp8 hi (rows 0-511) + lo (512-1023)
            aag_in = [dram.tile([2 * HID_SH, SC // 2], BF16, name=f"aag_in{sc}")
                      for sc in range(NSC)]
            aag_out = [dram.tile([N_CORES * 2 * HID_SH, SC // 2], BF16,
                                 name=f"aag_out{sc}", addr_space="Shared")
                       for sc in range(NSC)]
            rs2_in = [dram.tile([mc * P, S], BF16, name=f"rs2_in{pi}")
                      for pi, mc in enumerate(PIECES)]
            rs2_out = [dram.tile([mc * P // N_CORES, S], BF16,
                                 name=f"rs2_out{pi}")
                       for pi, mc in enumerate(PIECES)]

            ag1_v = [t[:].rearrange("(c r) s -> c r s", r=BLK) for t in ag1_out]
            ag2_v = [t[:].rearrange("(c r) s -> c r s", r=BLK) for t in ag2_out]
            ag1_8 = [t[:].bitcast(F8).rearrange("(c r) s -> c r s", r=BLK)
                     for t in ag1_out]
            ag2_8 = [t[:].bitcast(F8).rearrange("(c r) s -> c r s", r=BLK)
                     for t in ag2_out]

            # per-chunk ag1 payload regions (hi fp8 | lo fp8, + ssq row)
            def pay1_hi(t):
                return t[0:HID_SH, 0:SC // 2].bitcast(F8) \
                    .rearrange("(i p) s -> p i s", p=P)

            def pay1_lo(t):
                return t[0:HID_SH, SC // 2:SC].bitcast(F8) \
                    .rearrange("(i p) s -> p i s", p=P)
            aag_in8 = [t[:].bitcast(F8) for t in aag_in]           # [1024, SC]
            aag_out8 = [t[:].bitcast(F8).rearrange("(c r) s -> c r s",
                                                   r=2 * HID_SH)
                        for t in aag_out]                          # [8,1024,SC]

            # payload hi/lo region views as [p, i, s] fp8
            def pay_hi(t, cb0):
                return t[0:HID_SH, cb0:cb0 + SC // 2].bitcast(F8) \
                    .rearrange("(i p) s -> p i s", p=P)

            def pay_lo(t, cb0):
                return t[0:HID_SH, SH // 2 + cb0:SH // 2 + cb0 + SC // 2] \
                    .bitcast(F8).rearrange("(i p) s -> p i s", p=P)

            # =========== era A pools (attention + residual stream) ===========
            a_es = contextlib.ExitStack()
            apers = a_es.enter_context(tc.tile_pool(name="apers", bufs=1))
            awork = a_es.enter_context(tc.tile_pool(name="awork", bufs=1))
            owork = a_es.enter_context(tc.tile_pool(name="owork", bufs=1))

            _cnt = [0]

            def wtile(pool, shape, dt, tag, bufs):
                _cnt[0] += 1
                return pool.tile(shape, dt, tag=tag, bufs=bufs,
                                 name=f"t_{_cnt[0]}")

            # ---- phase 1: x = h + r; hi/lo fp8 + ssq into payload ----
            with tc.tile_pool(name="p1", bufs=1) as p1:
                for sc in range(NSC):
                    cs = slice(sc * SC, (sc + 1) * SC)
                    ps = rowps.tile([1, SC], F32, tag="row", name=f"ssq1p{sc}")
                    hf = wtile(p1, [P, NT_HSH, SC], BF16, "hf", 2)
                    rf = wtile(p1, [P, NT_HSH, SC], BF16, "rf", 2)
                    nc.sync.dma_start(hf[:], hT_v[:, :, cs])
                    nc.sync.dma_start(rf[:], rT_v[:, :, cs])
                    xt = wtile(p1, [P, NT_HSH, SC], F32, "xt", 2)
                    nc.vector.tensor_tensor(xt[:], hf[:], rf[:], ALU.add)
                    xhi = wtile(p1, [P, NT_HSH, SC], F8, "xhi", 2)
                    nc.vector.tensor_copy(xhi[:], xt[:])
                    xlo = wtile(p1, [P, NT_HSH, SC], F8, "xlo", 2)
                    nc.vector.tensor_tensor(xlo[:], xt[:], xhi[:], ALU.subtract)
                    nc.gpsimd.dma_start(pay1_hi(ag1_in[sc]), xhi[:])
                    nc.sync.dma_start(pay1_lo(ag1_in[sc]), xlo[:])
                    sq = wtile(p1, [P, NT_HSH, SC], BF16, "sq", 2)
                    nc.scalar.activation(sq[:], xt[:], AF.Square)
                    for i in range(NT_HSH):
                        nc.tensor.matmul(ps[:], ones_bf[:], sq[:, i, :],
                                         start=(i == 0), stop=(i == NT_HSH - 1))
                    ssq1b = awork.tile([1, SC], BF16, tag="ssq1b", bufs=2,
                                       name=f"ssq1b{sc}")
                    nc.vector.tensor_copy(ssq1b[:], ps[:])
                    nc.sync.dma_start(
                        ag1_in[sc][HID_SH:HID_SH + 1, 0:SC], ssq1b[:])
                    nc.gpsimd.collective_compute(
                        "AllGather", ALU.bypass, replica_groups=RG,
                        ins=[ag1_in[sc][:].opt()], outs=[ag1_out[sc][:].opt()])

            # ---- rope tables + masks (after AGs so phase-1 wins queues) ----
            ident = apers.tile([P, P], BF16)
            make_identity(nc, ident[:])
            cos2 = apers.tile([P, S], BF16)
            sin_neg = apers.tile([P, S], BF16)
            cmask = []
            for j in range(SC // P):
                mk = apers.tile([P, SC], BF16, name=f"cmask{j}")
                nc.vector.memset(mk[:], 1.0)
                nc.gpsimd.affine_select(mk[:], mk[:], pattern=[[1, SC]],
                                        base=-j * P, channel_multiplier=-1,
                                        compare_op=ALU.is_ge, fill=0.0)
                cmask.append(mk)

            nc.sync.dma_start(cos2[:], cos_in[:])
            nc.sync.dma_start(sin_neg[:], sin_in[:])

            # fp8 identity * SWO for the fused residual add in o_proj
            sw_id = apers.tile([P, 2, P], F8, name="sw_id")
            nc.scalar.activation(sw_id[:, 0, :], ident[:], AF.Copy, scale=SWO)
            nc.scalar.activation(sw_id[:, 1, :], ident[:], AF.Copy, scale=SWO)

            # ---- persistent attention-era tiles ----
            kT = apers.tile([P, S], BF16, name="kT")
            vT = apers.tile([P, S], BF16, name="vT")
            s1b = apers.tile([P, S], BF16, name="s1b")

            # wo SBUF cache: created late, in the region wqp frees
            wo_es = contextlib.ExitStack()

            # wq SBUF cache + gathered-x tiles (freed after qkv3)
            wq_es = contextlib.ExitStack()
            wqp = wq_es.enter_context(tc.tile_pool(name="wqp", bufs=1))
            wq_sb_hi = [wqp.tile([P, NPH, 2, P], F8, name=f"wqh{j}") for j in range(NJ)]
            wq_sb_lo = [wqp.tile([P, NPH, 2, P], F8, name=f"wql{j}") for j in range(NJ)]
            for j in range(NJ):
                nc.scalar.dma_start(wq_sb_hi[j][:], wslice(wq_hi, j))
            for j in range(NJ):
                nc.scalar.dma_start(wq_sb_lo[j][:], wslice(wq_lo, j))

            def emit_s1b(sc):
                cs = slice(sc * SC, (sc + 1) * SC)
                srows_b = awork.tile([8, SC], BF16, tag="srb", bufs=1,
                                     name=f"sr1b{sc}")
                nc.gpsimd.dma_start(srows_b[:], ag1_v[sc][:, HID_SH, 0:SC])
                srows = awork.tile([8, SC], F32, tag="srf", bufs=1,
                                   name=f"sr1f{sc}")
                nc.vector.tensor_copy(srows[:], srows_b[:])
                ssum = awork.tile([8, SC], F32, tag="ssum", bufs=1,
                                  name=f"ss1{sc}")
                nc.gpsimd.partition_all_reduce(ssum[:], srows[:], channels=8,
                                               reduce_op=ReduceOp.add)
                var = awork.tile([1, SC], F32, tag="var", bufs=1, name=f"v1{sc}")
                nc.scalar.activation(var[:], ssum[:1, :], AF.Copy,
                                     scale=1.0 / HID, bias=EPS)
                nc.vector.reciprocal(var[:], var[:])
                varb = awork.tile([1, SC], BF16, tag="varb", bufs=1,
                                  name=f"v1b{sc}")
                nc.scalar.activation(varb[:], var[:], AF.Sqrt, scale=CE1 * CE1)
                nc.gpsimd.partition_broadcast(s1b[:, cs], varb[:])

            qcs = {}
            gqs = {}

            def load_gq(sc):
                ghi, glo = [], []
                for cb in range(N_CORES):
                    g = wqp.tile([P, 2, 2, SC], F8, tag="ghi", bufs=16,
                                 name=f"gh{cb}_{sc}")
                    nc.gpsimd.dma_start(
                        g[:], ag1_8[sc][cb, 0:HID_SH, 0:SC]
                        .rearrange("(tp two p) n -> p tp two n", tp=2, two=2))
                    ghi.append(g)
                    g = wqp.tile([P, 2, 2, SC], F8, tag="glo", bufs=16,
                                 name=f"gl{cb}_{sc}")
                    nc.sync.dma_start(
                        g[:], ag1_8[sc][cb, 0:HID_SH, SC:2 * SC]
                        .rearrange("(tp two p) n -> p tp two n", tp=2, two=2))
                    glo.append(g)
                gqs[sc] = (ghi, glo)

            def emit_qkv(sc):
                cs = slice(sc * SC, (sc + 1) * SC)
                if sc not in gqs:
                    load_gq(sc)
                ghi, glo = gqs[sc]

                def rhs(lst, g):
                    return lst[g // 2][:, g % 2]

                qc = {}
                pss = [acc.tile([P, SC], F32, tag="acc", name=f"qk{j}_{sc}")
                       for j in range(NJ)]
                for g in range(NPH):
                    for j in range(NJ):
                        nc.tensor.matmul(pss[j][:], wq_sb_hi[j][:, g],
                                         rhs(ghi, g), start=(g == 0),
                                         stop=False, perf_mode=DR)
                for g in range(NPH):
                    for j in range(NJ):
                        nc.tensor.matmul(pss[j][:], wq_sb_hi[j][:, g],
                                         rhs(glo, g), start=False, stop=False,
                                         perf_mode=DR)
                for g in range(NPH):
                    for j in range(NJ):
                        nc.tensor.matmul(pss[j][:], wq_sb_lo[j][:, g],
                                         rhs(ghi, g), start=False,
                                         stop=(g == NPH - 1), perf_mode=DR)
                for j in range(NJ):
                    if j < HQ:
                        dst = awork.tile([P, SC], BF16, tag="qc", bufs=8,
                                         name=f"qc{j}_{sc}")
                        qc[j] = dst
                        nc.vector.tensor_tensor(dst[:], pss[j][:], s1b[:, cs],
                                                ALU.mult)
                    else:
                        dst = kT if j == HQ else vT
                        nc.vector.tensor_tensor(dst[:, cs], pss[j][:],
                                                s1b[:, cs], ALU.mult)
                qcs[sc] = qc

            def emit_attn(sc):
                cs = slice(sc * SC, (sc + 1) * SC)
                qc = qcs[sc]
                # rope on q tiles + k chunk
                for j in range(HQ + 1):
                    tv = qc[j][:] if j < HQ else kT[:, cs]
                    swp = wtile(awork, [P, SC], BF16, "t1k", 3)
                    nc.sync.dma_start(swp[:64, :], tv[64:, :])
                    nc.sync.dma_start(swp[64:, :], tv[:64, :])
                    m1 = wtile(awork, [P, SC], BF16, "t1k", 3)
                    nc.vector.tensor_tensor(m1[:], tv, cos2[:, cs], ALU.mult)
                    m2 = wtile(awork, [P, SC], BF16, "t1k", 3)
                    nc.vector.tensor_tensor(m2[:], swp[:], sin_neg[:, cs], ALU.mult)
                    nc.vector.tensor_tensor(tv, m1[:], m2[:], ALU.add)

                # v transpose in place
                for t in range(sc * (SC // P), (sc + 1) * (SC // P)):
                    pst = tpps.tile([P, P], BF16, tag="tp", name=f"tp{t}")
                    nc.tensor.transpose(pst[:], vT[:, t * P:(t + 1) * P], ident[:])
                    nc.vector.tensor_copy(vT[:, t * P:(t + 1) * P], pst[:])

                # attention: 4 heads x this chunk; fp8 hi/lo attn output
                nsk = (sc + 1) * (SC // P)
                ahi = awork.tile([P, HQ, SC], F8, tag="ahi", bufs=1,
                                 name=f"ahi{sc}")
                alo = awork.tile([P, HQ, SC], F8, tag="alo", bufs=1,
                                 name=f"alo{sc}")
                for h in range(HQ):
                    pv = acc.tile([P, SC], F32, tag="acc", name=f"pv{h}_{sc}")
                    rs = rowps.tile([1, SC], F32, tag="row", name=f"rs{h}_{sc}")
                    for skt in range(nsk):
                        sps = acc.tile([P, SC], F32, tag="acc",
                                       name=f"s{h}_{sc}_{skt}")
                        nc.tensor.matmul(sps[:], kT[:, skt * P:(skt + 1) * P],
                                         qc[h][:], start=True, stop=True)
                        ex = wtile(awork, [P, SC], BF16, "ex", 3)
                        nc.scalar.activation(ex[:], sps[:], AF.Exp,
                                             scale=inv_sqrt_d)
                        if skt >= 4 * sc:
                            nc.vector.tensor_tensor(ex[:], ex[:],
                                                    cmask[skt - 4 * sc][:],
                                                    ALU.mult)
                        nc.tensor.matmul(rs[:], ones_bf[:], ex[:],
                                         start=(skt == 0), stop=(skt == nsk - 1))
                        nc.tensor.matmul(pv[:], vT[:, skt * P:(skt + 1) * P],
                                         ex[:], start=(skt == 0),
                                         stop=(skt == nsk - 1))
                    rcp = awork.tile([1, SC], F32, tag="rcp", bufs=1,
                                     name=f"rcp{h}_{sc}")
                    nc.vector.reciprocal(rcp[:], rs[:])
                    rcpb = wtile(awork, [P, SC], F32, "rcpb", 1)
                    nc.gpsimd.partition_broadcast(rcpb[:], rcp[:])
                    a32 = wtile(awork, [P, SC], F32, "a32", 1)
                    nc.vector.tensor_tensor(a32[:], pv[:], rcpb[:], ALU.mult)
                    nc.vector.tensor_copy(ahi[:, h, :], a32[:])
                    nc.vector.tensor_tensor(alo[:, h, :], a32[:], ahi[:, h, :],
                                            ALU.subtract)

                # publish fp8 attn out + AllGather for column-sharded o_proj
                nc.scalar.dma_start(
                    aag_in8[sc][0:HID_SH, :]
                    .rearrange("(h p) s -> p h s", p=P), ahi[:])
                nc.sync.dma_start(
                    aag_in8[sc][HID_SH:2 * HID_SH, :]
                    .rearrange("(h p) s -> p h s", p=P), alo[:])
                nc.gpsimd.collective_compute(
                    "AllGather", ALU.bypass, replica_groups=RG,
                    ins=[aag_in[sc][:].opt()], outs=[aag_out[sc][:].opt()])

            def emit_o(sc):
                """Column-sharded o_proj for chunk sc with fused residual2."""
                cs = slice(sc * SC, (sc + 1) * SC)
                hh = sc // 2
                cb0 = (sc % 2) * (SC // 2)
                # x (residual stream) hi/lo pair for the fused add
                xp = owork.tile([P, NT_HSH, 2, SC], F8, tag="xp", bufs=1,
                                name=f"xp{sc}")
                nc.scalar.dma_start(xp[:, :, 0, :], pay1_hi(ag1_in[sc]))
                nc.sync.dma_start(xp[:, :, 1, :], pay1_lo(ag1_in[sc]))

                r2f = owork.tile([P, NT_HSH, SC], F32, tag="r2f", bufs=1,
                                 name=f"r2f{sc}")
                r2hi = owork.tile([P, NT_HSH, SC], F8, tag="r2hi", bufs=1,
                                  name=f"r2hi{sc}")
                r2lo = owork.tile([P, NT_HSH, SC], F8, tag="r2lo", bufs=1,
                                  name=f"r2lo{sc}")
                sq = owork.tile([P, NT_HSH, SC], BF16, tag="r2sq", bufs=1,
                                name=f"r2sq{sc}")
                ps2 = rowps.tile([1, SC], F32, tag="row", name=f"ssq2_{sc}")
                pss = [acc.tile([P, SC], F32, tag="acc", name=f"o{m}_{sc}")
                       for m in range(NT_HSH)]
                for g in range(NPH):
                    cb, j = g // 2, g % 2
                    ghi = owork.tile([P, 2, SC], F8, tag="oghi", bufs=4,
                                     name=f"ogh{g}_{sc}")
                    nc.scalar.dma_start(
                        ghi[:], aag_out8[sc][cb, j * 256:(j + 1) * 256, :]
                        .rearrange("(two p) s -> p two s", p=P))
                    glo = owork.tile([P, 2, SC], F8, tag="oglo", bufs=4,
                                     name=f"ogl{g}_{sc}")
                    nc.sync.dma_start(
                        glo[:], aag_out8[sc][cb, HID_SH + j * 256:
                                             HID_SH + (j + 1) * 256, :]
                        .rearrange("(two p) s -> p two s", p=P))
                    for m in range(NT_HSH):
                        nc.tensor.matmul(pss[m][:],
                                         wo_sb_hi[:, g, :, m * P:(m + 1) * P],
                                         ghi[:], start=(g == 0), stop=False,
                                         perf_mode=DR)
                    for m in range(NT_HSH):
                        nc.tensor.matmul(pss[m][:],
                                         wo_sb_hi[:, g, :, m * P:(m + 1) * P],
                                         glo[:], start=False, stop=False,
                                         perf_mode=DR)
                    for m in range(NT_HSH):
                        nc.tensor.matmul(pss[m][:],
                                         wo_sb_lo[:, g, :, m * P:(m + 1) * P],
                                         ghi[:], start=False, stop=False,
                                         perf_mode=DR)
                for m in range(NT_HSH):
                    # += SWO * (xhi + xlo): fused residual add
                    nc.tensor.matmul(pss[m][:], sw_id[:], xp[:, m],
                                     start=False, stop=True, perf_mode=DR)
                    nc.scalar.activation(r2f[:, m, :], pss[m][:], AF.Copy,
                                         scale=CO)
                    nc.vector.tensor_copy(r2hi[:, m, :], r2f[:, m, :])
                    nc.vector.tensor_tensor(r2lo[:, m, :], r2f[:, m, :],
                                            r2hi[:, m, :], ALU.subtract)
                    nc.scalar.activation(sq[:, m, :], r2f[:, m, :], AF.Square)
                    nc.tensor.matmul(ps2[:], ones_bf[:], sq[:, m, :],
                                     start=(m == 0), stop=(m == NT_HSH - 1))
                nc.gpsimd.dma_start(res2_v[:, :, cs], r2f[:])
                nc.gpsimd.dma_start(pay_hi(ag2_in[hh], cb0), r2hi[:])
                nc.gpsimd.dma_start(pay_lo(ag2_in[hh], cb0), r2lo[:])
                ssq2 = owork.tile([1, SC], BF16, tag="ssq2", bufs=2,
                                  name=f"sq2_{sc}")
                nc.vector.tensor_copy(ssq2[:], ps2[:])
                nc.gpsimd.dma_start(
                    ag2_in[hh][HID_SH:HID_SH + 1,
                               (sc % 2) * SC:(sc % 2) * SC + SC], ssq2[:])

            def emit_ag2(hh):
                nc.gpsimd.collective_compute(
                    "AllGather", ALU.bypass, replica_groups=RG,
                    ins=[ag2_in[hh][:].opt()], outs=[ag2_out[hh][:].opt()])

            # =========== era B pools (MLP) ===========
            # created mid-schedule (pool creation reserves SBUF immediately);
            # closures below bind these names at call time.
            b_es = contextlib.ExitStack()
            mpers = mwork = mstr = None
            s2b = None

            def emit_s2b(sc):
                cs = slice(sc * SC, (sc + 1) * SC)
                hh = sc // 2
                hcs = slice((sc % 2) * SC, (sc % 2) * SC + SC)
                srows_b = mwork.tile([8, SC], BF16, tag="srb2", bufs=1,
                                     name=f"sr2b{sc}")
                nc.gpsimd.dma_start(srows_b[:], ag2_v[hh][:, HID_SH, hcs])
                srows = mwork.tile([8, SC], F32, tag="srf2", bufs=1,
                                   name=f"sr2f{sc}")
                nc.vector.tensor_copy(srows[:], srows_b[:])
                ssum = mwork.tile([8, SC], F32, tag="ssum2", bufs=1,
                                  name=f"ss2{sc}")
                nc.gpsimd.partition_all_reduce(ssum[:], srows[:], channels=8,
                                               reduce_op=ReduceOp.add)
                var = mwork.tile([1, SC], F32, tag="var2", bufs=2, name=f"v2{sc}")
                nc.scalar.activation(var[:], ssum[:1, :], AF.Copy,
                                     scale=1.0 / (HID * CD), bias=EPS / CD)
                nc.vector.reciprocal(var[:], var[:])     # = CD / var
                varb2 = mwork.tile([1, SC], BF16, tag="vb2", bufs=2,
                                   name=f"vb2{sc}")
                nc.vector.tensor_copy(varb2[:], var[:])
                nc.gpsimd.partition_broadcast(s2b[:, cs], varb2[:])

            g2pool = {}

            def load_g2(half):
                g2p = g2pool["p"]
                ghi, glo = [], []
                for cb in range(N_CORES):
                    g = g2p.tile([P, 2, 2, SH], F8, tag="g2h", bufs=8,
                                 name=f"g2h{cb}_{half}")
                    for q2 in range(2):
                        qs = slice(q2 * SH // 2, (q2 + 1) * SH // 2)
                        nc.gpsimd.dma_start(
                            g[:, :, :, qs], ag2_8[half][cb, 0:HID_SH, qs]
                            .rearrange("(tp two p) n -> p tp two n",
                                       tp=2, two=2))
                    ghi.append(g)
                    g = g2p.tile([P, 2, 2, SH], F8, tag="g2l", bufs=8,
                                 name=f"g2l{cb}_{half}")
                    for q2 in range(2):
                        qs = slice(q2 * SH // 2, (q2 + 1) * SH // 2)
                        nc.sync.dma_start(
                            g[:, :, :, qs],
                            ag2_8[half][cb, 0:HID_SH,
                                        SH + q2 * SH // 2:
                                        SH + (q2 + 1) * SH // 2]
                            .rearrange("(tp two p) n -> p tp two n",
                                       tp=2, two=2))
                    glo.append(g)
                return ghi, glo


            g2 = {}

            # ============ era A schedule ============
            load_gq(0)
            emit_s1b(0)
            emit_qkv(0)
            load_gq(1)
            emit_s1b(1)
            emit_qkv(1)
            load_gq(2)
            emit_attn(0)
            emit_s1b(2)
            emit_qkv(2)
            load_gq(3)
            emit_attn(1)
            emit_s1b(3)
            emit_qkv(3)
            wq_es.close()
            mpers = b_es.enter_context(tc.tile_pool(name="mpers", bufs=1, side="right"))
            mwork = b_es.enter_context(tc.tile_pool(name="mwork", bufs=1, side="right"))
            wop = wo_es.enter_context(tc.tile_pool(name="wop", bufs=1))
            wo_sb_hi = wop.tile([P, NPH, 2, HID_SH], F8, name="wo_h")
            wo_sb_lo = wop.tile([P, NPH, 2, HID_SH], F8, name="wo_l")
            for g4 in range(0, NPH, 4):
                nc.gpsimd.dma_start(wo_sb_hi[:, g4:g4 + 4],
                                    wslice(wo_hi, 0, m=HID_SH)[:, g4:g4 + 4])
                nc.gpsimd.dma_start(wo_sb_lo[:, g4:g4 + 4],
                                    wslice(wo_lo, 0, m=HID_SH)[:, g4:g4 + 4])
            s2b = mpers.tile([P, S], BF16, name="s2b")
            g2pool["p"] = b_es.enter_context(tc.tile_pool(name="g2p", bufs=1, side="right"))
            emit_attn(2)
            emit_o(0)
            emit_o(1)
            emit_ag2(0)
            emit_attn(3)
            emit_s2b(0)
            emit_s2b(1)
            g2[0] = load_g2(0)
            emit_o(2)
            emit_o(3)
            emit_ag2(1)
            g2[1] = load_g2(1)
            emit_s2b(2)
            emit_s2b(3)
            wo_es.close()
            a_es.close()

            mstr = b_es.enter_context(tc.tile_pool(name="mstr", bufs=1, side="right"))
            utp = b_es.enter_context(tc.tile_pool(name="utp", bufs=1, side="right"))
            ut_hi = [utp.tile([P, 2, S], F8, name=f"uth{g}") for g in range(NPI)]
            ut_lo = [utp.tile([P, 2, S], F8, name=f"utl{g}") for g in range(NPI)]

            def emit_up(half, it_range):
                ghi, glo = g2[half]

                def rhs(lst, g, ncs):
                    return lst[g // 2][:, g % 2, :, ncs]

                for it in it_range:
                    wh = mstr.tile([P, NPH, 2, P], F8, tag="wuh", bufs=2,
                                   name=f"wuh{it}_{half}")
                    nc.scalar.dma_start(wh[:], wslice(wu_hi, it))
                    wl = mstr.tile([P, NPH, 2, P], F8, tag="wul", bufs=2,
                                   name=f"wul{it}_{half}")
                    nc.scalar.dma_start(wl[:], wslice(wu_lo, it))
                    pss = [acc.tile([P, SC], F32, tag="acc", name=f"up{it}_{2*half+ci}")
                           for ci in range(2)]
                    for g in range(NPH):
                        for ci in range(2):
                            nc.tensor.matmul(pss[ci][:], wh[:, g],
                                             rhs(ghi, g, slice(ci * SC, (ci + 1) * SC)),
                                             start=(g == 0), stop=False,
                                             perf_mode=DR)
                    for g in range(NPH):
                        for ci in range(2):
                            nc.tensor.matmul(pss[ci][:], wh[:, g],
                                             rhs(glo, g, slice(ci * SC, (ci + 1) * SC)),
                                             start=False, stop=False, perf_mode=DR)
                    for g in range(NPH):
                        for ci in range(2):
                            nc.tensor.matmul(pss[ci][:], wl[:, g],
                                             rhs(ghi, g, slice(ci * SC, (ci + 1) * SC)),
                                             start=False, stop=(g == NPH - 1),
                                             perf_mode=DR)
                    for ci in range(2):
                        sc = 2 * half + ci
                        cs = slice(sc * SC, (sc + 1) * SC)
                        rl = mwork.tile([P, SC], F32, tag="rl", bufs=2,
                                        name=f"rl{it}_{sc}")
                        nc.scalar.activation(rl[:], pss[ci][:], AF.Relu,
                                             scale=ALPHA_UP)
                        nc.vector.tensor_tensor(rl[:], rl[:], rl[:], ALU.mult)
                        nc.vector.tensor_copy(ut_hi[it // 2][:, it % 2, cs], rl[:])
                        nc.vector.tensor_tensor(ut_lo[it // 2][:, it % 2, cs],
                                                rl[:], ut_hi[it // 2][:, it % 2, cs],
                                                ALU.subtract)

            # ---- era B schedule ----
            emit_up(0, range(NT_INT))
            emit_up(1, range(NT_INT))

            # ---- down proj: 3-term DoubleRow over full S, uneven RS ----
            mstart = 0
            for pi, mc in enumerate(PIECES):
                rs2t = rs2_in[pi]
                rs2_fat = rs2t[:].rearrange("(q p) s -> p q s", p=P)
                for mq in range(mc):
                    m = mstart + mq
                    wh = mstr.tile([P, NPI, 2, P], F8, tag="wdh", bufs=3,
                                   name=f"wdh{m}")
                    nc.scalar.dma_start(wh[:], wslice(wd_hi, m))
                    wl = mstr.tile([P, NPI, 2, P], F8, tag="wdl", bufs=3,
                                   name=f"wdl{m}")
                    nc.scalar.dma_start(wl[:], wslice(wd_lo, m))
                    evf = mwork.tile([P, NSC, SC], BF16, tag="dnev", bufs=1,
                                     name=f"dev{m}")
                    for sc in range(NSC):
                        cs = slice(sc * SC, (sc + 1) * SC)
                        ps = acc.tile([P, SC], F32, tag="acc",
                                      name=f"dn{m}_{sc}")
                        for g in range(NPI):
                            nc.tensor.matmul(ps[:], wh[:, g],
                                             ut_hi[g][:, :, cs],
                                             start=(g == 0), stop=False,
                                             perf_mode=DR)
                        for g in range(NPI):
                            nc.tensor.matmul(ps[:], wh[:, g],
                                             ut_lo[g][:, :, cs],
                                             start=False, stop=False,
                                             perf_mode=DR)
                        for g in range(NPI):
                            nc.tensor.matmul(ps[:], wl[:, g],
                                             ut_hi[g][:, :, cs],
                                             start=False,
                                             stop=(g == NPI - 1),
                                             perf_mode=DR)
                        nc.vector.tensor_tensor(evf[:, sc, :], ps[:],
                                                s2b[:, cs], ALU.mult)
                    nc.sync.dma_start(rs2_fat[:, mq], evf[:])
                nc.gpsimd.collective_compute(
                    "ReduceScatter", ALU.add, replica_groups=RG,
                    ins=[rs2t[:].opt()], outs=[rs2_out[pi][:].opt()])
                orow = mstart * P // N_CORES
                nc.gpsimd.dma_start(
                    out_mlp[orow:orow + mc * P // N_CORES, :], rs2_out[pi][:])
                mstart += mc
            b_es.close()

    nc.compile()
    return nc


def _q8_pair(x):
    x32 = np.asarray(x, np.float32)
    hi = np.asarray(np.clip(x32, -240, 240), F8NP)
    lo = np.asarray(np.clip(x32 - hi.astype(np.float32), -240, 240), F8NP)
    return np.ascontiguousarray(hi), np.ascontiguousarray(lo)


def _lay_tiles(w, mt=P):
    """[K, M] fp8 -> [(M//mt)*128, (K//256)*2*mt] in SBUF tile order.

    Row = mtile*128 + p; cols = (kpair, two, m) flattened, so each per-tile
    DMA is one contiguous [128, (K//256)*2*mt] block."""
    K, M = w.shape
    a = w.reshape(K // 256, 2, P, M // mt, mt).transpose(3, 2, 0, 1, 4)
    return np.ascontiguousarray(a.reshape(M // mt * P, (K // 256) * 2 * mt))


def shard_inputs(positions, hidden_states, residual, qkv_w, o_w, up_w, down_w,
                 ln1_w, ln2_w):
    hTf = np.ascontiguousarray(
        np.asarray(hidden_states).reshape(S, HID).T.astype(ml_dtypes.bfloat16))
    rTf = np.ascontiguousarray(
        np.asarray(residual).reshape(S, HID).T.astype(ml_dtypes.bfloat16))
    pos = np.asarray(positions).reshape(S).astype(np.float64)
    inv = 1.0 / (THETA ** (np.arange(0, DHEAD, 2, dtype=np.float64) / DHEAD))
    fr = pos[:, None] * inv                      # [S, 64]
    cost = np.cos(fr).T.astype(np.float32)       # [64, S]
    sint = np.sin(fr).T.astype(np.float32)
    cos2 = np.ascontiguousarray(
        np.concatenate([cost, cost], 0).astype(ml_dtypes.bfloat16))
    sin_neg = np.ascontiguousarray(
        np.concatenate([-sint, sint], 0).astype(ml_dtypes.bfloat16))
    q_size = N_HEADS * DHEAD
    kv = N_KV * DHEAD
    w1 = np.asarray(qkv_w, np.float32) * np.asarray(ln1_w, np.float32)[:, None] * SW1
    wof = np.asarray(o_w, np.float32) * SWO
    wuf = np.asarray(up_w, np.float32) * np.asarray(ln2_w, np.float32)[:, None] * SWU
    wdf = np.asarray(down_w, np.float32) * SWD
    in_maps = []
    for c in range(N_CORES):
        wq_c = np.concatenate([
            w1[:, c * HQ * DHEAD:(c + 1) * HQ * DHEAD],
            w1[:, q_size + c * DHEAD:q_size + (c + 1) * DHEAD],
            w1[:, q_size + kv + c * DHEAD:q_size + kv + (c + 1) * DHEAD],
        ], axis=1)
        wq_h, wq_l = _q8_pair(wq_c)
        wo_h, wo_l = _q8_pair(wof[:, c * HID_SH:(c + 1) * HID_SH])
        wu_h, wu_l = _q8_pair(wuf[:, c * INT_SH:(c + 1) * INT_SH])
        wd_h, wd_l = _q8_pair(wdf[c * INT_SH:(c + 1) * INT_SH, :])
        wq_h, wq_l = _lay_tiles(wq_h), _lay_tiles(wq_l)
        wo_h, wo_l = _lay_tiles(wo_h, mt=HID_SH), _lay_tiles(wo_l, mt=HID_SH)
        wu_h, wu_l = _lay_tiles(wu_h), _lay_tiles(wu_l)
        wd_h, wd_l = _lay_tiles(wd_h), _lay_tiles(wd_l)
        in_maps.append({
            "hT": np.ascontiguousarray(hTf[c * HID_SH:(c + 1) * HID_SH]),
            "rT": np.ascontiguousarray(rTf[c * HID_SH:(c + 1) * HID_SH]),
            "cos2": cos2, "sin_neg": sin_neg,
            "wq_hi": wq_h, "wq_lo": wq_l,
            "wo_hi": wo_h, "wo_lo": wo_l,
            "wu_hi": wu_h, "wu_lo": wu_l,
            "wd_hi": wd_h, "wd_lo": wd_l,
        })
    return in_maps


_CACHE = {}


def kernel(**inputs):
    from concourse.bass_utils import run_bass_kernel_spmd
    if "nc" not in _CACHE:
        _CACHE["nc"] = build_graph()
    nc = _CACHE["nc"]
    in_maps = shard_inputs(**{k: np.asarray(v) for k, v in inputs.items()})
    res = run_bass_kernel_spmd(nc, in_maps, core_ids=list(range(N_CORES)),
                               trace=False)
    res2T = np.concatenate([res.results[c]["res2T"] for c in range(N_CORES)], axis=0)
    mlpT = np.empty((HID, S), np.float32)
    for c in range(N_CORES):
        mt = res.results[c]["mlpT"]
        mstart = 0
        for pi, mc in enumerate(PIECES):
            rows = mc * P // N_CORES          # rows per core for this piece
            orow = mstart * P // N_CORES
            g0 = mstart * P + c * rows        # global hid row start
            mlpT[g0:g0 + rows] = mt[orow:orow + rows]
            mstart += mc
    mlp_out = np.ascontiguousarray(mlpT.T).reshape(1, S, HID)
    residual2 = np.ascontiguousarray(res2T.T).reshape(1, S, HID)
    return mlp_out, residual2


# revision 37
# speedup vs baseline: 1.0231x; 1.0083x over previous
"""Arcee decoder layer on 8 TRN2 NeuronCores — TP8, fp8 hi/lo DoubleRow.

Sharding (8-way TP, transposed activation layout [hidden, seq] on device):
  - core c owns: q heads 4c..4c+3 + kv head c, residual rows 512c..512c+511,
    intermediate cols 2048c..2048c+2047.
  - Big GEMMs (qkv/o/up/down) run as 3-term hi/lo fp8e4m3 DoubleRow:
    W.x ~= Whi.xhi + Whi.xlo + Wlo.xhi, each term contracting 256 rows per
    0.5-cycle/row matmul. Weights pre-quantized on host (ln1/ln2 and
    per-tensor scales folded); activations split hi/lo on device.
  - RMSNorm: un-normalized residual stream AllGathered with per-core partial
    sum-of-squares row embedded; rsqrt scale folded into PSUM eviction.
  - AG payload [520, 1024] bf16 per S-half: rows 0-511 carry x rows as fp8
    bytes (hi in bf16 cols 0-511, lo in 512-1023), row 512 = bf16 ssq row.
  - o_proj is COLUMN-sharded: per-chunk fp8 hi/lo attention outputs are
    AllGathered (0.5 MiB/rank, much cheaper than the 4 MiB ReduceScatter of
    o partials), then each core computes the full contraction for its own
    512 hid rows. residual2 is fused into the o eviction via an extra
    DoubleRow matmul with a 128*I fp8 identity against the (xhi, xlo) pair,
    so res2 never round-trips through a collective.
  - down_proj partials reduce via bf16 ReduceScatter split into uneven
    pieces so the exposed tail is small.
  - attention (scores/softmax/PV) stays bf16. DMAs are batched into fat
    tiles and spread across queues to keep dispatch off the critical path.
"""
import sys

sys.path.insert(0, "/opt/trn_rl_repo")

import contextlib
import math
import numpy as np
import ml_dtypes

import concourse.bass as bass
import concourse.mybir as mybir
import concourse.tile as tile
from concourse import bacc
from concourse.bass_isa import ReduceOp
from concourse.masks import make_identity

F32 = mybir.dt.float32
BF16 = mybir.dt.bfloat16
F8 = mybir.dt.float8e4
F8E5 = mybir.dt.float8e5
I32 = mybir.dt.int32
AF = mybir.ActivationFunctionType
ALU = mybir.AluOpType
DR = mybir.MatmulPerfMode.DoubleRow
F8NP = ml_dtypes.float8_e4m3

N_CORES = 8
S = 2048
HID = 4096
N_HEADS = 32
N_KV = 8
DHEAD = 128
INTER = 16384
EPS = 1e-5
THETA = 10000.0

HQ = N_HEADS // N_CORES          # 4 q heads per core
HID_SH = HID // N_CORES          # 512 residual rows per core
INT_SH = INTER // N_CORES        # 2048 intermediate per core
NJ = HQ + 2                      # qkv col tiles per core (4q + k + v)
QKV_COLS = NJ * DHEAD            # 768
P = 128
SC = 512                         # seq chunk
NSC = S // SC                    # 4
SH = S // 2                      # 1024 (half)
NT_HID = HID // P                # 32
NT_HSH = HID_SH // P             # 4
NT_INT = INT_SH // P             # 16
NPH = NT_HID // 2                # 16 k-pairs over HID
NPI = NT_INT // 2                # 8 k-pairs over INT_SH
BLK = HID_SH + 8                 # 520 payload rows
TWO_PI = 2.0 * math.pi

# fp8 scales (activations unscaled; weights scaled on host)
SW1 = 1024.0
SWO = 128.0                      # must stay fp8-representable (identity add)
SWU = 1024.0
SWD = 1024.0
SQU = 0.25                       # scale on u = relu(z)^2
CE1 = 1.0 / SW1                  # qkv evict const (with rsqrt row)
CO = 1.0 / SWO                   # o evict const
ALPHA_UP = math.sqrt(SQU) / SWU  # relu evict scale
CD = 1.0 / (SWD * SQU)           # down evict const (with 1/var row)

# down RS pieces (m-tile counts; sum = 32); tapered so the tail is short.
PIECES = [8, 8, 6, 4, 2, 2, 2]

# softmax bias: probs stored as e5m2 exp(s*inv_sqrt_d - XC); max masked
# score*inv_sqrt_d is 12.47 and the min row-max is -4.03 for this input
# distribution, so XC=4 keeps exp in [3e-4, 4.8e3] — inside e5m2 range.
XC = 4.0


def build_graph():
    nc = bacc.Bacc(None, target_bir_lowering=False, debug=False)

    hT = nc.declare_dram_parameter("hT", [HID_SH, S], BF16, isOutput=False)
    rT = nc.declare_dram_parameter("rT", [HID_SH, S], BF16, isOutput=False)
    cos_in = nc.declare_dram_parameter("cos2", [P, S], BF16, isOutput=False)
    sin_in = nc.declare_dram_parameter("sin_neg", [P, S], BF16, isOutput=False)
    # weights arrive pre-laid-out in SBUF tile order (see _lay_tiles):
    # row = mtile*128 + p, cols = (t, two, m) flattened — every per-tile DMA
    # is a contiguous [128, X] block (full-width descriptors).
    wq_hi = nc.declare_dram_parameter("wq_hi", [NJ * P, NPH * 2 * P], F8, isOutput=False)
    wq_lo = nc.declare_dram_parameter("wq_lo", [NJ * P, NPH * 2 * P], F8, isOutput=False)
    wo_hi = nc.declare_dram_parameter("wo_hi", [P, NPH * 2 * HID_SH], F8, isOutput=False)
    wo_lo = nc.declare_dram_parameter("wo_lo", [P, NPH * 2 * HID_SH], F8, isOutput=False)
    wu_hi = nc.declare_dram_parameter("wu_hi", [NT_INT * P, NPH * 2 * P], F8, isOutput=False)
    wu_lo = nc.declare_dram_parameter("wu_lo", [NT_INT * P, NPH * 2 * P], F8, isOutput=False)
    wd_hi = nc.declare_dram_parameter("wd_hi", [NT_HID * P, NPI * 2 * P], F8, isOutput=False)
    wd_lo = nc.declare_dram_parameter("wd_lo", [NT_HID * P, NPI * 2 * P], F8, isOutput=False)
    out_res2 = nc.declare_dram_parameter("res2T", [HID_SH, S], F32, isOutput=True)
    out_mlp = nc.declare_dram_parameter("mlpT", [HID_SH, S], F32, isOutput=True)

    RG = [list(range(N_CORES))]
    inv_sqrt_d = 1.0 / math.sqrt(DHEAD)

    # per-tile contiguous weight slices -> [p, t(pair), two, m]
    def wslice(w, i, m=P):
        return w[i * P:(i + 1) * P, :].rearrange("p (t two m) -> p t two m",
                                                 two=2, m=m)

    hT_v = hT[:].rearrange("(i p) s -> p i s", p=P)
    rT_v = rT[:].rearrange("(i p) s -> p i s", p=P)
    res2_v = out_res2[:].rearrange("(i p) s -> p i s", p=P)

    with tile.TileContext(nc) as tc:
        with contextlib.ExitStack() as ctx:
            const = ctx.enter_context(tc.tile_pool(name="const", bufs=1))
            acc = ctx.enter_context(tc.tile_pool(name="acc", bufs=6, space="PSUM"))
            rowps = ctx.enter_context(tc.tile_pool(name="rowps", bufs=1, space="PSUM"))
            tpps = ctx.enter_context(tc.tile_pool(name="tpps", bufs=1, space="PSUM"))
            dram = ctx.enter_context(tc.tile_pool(name="dram", bufs=1, space="DRAM"))

            ones_bf = const.tile([P, 1], BF16)
            nc.vector.memset(ones_bf[:], 1.0)
            # DR lhsT needs pair-step %16==0, so the rowsum ones tile is
            # [P, 2, 16] (rows 0-15 of the result all carry the same sum)
            ones_f5p = const.tile([P, 2, 16], F8E5)
            nc.vector.memset(ones_f5p[:], 1.0)
            negxc = const.tile([P, 1], F32)
            nc.vector.memset(negxc[:], -XC)

            # DRAM scratch
            ag1_in = [dram.tile([BLK, SC], BF16, name=f"ag1_in{s_}")
                      for s_ in range(NSC)]
            ag1_out = [dram.tile([N_CORES * BLK, SC], BF16, name=f"ag1_out{s_}",
                                 addr_space="Shared") for s_ in range(NSC)]
            ag2_in = [dram.tile([BLK, SH], BF16, name=f"ag2_in{h}") for h in range(2)]
            ag2_out = [dram.tile([N_CORES * BLK, SH], BF16, name=f"ag2_out{h}",
                                 addr_space="Shared") for h in range(2)]
            # per-chunk attention-out AG: fp8 hi (rows 0-511) + lo (512-1023)
            aag_in = [dram.tile([2 * HID_SH, SC // 2], BF16, name=f"aag_in{sc}")
                      for sc in range(NSC)]
            aag_out = [dram.tile([N_CORES * 2 * HID_SH, SC // 2], BF16,
                                 name=f"aag_out{sc}", addr_space="Shared")
                       for sc in range(NSC)]
            rs2_in = [dram.tile([mc * P, S], BF16, name=f"rs2_in{pi}")
                      for pi, mc in enumerate(PIECES)]
            rs2_out = [dram.tile([mc * P // N_CORES, S], BF16,
                                 name=f"rs2_out{pi}")
                       for pi, mc in enumerate(PIECES)]

            ag1_v = [t[:].rearrange("(c r) s -> c r s", r=BLK) for t in ag1_out]
            ag2_v = [t[:].rearrange("(c r) s -> c r s", r=BLK) for t in ag2_out]
            ag1_8 = [t[:].bitcast(F8).rearrange("(c r) s -> c r s", r=BLK)
                     for t in ag1_out]
            ag2_8 = [t[:].bitcast(F8).rearrange("(c r) s -> c r s", r=BLK)
                     for t in ag2_out]

            # per-chunk ag1 payload regions (hi fp8 | lo fp8, + ssq row)
            def pay1_hi(t):
                return t[0:HID_SH, 0:SC // 2].bitcast(F8) \
                    .rearrange("(i p) s -> p i s", p=P)

            def pay1_lo(t):
                return t[0:HID_SH, SC // 2:SC].bitcast(F8) \
                    .rearrange("(i p) s -> p i s", p=P)
            aag_in8 = [t[:].bitcast(F8) for t in aag_in]           # [1024, SC]
            aag_out8 = [t[:].bitcast(F8).rearrange("(c r) s -> c r s",
                                                   r=2 * HID_SH)
                        for t in aag_out]                          # [8,1024,SC]

            # payload hi/lo region views as [p, i, s] fp8
            def pay_hi(t, cb0):
                return t[0:HID_SH, cb0:cb0 + SC // 2].bitcast(F8) \
                    .rearrange("(i p) s -> p i s", p=P)

            def pay_lo(t, cb0):
                return t[0:HID_SH, SH // 2 + cb0:SH // 2 + cb0 + SC // 2] \
                    .bitcast(F8).rearrange("(i p) s -> p i s", p=P)

            # =========== era A pools (attention + residual stream) ===========
            a_es = contextlib.ExitStack()
            apers = a_es.enter_context(tc.tile_pool(name="apers", bufs=1))
            awork = a_es.enter_context(tc.tile_pool(name="awork", bufs=1))
            owork = a_es.enter_context(tc.tile_pool(name="owork", bufs=1))

            _cnt = [0]

            def wtile(pool, shape, dt, tag, bufs):
                _cnt[0] += 1
                return pool.tile(shape, dt, tag=tag, bufs=bufs,
                                 name=f"t_{_cnt[0]}")

            # wq weight cache: created + loaded first so the 6MB of weight
            # DMA streams during phase-1 compute with nothing ahead of it
            wq_es = contextlib.ExitStack()
            wqw = wq_es.enter_context(tc.tile_pool(name="wqw", bufs=1))
            wq_sb_hi = [wqw.tile([P, NPH, 2, P], F8, name=f"wqh{j}") for j in range(NJ)]
            wq_sb_lo = [wqw.tile([P, NPH, 2, P], F8, name=f"wql{j}") for j in range(NJ)]
            for j in range(NJ):
                nc.scalar.dma_start(wq_sb_hi[j][:], wslice(wq_hi, j))
            for j in range(NJ):
                nc.scalar.dma_start(wq_sb_lo[j][:], wslice(wq_lo, j))

            # ---- phase 1: x = h + r; hi/lo fp8 + ssq into payload ----
            with tc.tile_pool(name="p1", bufs=1) as p1:
                for sc in range(NSC):
                    cs = slice(sc * SC, (sc + 1) * SC)
                    ps = rowps.tile([1, SC], F32, tag="row", name=f"ssq1p{sc}")
                    hf = wtile(p1, [P, NT_HSH, SC], BF16, "hf", 2)
                    rf = wtile(p1, [P, NT_HSH, SC], BF16, "rf", 2)
                    nc.sync.dma_start(hf[:], hT_v[:, :, cs])
                    nc.sync.dma_start(rf[:], rT_v[:, :, cs])
                    xt = wtile(p1, [P, NT_HSH, SC], F32, "xt", 2)
                    nc.vector.tensor_tensor(xt[:], hf[:], rf[:], ALU.add)
                    xhi = wtile(p1, [P, NT_HSH, SC], F8, "xhi", 2)
                    nc.vector.tensor_copy(xhi[:], xt[:])
                    xlo = wtile(p1, [P, NT_HSH, SC], F8, "xlo", 2)
                    nc.vector.tensor_tensor(xlo[:], xt[:], xhi[:], ALU.subtract)
                    nc.gpsimd.dma_start(pay1_hi(ag1_in[sc]), xhi[:])
                    nc.sync.dma_start(pay1_lo(ag1_in[sc]), xlo[:])
                    sq = wtile(p1, [P, NT_HSH, SC], BF16, "sq", 2)
                    nc.scalar.activation(sq[:], xt[:], AF.Square)
                    for i in range(NT_HSH):
                        nc.tensor.matmul(ps[:], ones_bf[:], sq[:, i, :],
                                         start=(i == 0), stop=(i == NT_HSH - 1))
                    ssq1b = awork.tile([1, SC], BF16, tag="ssq1b", bufs=2,
                                       name=f"ssq1b{sc}")
                    nc.vector.tensor_copy(ssq1b[:], ps[:])
                    nc.sync.dma_start(
                        ag1_in[sc][HID_SH:HID_SH + 1, 0:SC], ssq1b[:])
                    nc.gpsimd.collective_compute(
                        "AllGather", ALU.bypass, replica_groups=RG,
                        ins=[ag1_in[sc][:].opt()], outs=[ag1_out[sc][:].opt()])

            # ---- rope tables + masks (after AGs so phase-1 wins queues) ----
            ident = apers.tile([P, P], BF16)
            make_identity(nc, ident[:])
            cos2 = apers.tile([P, S], BF16)
            sin_neg = apers.tile([P, S], BF16)
            cmask = []
            for j in range(SC // P):
                mk = apers.tile([P, SC], BF16, name=f"cmask{j}")
                nc.vector.memset(mk[:], 1.0)
                nc.gpsimd.affine_select(mk[:], mk[:], pattern=[[1, SC]],
                                        base=-j * P, channel_multiplier=-1,
                                        compare_op=ALU.is_ge, fill=0.0)
                cmask.append(mk)

            nc.sync.dma_start(cos2[:], cos_in[:])
            nc.sync.dma_start(sin_neg[:], sin_in[:])

            # fp8 identity * SWO for the fused residual add in o_proj
            sw_id = apers.tile([P, 2, P], F8, name="sw_id")
            nc.scalar.activation(sw_id[:, 0, :], ident[:], AF.Copy, scale=SWO)
            nc.scalar.activation(sw_id[:, 1, :], ident[:], AF.Copy, scale=SWO)

            # ---- persistent attention-era tiles ----
            kT = apers.tile([P, S], BF16, name="kT")
            vT = apers.tile([P, S], BF16, name="vT")
            s1b = apers.tile([P, S], BF16, name="s1b")

            # wo SBUF cache: created late, in the region wqp frees
            wo_es = contextlib.ExitStack()

            # gathered-x tiles (freed after qkv3, with the wq weights)
            wqp = wq_es.enter_context(tc.tile_pool(name="gqp", bufs=1))

            def emit_s1b(sc):
                cs = slice(sc * SC, (sc + 1) * SC)
                srows_b = awork.tile([8, SC], BF16, tag="srb", bufs=1,
                                     name=f"sr1b{sc}")
                nc.gpsimd.dma_start(srows_b[:], ag1_v[sc][:, HID_SH, 0:SC])
                srows = awork.tile([8, SC], F32, tag="srf", bufs=1,
                                   name=f"sr1f{sc}")
                nc.vector.tensor_copy(srows[:], srows_b[:])
                ssum = awork.tile([8, SC], F32, tag="ssum", bufs=1,
                                  name=f"ss1{sc}")
                nc.gpsimd.partition_all_reduce(ssum[:], srows[:], channels=8,
                                               reduce_op=ReduceOp.add)
                var = awork.tile([1, SC], F32, tag="var", bufs=1, name=f"v1{sc}")
                nc.scalar.activation(var[:], ssum[:1, :], AF.Copy,
                                     scale=1.0 / HID, bias=EPS)
                nc.vector.reciprocal(var[:], var[:])
                varb = awork.tile([1, SC], BF16, tag="varb", bufs=1,
                                  name=f"v1b{sc}")
                nc.scalar.activation(varb[:], var[:], AF.Sqrt, scale=CE1 * CE1)
                nc.gpsimd.partition_broadcast(s1b[:, cs], varb[:])

            qcs = {}
            gqs = {}

            def load_gq(sc):
                ghi, glo = [], []
                for cb in range(N_CORES):
                    g = wqp.tile([P, 2, 2, SC], F8, tag="ghi", bufs=16,
                                 name=f"gh{cb}_{sc}")
                    nc.gpsimd.dma_start(
                        g[:], ag1_8[sc][cb, 0:HID_SH, 0:SC]
                        .rearrange("(tp two p) n -> p tp two n", tp=2, two=2))
                    ghi.append(g)
                    g = wqp.tile([P, 2, 2, SC], F8, tag="glo", bufs=16,
                                 name=f"gl{cb}_{sc}")
                    nc.sync.dma_start(
                        g[:], ag1_8[sc][cb, 0:HID_SH, SC:2 * SC]
                        .rearrange("(tp two p) n -> p tp two n", tp=2, two=2))
                    glo.append(g)
                gqs[sc] = (ghi, glo)

            def emit_qkv(sc):
                cs = slice(sc * SC, (sc + 1) * SC)
                if sc not in gqs:
                    load_gq(sc)
                ghi, glo = gqs[sc]

                def rhs(lst, g):
                    return lst[g // 2][:, g % 2]

                qc = {}
                pss = [acc.tile([P, SC], F32, tag="acc", name=f"qk{j}_{sc}")
                       for j in range(NJ)]
                for g in range(NPH):
                    for j in range(NJ):
                        nc.tensor.matmul(pss[j][:], wq_sb_hi[j][:, g],
                                         rhs(ghi, g), start=(g == 0),
                                         stop=False, perf_mode=DR)
                for g in range(NPH):
                    for j in range(NJ):
                        nc.tensor.matmul(pss[j][:], wq_sb_hi[j][:, g],
                                         rhs(glo, g), start=False, stop=False,
                                         perf_mode=DR)
                for g in range(NPH):
                    for j in range(NJ):
                        nc.tensor.matmul(pss[j][:], wq_sb_lo[j][:, g],
                                         rhs(ghi, g), start=False,
                                         stop=(g == NPH - 1), perf_mode=DR)
                for j in range(NJ):
                    if j < HQ:
                        dst = awork.tile([P, SC], BF16, tag="qc", bufs=8,
                                         name=f"qc{j}_{sc}")
                        qc[j] = dst
                        nc.vector.tensor_tensor(dst[:], pss[j][:], s1b[:, cs],
                                                ALU.mult)
                    else:
                        dst = kT if j == HQ else vT
                        nc.vector.tensor_tensor(dst[:, cs], pss[j][:],
                                                s1b[:, cs], ALU.mult)
                qcs[sc] = qc

            def emit_attn(sc):
                cs = slice(sc * SC, (sc + 1) * SC)
                qc = qcs[sc]
                # rope on q tiles + k chunk
                for j in range(HQ + 1):
                    tv = qc[j][:] if j < HQ else kT[:, cs]
                    swp = wtile(awork, [P, SC], BF16, "t1k", 3)
                    nc.sync.dma_start(swp[:64, :], tv[64:, :])
                    nc.sync.dma_start(swp[64:, :], tv[:64, :])
                    m1 = wtile(awork, [P, SC], BF16, "t1k", 3)
                    nc.vector.tensor_tensor(m1[:], tv, cos2[:, cs], ALU.mult)
                    m2 = wtile(awork, [P, SC], BF16, "t1k", 3)
                    nc.vector.tensor_tensor(m2[:], swp[:], sin_neg[:, cs], ALU.mult)
                    nc.vector.tensor_tensor(tv, m1[:], m2[:], ALU.add)

                # v transpose in place
                for t in range(sc * (SC // P), (sc + 1) * (SC // P)):
                    pst = tpps.tile([P, P], BF16, tag="tp", name=f"tp{t}")
                    nc.tensor.transpose(pst[:], vT[:, t * P:(t + 1) * P], ident[:])
                    nc.vector.tensor_copy(vT[:, t * P:(t + 1) * P], pst[:])

                # attention: 4 heads x this chunk; fp8 hi/lo attn output
                nsk = (sc + 1) * (SC // P)
                ahi = awork.tile([P, HQ, SC], F8, tag="ahi", bufs=1,
                                 name=f"ahi{sc}")
                alo = awork.tile([P, HQ, SC], F8, tag="alo", bufs=1,
                                 name=f"alo{sc}")
                for h in range(HQ):
                    pv = acc.tile([P, SC], F32, tag="acc", name=f"pv{h}_{sc}")
                    rs = rowps.tile([1, SC], F32, tag="row", name=f"rs{h}_{sc}")
                    for skt in range(nsk):
                        sps = acc.tile([P, SC], F32, tag="acc",
                                       name=f"s{h}_{sc}_{skt}")
                        nc.tensor.matmul(sps[:], kT[:, skt * P:(skt + 1) * P],
                                         qc[h][:], start=True, stop=True)
                        ex = wtile(awork, [P, SC], BF16, "ex", 3)
                        nc.scalar.activation(ex[:], sps[:], AF.Exp,
                                             scale=inv_sqrt_d)
                        if skt >= 4 * sc:
                            nc.vector.tensor_tensor(ex[:], ex[:],
                                                    cmask[skt - 4 * sc][:],
                                                    ALU.mult)
                        nc.tensor.matmul(rs[:], ones_bf[:], ex[:],
                                         start=(skt == 0), stop=(skt == nsk - 1))
                        nc.tensor.matmul(pv[:], vT[:, skt * P:(skt + 1) * P],
                                         ex[:], start=(skt == 0),
                                         stop=(skt == nsk - 1))
                    rcp = awork.tile([1, SC], F32, tag="rcp", bufs=1,
                                     name=f"rcp{h}_{sc}")
                    nc.vector.reciprocal(rcp[:], rs[:])
                    rcpb = wtile(awork, [P, SC], F32, "rcpb", 1)
                    nc.gpsimd.partition_broadcast(rcpb[:], rcp[:])
                    a32 = wtile(awork, [P, SC], F32, "a32", 1)
                    nc.vector.tensor_tensor(a32[:], pv[:], rcpb[:], ALU.mult)
                    nc.vector.tensor_copy(ahi[:, h, :], a32[:])
                    nc.vector.tensor_tensor(alo[:, h, :], a32[:], ahi[:, h, :],
                                            ALU.subtract)

                # publish fp8 attn out + AllGather for column-sharded o_proj
                nc.scalar.dma_start(
                    aag_in8[sc][0:HID_SH, :]
                    .rearrange("(h p) s -> p h s", p=P), ahi[:])
                nc.sync.dma_start(
                    aag_in8[sc][HID_SH:2 * HID_SH, :]
                    .rearrange("(h p) s -> p h s", p=P), alo[:])
                nc.gpsimd.collective_compute(
                    "AllGather", ALU.bypass, replica_groups=RG,
                    ins=[aag_in[sc][:].opt()], outs=[aag_out[sc][:].opt()])

            def emit_o(sc):
                """Column-sharded o_proj for chunk sc with fused residual2."""
                cs = slice(sc * SC, (sc + 1) * SC)
                hh = sc // 2
                cb0 = (sc % 2) * (SC // 2)
                # x (residual stream) hi/lo pair for the fused add
                xp = owork.tile([P, NT_HSH, 2, SC], F8, tag="xp", bufs=1,
                                name=f"xp{sc}")
                nc.scalar.dma_start(xp[:, :, 0, :], pay1_hi(ag1_in[sc]))
                nc.sync.dma_start(xp[:, :, 1, :], pay1_lo(ag1_in[sc]))

                r2f = owork.tile([P, NT_HSH, SC], F32, tag="r2f", bufs=1,
                                 name=f"r2f{sc}")
                r2hi = owork.tile([P, NT_HSH, SC], F8, tag="r2hi", bufs=1,
                                  name=f"r2hi{sc}")
                r2lo = owork.tile([P, NT_HSH, SC], F8, tag="r2lo", bufs=1,
                                  name=f"r2lo{sc}")
                sq = owork.tile([P, NT_HSH, SC], BF16, tag="r2sq", bufs=1,
                                name=f"r2sq{sc}")
                ps2 = rowps.tile([1, SC], F32, tag="row", name=f"ssq2_{sc}")
                pss = [acc.tile([P, SC], F32, tag="acc", name=f"o{m}_{sc}")
                       for m in range(NT_HSH)]
                for g in range(NPH):
                    cb, j = g // 2, g % 2
                    ghi = owork.tile([P, 2, SC], F8, tag="oghi", bufs=4,
                                     name=f"ogh{g}_{sc}")
                    nc.scalar.dma_start(
                        ghi[:], aag_out8[sc][cb, j * 256:(j + 1) * 256, :]
                        .rearrange("(two p) s -> p two s", p=P))
                    glo = owork.tile([P, 2, SC], F8, tag="oglo", bufs=4,
                                     name=f"ogl{g}_{sc}")
                    nc.sync.dma_start(
                        glo[:], aag_out8[sc][cb, HID_SH + j * 256:
                                             HID_SH + (j + 1) * 256, :]
                        .rearrange("(two p) s -> p two s", p=P))
                    for m in range(NT_HSH):
                        nc.tensor.matmul(pss[m][:],
                                         wo_sb_hi[:, g, :, m * P:(m + 1) * P],
                                         ghi[:], start=(g == 0), stop=False,
                                         perf_mode=DR)
                    for m in range(NT_HSH):
                        nc.tensor.matmul(pss[m][:],
                                         wo_sb_hi[:, g, :, m * P:(m + 1) * P],
                                         glo[:], start=False, stop=False,
                                         perf_mode=DR)
                    for m in range(NT_HSH):
                        nc.tensor.matmul(pss[m][:],
                                         wo_sb_lo[:, g, :, m * P:(m + 1) * P],
                                         ghi[:], start=False, stop=False,
                                         perf_mode=DR)
                for m in range(NT_HSH):
                    # += SWO * (xhi + xlo): fused residual add
                    nc.tensor.matmul(pss[m][:], sw_id[:], xp[:, m],
                                     start=False, stop=True, perf_mode=DR)
                    nc.scalar.activation(r2f[:, m, :], pss[m][:], AF.Copy,
                                         scale=CO)
                    nc.vector.tensor_copy(r2hi[:, m, :], r2f[:, m, :])
                    nc.vector.tensor_tensor(r2lo[:, m, :], r2f[:, m, :],
                                            r2hi[:, m, :], ALU.subtract)
                    nc.scalar.activation(sq[:, m, :], r2f[:, m, :], AF.Square)
                    nc.tensor.matmul(ps2[:], ones_bf[:], sq[:, m, :],
                                     start=(m == 0), stop=(m == NT_HSH - 1))
                nc.gpsimd.dma_start(res2_v[:, :, cs], r2f[:])
                nc.gpsimd.dma_start(pay_hi(ag2_in[hh], cb0), r2hi[:])
                nc.gpsimd.dma_start(pay_lo(ag2_in[hh], cb0), r2lo[:])
                ssq2 = owork.tile([1, SC], BF16, tag="ssq2", bufs=2,
                                  name=f"sq2_{sc}")
                nc.vector.tensor_copy(ssq2[:], ps2[:])
                nc.gpsimd.dma_start(
                    ag2_in[hh][HID_SH:HID_SH + 1,
                               (sc % 2) * SC:(sc % 2) * SC + SC], ssq2[:])

            def emit_ag2(hh):
                nc.gpsimd.collective_compute(
                    "AllGather", ALU.bypass, replica_groups=RG,
                    ins=[ag2_in[hh][:].opt()], outs=[ag2_out[hh][:].opt()])

            # =========== era B pools (MLP) ===========
            # created mid-schedule (pool creation reserves SBUF immediately);
            # closures below bind these names at call time.
            b_es = contextlib.ExitStack()
            mpers = mwork = mstr = None
            s2b = None

            def emit_s2b(sc):
                cs = slice(sc * SC, (sc + 1) * SC)
                hh = sc // 2
                hcs = slice((sc % 2) * SC, (sc % 2) * SC + SC)
                srows_b = mwork.tile([8, SC], BF16, tag="srb2", bufs=1,
                                     name=f"sr2b{sc}")
                nc.gpsimd.dma_start(srows_b[:], ag2_v[hh][:, HID_SH, hcs])
                srows = mwork.tile([8, SC], F32, tag="srf2", bufs=1,
                                   name=f"sr2f{sc}")
                nc.vector.tensor_copy(srows[:], srows_b[:])
                ssum = mwork.tile([8, SC], F32, tag="ssum2", bufs=1,
                                  name=f"ss2{sc}")
                nc.gpsimd.partition_all_reduce(ssum[:], srows[:], channels=8,
                                               reduce_op=ReduceOp.add)
                var = mwork.tile([1, SC], F32, tag="var2", bufs=2, name=f"v2{sc}")
                nc.scalar.activation(var[:], ssum[:1, :], AF.Copy,
                                     scale=1.0 / (HID * CD), bias=EPS / CD)
                nc.vector.reciprocal(var[:], var[:])     # = CD / var
                varb2 = mwork.tile([1, SC], BF16, tag="vb2", bufs=2,
                                   name=f"vb2{sc}")
                nc.vector.tensor_copy(varb2[:], var[:])
                nc.gpsimd.partition_broadcast(s2b[:, cs], varb2[:])

            g2pool = {}

            def load_g2(half):
                g2p = g2pool["p"]
                ghi, glo = [], []
                for cb in range(N_CORES):
                    g = g2p.tile([P, 2, 2, SH], F8, tag="g2h", bufs=8,
                                 name=f"g2h{cb}_{half}")
                    for q2 in range(2):
                        qs = slice(q2 * SH // 2, (q2 + 1) * SH // 2)
                        nc.gpsimd.dma_start(
                            g[:, :, :, qs], ag2_8[half][cb, 0:HID_SH, qs]
                            .rearrange("(tp two p) n -> p tp two n",
                                       tp=2, two=2))
                    ghi.append(g)
                    g = g2p.tile([P, 2, 2, SH], F8, tag="g2l", bufs=8,
                                 name=f"g2l{cb}_{half}")
                    for q2 in range(2):
                        qs = slice(q2 * SH // 2, (q2 + 1) * SH // 2)
                        nc.sync.dma_start(
                            g[:, :, :, qs],
                            ag2_8[half][cb, 0:HID_SH,
                                        SH + q2 * SH // 2:
                                        SH + (q2 + 1) * SH // 2]
                            .rearrange("(tp two p) n -> p tp two n",
                                       tp=2, two=2))
                    glo.append(g)
                return ghi, glo


            g2 = {}

            # ============ era A schedule ============
            load_gq(0)
            emit_s1b(0)
            emit_qkv(0)
            load_gq(1)
            emit_s1b(1)
            emit_qkv(1)
            load_gq(2)
            emit_attn(0)
            emit_s1b(2)
            emit_qkv(2)
            load_gq(3)
            emit_attn(1)
            emit_s1b(3)
            emit_qkv(3)
            wq_es.close()
            mpers = b_es.enter_context(tc.tile_pool(name="mpers", bufs=1, side="right"))
            mwork = b_es.enter_context(tc.tile_pool(name="mwork", bufs=1, side="right"))
            wop = wo_es.enter_context(tc.tile_pool(name="wop", bufs=1))
            wo_sb_hi = wop.tile([P, NPH, 2, HID_SH], F8, name="wo_h")
            wo_sb_lo = wop.tile([P, NPH, 2, HID_SH], F8, name="wo_l")
            for g4 in range(0, NPH, 4):
                nc.gpsimd.dma_start(wo_sb_hi[:, g4:g4 + 4],
                                    wslice(wo_hi, 0, m=HID_SH)[:, g4:g4 + 4])
                nc.gpsimd.dma_start(wo_sb_lo[:, g4:g4 + 4],
                                    wslice(wo_lo, 0, m=HID_SH)[:, g4:g4 + 4])
            s2b = mpers.tile([P, S], BF16, name="s2b")
            g2pool["p"] = b_es.enter_context(tc.tile_pool(name="g2p", bufs=1, side="right"))
            emit_attn(2)
            emit_o(0)
            emit_o(1)
            emit_ag2(0)
            emit_attn(3)
            emit_s2b(0)
            emit_s2b(1)
            g2[0] = load_g2(0)
            emit_o(2)
            emit_o(3)
            emit_ag2(1)
            g2[1] = load_g2(1)
            emit_s2b(2)
            emit_s2b(3)
            wo_es.close()
            a_es.close()

            mstr = b_es.enter_context(tc.tile_pool(name="mstr", bufs=1, side="right"))
            utp = b_es.enter_context(tc.tile_pool(name="utp", bufs=1, side="right"))
            ut_hi = [utp.tile([P, 2, S], F8, name=f"uth{g}") for g in range(NPI)]
            ut_lo = [utp.tile([P, 2, S], F8, name=f"utl{g}") for g in range(NPI)]

            def emit_up(half, it_range):
                ghi, glo = g2[half]

                def rhs(lst, g, ncs):
                    return lst[g // 2][:, g % 2, :, ncs]

                for it in it_range:
                    wh = mstr.tile([P, NPH, 2, P], F8, tag="wuh", bufs=2,
                                   name=f"wuh{it}_{half}")
                    nc.scalar.dma_start(wh[:], wslice(wu_hi, it))
                    wl = mstr.tile([P, NPH, 2, P], F8, tag="wul", bufs=2,
                                   name=f"wul{it}_{half}")
                    nc.scalar.dma_start(wl[:], wslice(wu_lo, it))
                    pss = [acc.tile([P, SC], F32, tag="acc", name=f"up{it}_{2*half+ci}")
                           for ci in range(2)]
                    for g in range(NPH):
                        for ci in range(2):
                            nc.tensor.matmul(pss[ci][:], wh[:, g],
                                             rhs(ghi, g, slice(ci * SC, (ci + 1) * SC)),
                                             start=(g == 0), stop=False,
                                             perf_mode=DR)
                    for g in range(NPH):
                        for ci in range(2):
                            nc.tensor.matmul(pss[ci][:], wh[:, g],
                                             rhs(glo, g, slice(ci * SC, (ci + 1) * SC)),
                                             start=False, stop=False, perf_mode=DR)
                    for g in range(NPH):
                        for ci in range(2):
                            nc.tensor.matmul(pss[ci][:], wl[:, g],
                                             rhs(ghi, g, slice(ci * SC, (ci + 1) * SC)),
                                             start=False, stop=(g == NPH - 1),
                                             perf_mode=DR)
                    for ci in range(2):
                        sc = 2 * half + ci
                        cs = slice(sc * SC, (sc + 1) * SC)
                        rl = mwork.tile([P, SC], F32, tag="rl", bufs=2,
                                        name=f"rl{it}_{sc}")
                        nc.scalar.activation(rl[:], pss[ci][:], AF.Relu,
                                             scale=ALPHA_UP)
                        nc.vector.tensor_tensor(rl[:], rl[:], rl[:], ALU.mult)
                        nc.vector.tensor_copy(ut_hi[it // 2][:, it % 2, cs], rl[:])
                        nc.vector.tensor_tensor(ut_lo[it // 2][:, it % 2, cs],
                                                rl[:], ut_hi[it // 2][:, it % 2, cs],
                                                ALU.subtract)

            # ---- era B schedule ----
            emit_up(0, range(NT_INT))
            emit_up(1, range(NT_INT))

            # ---- down proj: 3-term DoubleRow over full S, uneven RS ----
            mstart = 0
            for pi, mc in enumerate(PIECES):
                rs2t = rs2_in[pi]
                rs2_fat = rs2t[:].rearrange("(q p) s -> p q s", p=P)
                for mq in range(mc):
                    m = mstart + mq
                    wh = mstr.tile([P, NPI, 2, P], F8, tag="wdh", bufs=3,
                                   name=f"wdh{m}")
                    nc.scalar.dma_start(wh[:], wslice(wd_hi, m))
                    wl = mstr.tile([P, NPI, 2, P], F8, tag="wdl", bufs=3,
                                   name=f"wdl{m}")
                    nc.scalar.dma_start(wl[:], wslice(wd_lo, m))
                    evf = mwork.tile([P, NSC, SC], BF16, tag="dnev", bufs=1,
                                     name=f"dev{m}")
                    for sc in range(NSC):
                        cs = slice(sc * SC, (sc + 1) * SC)
                        ps = acc.tile([P, SC], F32, tag="acc",
                                      name=f"dn{m}_{sc}")
                        for g in range(NPI):
                            nc.tensor.matmul(ps[:], wh[:, g],
                                             ut_hi[g][:, :, cs],
                                             start=(g == 0), stop=False,
                                             perf_mode=DR)
                        for g in range(NPI):
                            nc.tensor.matmul(ps[:], wh[:, g],
                                             ut_lo[g][:, :, cs],
                                             start=False, stop=False,
                                             perf_mode=DR)
                        for g in range(NPI):
                            nc.tensor.matmul(ps[:], wl[:, g],
                                             ut_hi[g][:, :, cs],
                                             start=False,
                                             stop=(g == NPI - 1),
                                             perf_mode=DR)
                        nc.vector.tensor_tensor(evf[:, sc, :], ps[:],
                                                s2b[:, cs], ALU.mult)
                    nc.sync.dma_start(rs2_fat[:, mq], evf[:])
                nc.gpsimd.collective_compute(
                    "ReduceScatter", ALU.add, replica_groups=RG,
                    ins=[rs2t[:].opt()], outs=[rs2_out[pi][:].opt()])
                orow = mstart * P // N_CORES
                nc.gpsimd.dma_start(
                    out_mlp[orow:orow + mc * P // N_CORES, :], rs2_out[pi][:])
                mstart += mc
            b_es.close()

    nc.compile()
    return nc


def _q8_pair(x):
    x32 = np.asarray(x, np.float32)
    hi = np.asarray(np.clip(x32, -240, 240), F8NP)
    lo = np.asarray(np.clip(x32 - hi.astype(np.float32), -240, 240), F8NP)
    return np.ascontiguousarray(hi), np.ascontiguousarray(lo)


def _lay_tiles(w, mt=P):
    """[K, M] fp8 -> [(M//mt)*128, (K//256)*2*mt] in SBUF tile order.

    Row = mtile*128 + p; cols = (kpair, two, m) flattened, so each per-tile
    DMA is one contiguous [128, (K//256)*2*mt] block."""
    K, M = w.shape
    a = w.reshape(K // 256, 2, P, M // mt, mt).transpose(3, 2, 0, 1, 4)
    return np.ascontiguousarray(a.reshape(M // mt * P, (K // 256) * 2 * mt))


def shard_inputs(positions, hidden_states, residual, qkv_w, o_w, up_w, down_w,
                 ln1_w, ln2_w):
    hTf = np.ascontiguousarray(
        np.asarray(hidden_states).reshape(S, HID).T.astype(ml_dtypes.bfloat16))
    rTf = np.ascontiguousarray(
        np.asarray(residual).reshape(S, HID).T.astype(ml_dtypes.bfloat16))
    pos = np.asarray(positions).reshape(S).astype(np.float64)
    inv = 1.0 / (THETA ** (np.arange(0, DHEAD, 2, dtype=np.float64) / DHEAD))
    fr = pos[:, None] * inv                      # [S, 64]
    cost = np.cos(fr).T.astype(np.float32)       # [64, S]
    sint = np.sin(fr).T.astype(np.float32)
    cos2 = np.ascontiguousarray(
        np.concatenate([cost, cost], 0).astype(ml_dtypes.bfloat16))
    sin_neg = np.ascontiguousarray(
        np.concatenate([-sint, sint], 0).astype(ml_dtypes.bfloat16))
    q_size = N_HEADS * DHEAD
    kv = N_KV * DHEAD
    w1 = np.asarray(qkv_w, np.float32) * np.asarray(ln1_w, np.float32)[:, None] * SW1
    wof = np.asarray(o_w, np.float32) * SWO
    wuf = np.asarray(up_w, np.float32) * np.asarray(ln2_w, np.float32)[:, None] * SWU
    wdf = np.asarray(down_w, np.float32) * SWD
    in_maps = []
    for c in range(N_CORES):
        wq_c = np.concatenate([
            w1[:, c * HQ * DHEAD:(c + 1) * HQ * DHEAD],
            w1[:, q_size + c * DHEAD:q_size + (c + 1) * DHEAD],
            w1[:, q_size + kv + c * DHEAD:q_size + kv + (c + 1) * DHEAD],
        ], axis=1)
        wq_h, wq_l = _q8_pair(wq_c)
        wo_h, wo_l = _q8_pair(wof[:, c * HID_SH:(c + 1) * HID_SH])
        wu_h, wu_l = _q8_pair(wuf[:, c * INT_SH:(c + 1) * INT_SH])
        wd_h, wd_l = _q8_pair(wdf[c * INT_SH:(c + 1) * INT_SH, :])
        wq_h, wq_l = _lay_tiles(wq_h), _lay_tiles(wq_l)
        wo_h, wo_l = _lay_tiles(wo_h, mt=HID_SH), _lay_tiles(wo_l, mt=HID_SH)
        wu_h, wu_l = _lay_tiles(wu_h), _lay_tiles(wu_l)
        wd_h, wd_l = _lay_tiles(wd_h), _lay_tiles(wd_l)
        in_maps.append({
            "hT": np.ascontiguousarray(hTf[c * HID_SH:(c + 1) * HID_SH]),
            "rT": np.ascontiguousarray(rTf[c * HID_SH:(c + 1) * HID_SH]),
            "cos2": cos2, "sin_neg": sin_neg,
            "wq_hi": wq_h, "wq_lo": wq_l,
            "wo_hi": wo_h, "wo_lo": wo_l,
            "wu_hi": wu_h, "wu_lo": wu_l,
            "wd_hi": wd_h, "wd_lo": wd_l,
        })
    return in_maps


_CACHE = {}


def kernel(**inputs):
    from concourse.bass_utils import run_bass_kernel_spmd
    if "nc" not in _CACHE:
        _CACHE["nc"] = build_graph()
    nc = _CACHE["nc"]
    in_maps = shard_inputs(**{k: np.asarray(v) for k, v in inputs.items()})
    res = run_bass_kernel_spmd(nc, in_maps, core_ids=list(range(N_CORES)),
                               trace=False)
    res2T = np.concatenate([res.results[c]["res2T"] for c in range(N_CORES)], axis=0)
    mlpT = np.empty((HID, S), np.float32)
    for c in range(N_CORES):
        mt = res.results[c]["mlpT"]
        mstart = 0
        for pi, mc in enumerate(PIECES):
            rows = mc * P // N_CORES          # rows per core for this piece
            orow = mstart * P // N_CORES
            g0 = mstart * P + c * rows        # global hid row start
            mlpT[g0:g0 + rows] = mt[orow:orow + rows]
            mstart += mc
    mlp_out = np.ascontiguousarray(mlpT.T).reshape(1, S, HID)
    residual2 = np.ascontiguousarray(res2T.T).reshape(1, S, HID)
    return mlp_out, residual2


# revision 38
# speedup vs baseline: 1.0283x; 1.0051x over previous
"""Arcee decoder layer on 8 TRN2 NeuronCores — TP8, fp8 hi/lo DoubleRow.

Sharding (8-way TP, transposed activation layout [hidden, seq] on device):
  - core c owns: q heads 4c..4c+3 + kv head c, residual rows 512c..512c+511,
    intermediate cols 2048c..2048c+2047.
  - Big GEMMs (qkv/o/up/down) run as 3-term hi/lo fp8e4m3 DoubleRow:
    W.x ~= Whi.xhi + Whi.xlo + Wlo.xhi, each term contracting 256 rows per
    0.5-cycle/row matmul. Weights pre-quantized on host (ln1/ln2 and
    per-tensor scales folded); activations split hi/lo on device.
  - RMSNorm: un-normalized residual stream AllGathered with per-core partial
    sum-of-squares row embedded; rsqrt scale folded into PSUM eviction.
  - AG payload [520, 1024] bf16 per S-half: rows 0-511 carry x rows as fp8
    bytes (hi in bf16 cols 0-511, lo in 512-1023), row 512 = bf16 ssq row.
  - o_proj is COLUMN-sharded: per-chunk fp8 hi/lo attention outputs are
    AllGathered (0.5 MiB/rank, much cheaper than the 4 MiB ReduceScatter of
    o partials), then each core computes the full contraction for its own
    512 hid rows. residual2 is fused into the o eviction via an extra
    DoubleRow matmul with a 128*I fp8 identity against the (xhi, xlo) pair,
    so res2 never round-trips through a collective.
  - down_proj partials reduce via bf16 ReduceScatter split into uneven
    pieces so the exposed tail is small.
  - attention (scores/softmax/PV) stays bf16. DMAs are batched into fat
    tiles and spread across queues to keep dispatch off the critical path.
"""
import sys

sys.path.insert(0, "/opt/trn_rl_repo")

import contextlib
import math
import numpy as np
import ml_dtypes

import concourse.bass as bass
import concourse.mybir as mybir
import concourse.tile as tile
from concourse import bacc
from concourse.bass_isa import ReduceOp
from concourse.masks import make_identity

F32 = mybir.dt.float32
BF16 = mybir.dt.bfloat16
F8 = mybir.dt.float8e4
F8E5 = mybir.dt.float8e5
I32 = mybir.dt.int32
AF = mybir.ActivationFunctionType
ALU = mybir.AluOpType
DR = mybir.MatmulPerfMode.DoubleRow
F8NP = ml_dtypes.float8_e4m3

N_CORES = 8
S = 2048
HID = 4096
N_HEADS = 32
N_KV = 8
DHEAD = 128
INTER = 16384
EPS = 1e-5
THETA = 10000.0

HQ = N_HEADS // N_CORES          # 4 q heads per core
HID_SH = HID // N_CORES          # 512 residual rows per core
INT_SH = INTER // N_CORES        # 2048 intermediate per core
NJ = HQ + 2                      # qkv col tiles per core (4q + k + v)
QKV_COLS = NJ * DHEAD            # 768
P = 128
SC = 512                         # seq chunk
NSC = S // SC                    # 4
SH = S // 2                      # 1024 (half)
NT_HID = HID // P                # 32
NT_HSH = HID_SH // P             # 4
NT_INT = INT_SH // P             # 16
NPH = NT_HID // 2                # 16 k-pairs over HID
NPI = NT_INT // 2                # 8 k-pairs over INT_SH
BLK = HID_SH + 8                 # 520 payload rows
TWO_PI = 2.0 * math.pi

# fp8 scales (activations unscaled; weights scaled on host)
SW1 = 1024.0
SWO = 128.0                      # must stay fp8-representable (identity add)
SWU = 1024.0
SWD = 1024.0
SQU = 0.25                       # scale on u = relu(z)^2
CE1 = 1.0 / SW1                  # qkv evict const (with rsqrt row)
CO = 1.0 / SWO                   # o evict const
ALPHA_UP = math.sqrt(SQU) / SWU  # relu evict scale
CD = 1.0 / (SWD * SQU)           # down evict const (with 1/var row)

# down RS pieces (m-tile counts; sum = 32); tapered so the tail is short.
PIECES = [8, 8, 6, 4, 2, 2, 2]

# softmax bias: probs stored as e5m2 exp(s*inv_sqrt_d - XC); max masked
# score*inv_sqrt_d is 12.47 and the min row-max is -4.03 for this input
# distribution, so XC=4 keeps exp in [3e-4, 4.8e3] — inside e5m2 range.
XC = 4.0


def build_graph():
    nc = bacc.Bacc(None, target_bir_lowering=False, debug=False)

    hT = nc.declare_dram_parameter("hT", [HID_SH, S], BF16, isOutput=False)
    rT = nc.declare_dram_parameter("rT", [HID_SH, S], BF16, isOutput=False)
    cos_in = nc.declare_dram_parameter("cos2", [P, S], BF16, isOutput=False)
    sin_in = nc.declare_dram_parameter("sin_neg", [P, S], BF16, isOutput=False)
    # weights arrive pre-laid-out in SBUF tile order (see _lay_tiles):
    # row = mtile*128 + p, cols = (t, two, m) flattened — every per-tile DMA
    # is a contiguous [128, X] block (full-width descriptors).
    wq_hi = nc.declare_dram_parameter("wq_hi", [NJ * P, NPH * 2 * P], F8, isOutput=False)
    wq_lo = nc.declare_dram_parameter("wq_lo", [NJ * P, NPH * 2 * P], F8, isOutput=False)
    wo_hi = nc.declare_dram_parameter("wo_hi", [P, NPH * 2 * HID_SH], F8, isOutput=False)
    wo_lo = nc.declare_dram_parameter("wo_lo", [P, NPH * 2 * HID_SH], F8, isOutput=False)
    wu_hi = nc.declare_dram_parameter("wu_hi", [NT_INT * P, NPH * 2 * P], F8, isOutput=False)
    wu_lo = nc.declare_dram_parameter("wu_lo", [NT_INT * P, NPH * 2 * P], F8, isOutput=False)
    wd_hi = nc.declare_dram_parameter("wd_hi", [NT_HID * P, NPI * 2 * P], F8, isOutput=False)
    wd_lo = nc.declare_dram_parameter("wd_lo", [NT_HID * P, NPI * 2 * P], F8, isOutput=False)
    out_res2 = nc.declare_dram_parameter("res2T", [HID_SH, S], F32, isOutput=True)
    out_mlp = nc.declare_dram_parameter("mlpT", [HID_SH, S], F32, isOutput=True)

    RG = [list(range(N_CORES))]
    inv_sqrt_d = 1.0 / math.sqrt(DHEAD)

    # per-tile contiguous weight slices -> [p, t(pair), two, m]
    def wslice(w, i, m=P):
        return w[i * P:(i + 1) * P, :].rearrange("p (t two m) -> p t two m",
                                                 two=2, m=m)

    hT_v = hT[:].rearrange("(i p) s -> p i s", p=P)
    rT_v = rT[:].rearrange("(i p) s -> p i s", p=P)
    res2_v = out_res2[:].rearrange("(i p) s -> p i s", p=P)

    with tile.TileContext(nc) as tc:
        with contextlib.ExitStack() as ctx:
            const = ctx.enter_context(tc.tile_pool(name="const", bufs=1))
            acc = ctx.enter_context(tc.tile_pool(name="acc", bufs=6, space="PSUM"))
            rowps = ctx.enter_context(tc.tile_pool(name="rowps", bufs=1, space="PSUM"))
            tpps = ctx.enter_context(tc.tile_pool(name="tpps", bufs=1, space="PSUM"))
            dram = ctx.enter_context(tc.tile_pool(name="dram", bufs=1, space="DRAM"))

            ones_bf = const.tile([P, 1], BF16)
            nc.vector.memset(ones_bf[:], 1.0)
            # DR lhsT needs pair-step %16==0, so the rowsum ones tile is
            # [P, 2, 16] (rows 0-15 of the result all carry the same sum)
            ones_f5p = const.tile([P, 2, 16], F8E5)
            nc.vector.memset(ones_f5p[:], 1.0)
            negxc = const.tile([P, 1], F32)
            nc.vector.memset(negxc[:], -XC)

            # DRAM scratch
            ag1_in = [dram.tile([BLK, SC], BF16, name=f"ag1_in{s_}")
                      for s_ in range(NSC)]
            ag1_out = [dram.tile([N_CORES * BLK, SC], BF16, name=f"ag1_out{s_}",
                                 addr_space="Shared") for s_ in range(NSC)]
            ag2_in = [dram.tile([BLK, SH], BF16, name=f"ag2_in{h}") for h in range(2)]
            ag2_out = [dram.tile([N_CORES * BLK, SH], BF16, name=f"ag2_out{h}",
                                 addr_space="Shared") for h in range(2)]
            # per-chunk attention-out AG: fp8 hi (rows 0-511) + lo (512-1023)
            aag_in = [dram.tile([2 * HID_SH, SC // 2], BF16, name=f"aag_in{sc}")
                      for sc in range(NSC)]
            aag_out = [dram.tile([N_CORES * 2 * HID_SH, SC // 2], BF16,
                                 name=f"aag_out{sc}", addr_space="Shared")
                       for sc in range(NSC)]
            rs2_in = [dram.tile([mc * P, S], BF16, name=f"rs2_in{pi}")
                      for pi, mc in enumerate(PIECES)]
            rs2_out = [dram.tile([mc * P // N_CORES, S], BF16,
                                 name=f"rs2_out{pi}")
                       for pi, mc in enumerate(PIECES)]

            ag1_v = [t[:].rearrange("(c r) s -> c r s", r=BLK) for t in ag1_out]
            ag2_v = [t[:].rearrange("(c r) s -> c r s", r=BLK) for t in ag2_out]
            ag1_8 = [t[:].bitcast(F8).rearrange("(c r) s -> c r s", r=BLK)
                     for t in ag1_out]
            ag2_8 = [t[:].bitcast(F8).rearrange("(c r) s -> c r s", r=BLK)
                     for t in ag2_out]

            # per-chunk ag1 payload regions (hi fp8 | lo fp8, + ssq row)
            def pay1_hi(t):
                return t[0:HID_SH, 0:SC // 2].bitcast(F8) \
                    .rearrange("(i p) s -> p i s", p=P)

            def pay1_lo(t):
                return t[0:HID_SH, SC // 2:SC].bitcast(F8) \
                    .rearrange("(i p) s -> p i s", p=P)
            aag_in8 = [t[:].bitcast(F8) for t in aag_in]           # [1024, SC]
            aag_out8 = [t[:].bitcast(F8).rearrange("(c r) s -> c r s",
                                                   r=2 * HID_SH)
                        for t in aag_out]                          # [8,1024,SC]

            # payload hi/lo region views as [p, i, s] fp8
            def pay_hi(t, cb0):
                return t[0:HID_SH, cb0:cb0 + SC // 2].bitcast(F8) \
                    .rearrange("(i p) s -> p i s", p=P)

            def pay_lo(t, cb0):
                return t[0:HID_SH, SH // 2 + cb0:SH // 2 + cb0 + SC // 2] \
                    .bitcast(F8).rearrange("(i p) s -> p i s", p=P)

            # =========== era A pools (attention + residual stream) ===========
            a_es = contextlib.ExitStack()
            apers = a_es.enter_context(tc.tile_pool(name="apers", bufs=1))
            awork = a_es.enter_context(tc.tile_pool(name="awork", bufs=1))
            owork = a_es.enter_context(tc.tile_pool(name="owork", bufs=1))

            _cnt = [0]

            def wtile(pool, shape, dt, tag, bufs):
                _cnt[0] += 1
                return pool.tile(shape, dt, tag=tag, bufs=bufs,
                                 name=f"t_{_cnt[0]}")

            # wq weight cache: created + loaded first so the 6MB of weight
            # DMA streams during phase-1 compute with nothing ahead of it
            wq_es = contextlib.ExitStack()
            wqw = wq_es.enter_context(tc.tile_pool(name="wqw", bufs=1))
            wq_sb_hi = [wqw.tile([P, NPH, 2, P], F8, name=f"wqh{j}") for j in range(NJ)]
            wq_sb_lo = [wqw.tile([P, NPH, 2, P], F8, name=f"wql{j}") for j in range(NJ)]
            for j in range(NJ):
                nc.scalar.dma_start(wq_sb_hi[j][:], wslice(wq_hi, j))
            for j in range(NJ):
                nc.scalar.dma_start(wq_sb_lo[j][:], wslice(wq_lo, j))

            # ---- phase 1: x = h + r; hi/lo fp8 + ssq into payload ----
            with tc.tile_pool(name="p1", bufs=1) as p1:
                for sc in range(NSC):
                    cs = slice(sc * SC, (sc + 1) * SC)
                    ps = rowps.tile([1, SC], F32, tag="row", name=f"ssq1p{sc}")
                    hf = wtile(p1, [P, NT_HSH, SC], BF16, "hf", 2)
                    rf = wtile(p1, [P, NT_HSH, SC], BF16, "rf", 2)
                    nc.sync.dma_start(hf[:], hT_v[:, :, cs])
                    nc.sync.dma_start(rf[:], rT_v[:, :, cs])
                    xt = wtile(p1, [P, NT_HSH, SC], F32, "xt", 2)
                    nc.vector.tensor_tensor(xt[:], hf[:], rf[:], ALU.add)
                    xhi = wtile(p1, [P, NT_HSH, SC], F8, "xhi", 2)
                    nc.vector.tensor_copy(xhi[:], xt[:])
                    xlo = wtile(p1, [P, NT_HSH, SC], F8, "xlo", 2)
                    nc.vector.tensor_tensor(xlo[:], xt[:], xhi[:], ALU.subtract)
                    nc.gpsimd.dma_start(pay1_hi(ag1_in[sc]), xhi[:])
                    nc.sync.dma_start(pay1_lo(ag1_in[sc]), xlo[:])
                    sq = wtile(p1, [P, NT_HSH, SC], BF16, "sq", 2)
                    nc.vector.tensor_tensor(sq[:], xt[:], xt[:], ALU.mult)
                    for i in range(NT_HSH):
                        nc.tensor.matmul(ps[:], ones_bf[:], sq[:, i, :],
                                         start=(i == 0), stop=(i == NT_HSH - 1))
                    ssq1b = awork.tile([1, SC], BF16, tag="ssq1b", bufs=2,
                                       name=f"ssq1b{sc}")
                    nc.vector.tensor_copy(ssq1b[:], ps[:])
                    nc.sync.dma_start(
                        ag1_in[sc][HID_SH:HID_SH + 1, 0:SC], ssq1b[:])
                    nc.gpsimd.collective_compute(
                        "AllGather", ALU.bypass, replica_groups=RG,
                        ins=[ag1_in[sc][:].opt()], outs=[ag1_out[sc][:].opt()])

            # ---- rope tables + masks (after AGs so phase-1 wins queues) ----
            ident = apers.tile([P, P], BF16)
            make_identity(nc, ident[:])
            cos2 = apers.tile([P, S], BF16)
            sin_neg = apers.tile([P, S], BF16)
            cmask = []
            for j in range(SC // P):
                mk = apers.tile([P, SC], BF16, name=f"cmask{j}")
                nc.vector.memset(mk[:], 1.0)
                nc.gpsimd.affine_select(mk[:], mk[:], pattern=[[1, SC]],
                                        base=-j * P, channel_multiplier=-1,
                                        compare_op=ALU.is_ge, fill=0.0)
                cmask.append(mk)

            nc.sync.dma_start(cos2[:], cos_in[:])
            nc.sync.dma_start(sin_neg[:], sin_in[:])

            # fp8 identity * SWO for the fused residual add in o_proj
            sw_id = apers.tile([P, 2, P], F8, name="sw_id")
            nc.scalar.activation(sw_id[:, 0, :], ident[:], AF.Copy, scale=SWO)
            nc.scalar.activation(sw_id[:, 1, :], ident[:], AF.Copy, scale=SWO)

            # ---- persistent attention-era tiles ----
            kT = apers.tile([P, S], BF16, name="kT")
            vT = apers.tile([P, S], BF16, name="vT")
            s1b = apers.tile([P, S], BF16, name="s1b")

            # wo SBUF cache: created late, in the region wqp frees
            wo_es = contextlib.ExitStack()

            # gathered-x tiles (freed after qkv3, with the wq weights)
            wqp = wq_es.enter_context(tc.tile_pool(name="gqp", bufs=1))

            def emit_s1b(sc):
                cs = slice(sc * SC, (sc + 1) * SC)
                srows_b = awork.tile([8, SC], BF16, tag="srb", bufs=1,
                                     name=f"sr1b{sc}")
                nc.gpsimd.dma_start(srows_b[:], ag1_v[sc][:, HID_SH, 0:SC])
                srows = awork.tile([8, SC], F32, tag="srf", bufs=1,
                                   name=f"sr1f{sc}")
                nc.vector.tensor_copy(srows[:], srows_b[:])
                ssum = awork.tile([8, SC], F32, tag="ssum", bufs=1,
                                  name=f"ss1{sc}")
                nc.gpsimd.partition_all_reduce(ssum[:], srows[:], channels=8,
                                               reduce_op=ReduceOp.add)
                var = awork.tile([1, SC], F32, tag="var", bufs=1, name=f"v1{sc}")
                nc.scalar.activation(var[:], ssum[:1, :], AF.Copy,
                                     scale=1.0 / HID, bias=EPS)
                nc.vector.reciprocal(var[:], var[:])
                varb = awork.tile([1, SC], BF16, tag="varb", bufs=1,
                                  name=f"v1b{sc}")
                nc.scalar.activation(varb[:], var[:], AF.Sqrt, scale=CE1 * CE1)
                nc.gpsimd.partition_broadcast(s1b[:, cs], varb[:])

            qcs = {}
            gqs = {}

            def load_gq(sc):
                ghi, glo = [], []
                for cb in range(N_CORES):
                    g = wqp.tile([P, 2, 2, SC], F8, tag="ghi", bufs=16,
                                 name=f"gh{cb}_{sc}")
                    nc.gpsimd.dma_start(
                        g[:], ag1_8[sc][cb, 0:HID_SH, 0:SC]
                        .rearrange("(tp two p) n -> p tp two n", tp=2, two=2))
                    ghi.append(g)
                    g = wqp.tile([P, 2, 2, SC], F8, tag="glo", bufs=16,
                                 name=f"gl{cb}_{sc}")
                    nc.sync.dma_start(
                        g[:], ag1_8[sc][cb, 0:HID_SH, SC:2 * SC]
                        .rearrange("(tp two p) n -> p tp two n", tp=2, two=2))
                    glo.append(g)
                gqs[sc] = (ghi, glo)

            def emit_qkv(sc):
                cs = slice(sc * SC, (sc + 1) * SC)
                if sc not in gqs:
                    load_gq(sc)
                ghi, glo = gqs[sc]

                def rhs(lst, g):
                    return lst[g // 2][:, g % 2]

                qc = {}
                pss = [acc.tile([P, SC], F32, tag="acc", name=f"qk{j}_{sc}")
                       for j in range(NJ)]
                for g in range(NPH):
                    for j in range(NJ):
                        nc.tensor.matmul(pss[j][:], wq_sb_hi[j][:, g],
                                         rhs(ghi, g), start=(g == 0),
                                         stop=False, perf_mode=DR)
                for g in range(NPH):
                    for j in range(NJ):
                        nc.tensor.matmul(pss[j][:], wq_sb_hi[j][:, g],
                                         rhs(glo, g), start=False, stop=False,
                                         perf_mode=DR)
                for g in range(NPH):
                    for j in range(NJ):
                        nc.tensor.matmul(pss[j][:], wq_sb_lo[j][:, g],
                                         rhs(ghi, g), start=False,
                                         stop=(g == NPH - 1), perf_mode=DR)
                for j in range(NJ):
                    if j < HQ:
                        dst = awork.tile([P, SC], BF16, tag="qc", bufs=8,
                                         name=f"qc{j}_{sc}")
                        qc[j] = dst
                        nc.vector.tensor_tensor(dst[:], pss[j][:], s1b[:, cs],
                                                ALU.mult)
                    else:
                        dst = kT if j == HQ else vT
                        nc.vector.tensor_tensor(dst[:, cs], pss[j][:],
                                                s1b[:, cs], ALU.mult)
                qcs[sc] = qc

            def emit_attn(sc):
                cs = slice(sc * SC, (sc + 1) * SC)
                qc = qcs[sc]
                # rope on q tiles + k chunk
                for j in range(HQ + 1):
                    tv = qc[j][:] if j < HQ else kT[:, cs]
                    swp = wtile(awork, [P, SC], BF16, "t1k", 3)
                    nc.sync.dma_start(swp[:64, :], tv[64:, :])
                    nc.sync.dma_start(swp[64:, :], tv[:64, :])
                    m1 = wtile(awork, [P, SC], BF16, "t1k", 3)
                    nc.vector.tensor_tensor(m1[:], tv, cos2[:, cs], ALU.mult)
                    m2 = wtile(awork, [P, SC], BF16, "t1k", 3)
                    nc.vector.tensor_tensor(m2[:], swp[:], sin_neg[:, cs], ALU.mult)
                    nc.vector.tensor_tensor(tv, m1[:], m2[:], ALU.add)

                # v transpose in place
                for t in range(sc * (SC // P), (sc + 1) * (SC // P)):
                    pst = tpps.tile([P, P], BF16, tag="tp", name=f"tp{t}")
                    nc.tensor.transpose(pst[:], vT[:, t * P:(t + 1) * P], ident[:])
                    nc.vector.tensor_copy(vT[:, t * P:(t + 1) * P], pst[:])

                # attention: 4 heads x this chunk; fp8 hi/lo attn output
                nsk = (sc + 1) * (SC // P)
                ahi = awork.tile([P, HQ, SC], F8, tag="ahi", bufs=1,
                                 name=f"ahi{sc}")
                alo = awork.tile([P, HQ, SC], F8, tag="alo", bufs=1,
                                 name=f"alo{sc}")
                for h in range(HQ):
                    pv = acc.tile([P, SC], F32, tag="acc", name=f"pv{h}_{sc}")
                    rs = rowps.tile([1, SC], F32, tag="row", name=f"rs{h}_{sc}")
                    for skt in range(nsk):
                        sps = acc.tile([P, SC], F32, tag="acc",
                                       name=f"s{h}_{sc}_{skt}")
                        nc.tensor.matmul(sps[:], kT[:, skt * P:(skt + 1) * P],
                                         qc[h][:], start=True, stop=True)
                        ex = wtile(awork, [P, SC], BF16, "ex", 3)
                        nc.scalar.activation(ex[:], sps[:], AF.Exp,
                                             scale=inv_sqrt_d)
                        if skt >= 4 * sc:
                            nc.vector.tensor_tensor(ex[:], ex[:],
                                                    cmask[skt - 4 * sc][:],
                                                    ALU.mult)
                        nc.tensor.matmul(rs[:], ones_bf[:], ex[:],
                                         start=(skt == 0), stop=(skt == nsk - 1))
                        nc.tensor.matmul(pv[:], vT[:, skt * P:(skt + 1) * P],
                                         ex[:], start=(skt == 0),
                                         stop=(skt == nsk - 1))
                    rcp = awork.tile([1, SC], F32, tag="rcp", bufs=1,
                                     name=f"rcp{h}_{sc}")
                    nc.vector.reciprocal(rcp[:], rs[:])
                    rcpb = wtile(awork, [P, SC], F32, "rcpb", 1)
                    nc.gpsimd.partition_broadcast(rcpb[:], rcp[:])
                    a32 = wtile(awork, [P, SC], F32, "a32", 1)
                    nc.vector.tensor_tensor(a32[:], pv[:], rcpb[:], ALU.mult)
                    nc.vector.tensor_copy(ahi[:, h, :], a32[:])
                    nc.vector.tensor_tensor(alo[:, h, :], a32[:], ahi[:, h, :],
                                            ALU.subtract)

                # publish fp8 attn out + AllGather for column-sharded o_proj
                nc.scalar.dma_start(
                    aag_in8[sc][0:HID_SH, :]
                    .rearrange("(h p) s -> p h s", p=P), ahi[:])
                nc.sync.dma_start(
                    aag_in8[sc][HID_SH:2 * HID_SH, :]
                    .rearrange("(h p) s -> p h s", p=P), alo[:])
                nc.gpsimd.collective_compute(
                    "AllGather", ALU.bypass, replica_groups=RG,
                    ins=[aag_in[sc][:].opt()], outs=[aag_out[sc][:].opt()])

            def emit_o(sc):
                """Column-sharded o_proj for chunk sc with fused residual2."""
                cs = slice(sc * SC, (sc + 1) * SC)
                hh = sc // 2
                cb0 = (sc % 2) * (SC // 2)
                # x (residual stream) hi/lo pair for the fused add
                xp = owork.tile([P, NT_HSH, 2, SC], F8, tag="xp", bufs=1,
                                name=f"xp{sc}")
                nc.scalar.dma_start(xp[:, :, 0, :], pay1_hi(ag1_in[sc]))
                nc.sync.dma_start(xp[:, :, 1, :], pay1_lo(ag1_in[sc]))

                r2f = owork.tile([P, NT_HSH, SC], F32, tag="r2f", bufs=1,
                                 name=f"r2f{sc}")
                r2hi = owork.tile([P, NT_HSH, SC], F8, tag="r2hi", bufs=1,
                                  name=f"r2hi{sc}")
                r2lo = owork.tile([P, NT_HSH, SC], F8, tag="r2lo", bufs=1,
                                  name=f"r2lo{sc}")
                sq = owork.tile([P, NT_HSH, SC], BF16, tag="r2sq", bufs=1,
                                name=f"r2sq{sc}")
                ps2 = rowps.tile([1, SC], F32, tag="row", name=f"ssq2_{sc}")
                pss = [acc.tile([P, SC], F32, tag="acc", name=f"o{m}_{sc}")
                       for m in range(NT_HSH)]
                for g in range(NPH):
                    cb, j = g // 2, g % 2
                    ghi = owork.tile([P, 2, SC], F8, tag="oghi", bufs=4,
                                     name=f"ogh{g}_{sc}")
                    nc.scalar.dma_start(
                        ghi[:], aag_out8[sc][cb, j * 256:(j + 1) * 256, :]
                        .rearrange("(two p) s -> p two s", p=P))
                    glo = owork.tile([P, 2, SC], F8, tag="oglo", bufs=4,
                                     name=f"ogl{g}_{sc}")
                    nc.sync.dma_start(
                        glo[:], aag_out8[sc][cb, HID_SH + j * 256:
                                             HID_SH + (j + 1) * 256, :]
                        .rearrange("(two p) s -> p two s", p=P))
                    for m in range(NT_HSH):
                        nc.tensor.matmul(pss[m][:],
                                         wo_sb_hi[:, g, :, m * P:(m + 1) * P],
                                         ghi[:], start=(g == 0), stop=False,
                                         perf_mode=DR)
                    for m in range(NT_HSH):
                        nc.tensor.matmul(pss[m][:],
                                         wo_sb_hi[:, g, :, m * P:(m + 1) * P],
                                         glo[:], start=False, stop=False,
                                         perf_mode=DR)
                    for m in range(NT_HSH):
                        nc.tensor.matmul(pss[m][:],
                                         wo_sb_lo[:, g, :, m * P:(m + 1) * P],
                                         ghi[:], start=False, stop=False,
                                         perf_mode=DR)
                for m in range(NT_HSH):
                    # += SWO * (xhi + xlo): fused residual add
                    nc.tensor.matmul(pss[m][:], sw_id[:], xp[:, m],
                                     start=False, stop=True, perf_mode=DR)
                    nc.scalar.activation(r2f[:, m, :], pss[m][:], AF.Copy,
                                         scale=CO)
                    nc.vector.tensor_copy(r2hi[:, m, :], r2f[:, m, :])
                    nc.vector.tensor_tensor(r2lo[:, m, :], r2f[:, m, :],
                                            r2hi[:, m, :], ALU.subtract)
                    nc.scalar.activation(sq[:, m, :], r2f[:, m, :], AF.Square)
                    nc.tensor.matmul(ps2[:], ones_bf[:], sq[:, m, :],
                                     start=(m == 0), stop=(m == NT_HSH - 1))
                nc.gpsimd.dma_start(res2_v[:, :, cs], r2f[:])
                nc.gpsimd.dma_start(pay_hi(ag2_in[hh], cb0), r2hi[:])
                nc.gpsimd.dma_start(pay_lo(ag2_in[hh], cb0), r2lo[:])
                ssq2 = owork.tile([1, SC], BF16, tag="ssq2", bufs=2,
                                  name=f"sq2_{sc}")
                nc.vector.tensor_copy(ssq2[:], ps2[:])
                nc.gpsimd.dma_start(
                    ag2_in[hh][HID_SH:HID_SH + 1,
                               (sc % 2) * SC:(sc % 2) * SC + SC], ssq2[:])

            def emit_ag2(hh):
                nc.gpsimd.collective_compute(
                    "AllGather", ALU.bypass, replica_groups=RG,
                    ins=[ag2_in[hh][:].opt()], outs=[ag2_out[hh][:].opt()])

            # =========== era B pools (MLP) ===========
            # created mid-schedule (pool creation reserves SBUF immediately);
            # closures below bind these names at call time.
            b_es = contextlib.ExitStack()
            mpers = mwork = mstr = None
            s2b = None

            def emit_s2b(sc):
                cs = slice(sc * SC, (sc + 1) * SC)
                hh = sc // 2
                hcs = slice((sc % 2) * SC, (sc % 2) * SC + SC)
                srows_b = mwork.tile([8, SC], BF16, tag="srb2", bufs=1,
                                     name=f"sr2b{sc}")
                nc.gpsimd.dma_start(srows_b[:], ag2_v[hh][:, HID_SH, hcs])
                srows = mwork.tile([8, SC], F32, tag="srf2", bufs=1,
                                   name=f"sr2f{sc}")
                nc.vector.tensor_copy(srows[:], srows_b[:])
                ssum = mwork.tile([8, SC], F32, tag="ssum2", bufs=1,
                                  name=f"ss2{sc}")
                nc.gpsimd.partition_all_reduce(ssum[:], srows[:], channels=8,
                                               reduce_op=ReduceOp.add)
                var = mwork.tile([1, SC], F32, tag="var2", bufs=2, name=f"v2{sc}")
                nc.scalar.activation(var[:], ssum[:1, :], AF.Copy,
                                     scale=1.0 / (HID * CD), bias=EPS / CD)
                nc.vector.reciprocal(var[:], var[:])     # = CD / var
                varb2 = mwork.tile([1, SC], BF16, tag="vb2", bufs=2,
                                   name=f"vb2{sc}")
                nc.vector.tensor_copy(varb2[:], var[:])
                nc.gpsimd.partition_broadcast(s2b[:, cs], varb2[:])

            g2pool = {}

            def load_g2(half):
                g2p = g2pool["p"]
                ghi, glo = [], []
                for cb in range(N_CORES):
                    g = g2p.tile([P, 2, 2, SH], F8, tag="g2h", bufs=8,
                                 name=f"g2h{cb}_{half}")
                    for q2 in range(2):
                        qs = slice(q2 * SH // 2, (q2 + 1) * SH // 2)
                        nc.gpsimd.dma_start(
                            g[:, :, :, qs], ag2_8[half][cb, 0:HID_SH, qs]
                            .rearrange("(tp two p) n -> p tp two n",
                                       tp=2, two=2))
                    ghi.append(g)
                    g = g2p.tile([P, 2, 2, SH], F8, tag="g2l", bufs=8,
                                 name=f"g2l{cb}_{half}")
                    for q2 in range(2):
                        qs = slice(q2 * SH // 2, (q2 + 1) * SH // 2)
                        nc.sync.dma_start(
                            g[:, :, :, qs],
                            ag2_8[half][cb, 0:HID_SH,
                                        SH + q2 * SH // 2:
                                        SH + (q2 + 1) * SH // 2]
                            .rearrange("(tp two p) n -> p tp two n",
                                       tp=2, two=2))
                    glo.append(g)
                return ghi, glo


            g2 = {}

            # ============ era A schedule ============
            load_gq(0)
            emit_s1b(0)
            emit_qkv(0)
            load_gq(1)
            emit_s1b(1)
            emit_qkv(1)
            load_gq(2)
            emit_attn(0)
            emit_s1b(2)
            emit_qkv(2)
            load_gq(3)
            emit_attn(1)
            emit_s1b(3)
            emit_qkv(3)
            wq_es.close()
            mpers = b_es.enter_context(tc.tile_pool(name="mpers", bufs=1, side="right"))
            mwork = b_es.enter_context(tc.tile_pool(name="mwork", bufs=1, side="right"))
            wop = wo_es.enter_context(tc.tile_pool(name="wop", bufs=1))
            wo_sb_hi = wop.tile([P, NPH, 2, HID_SH], F8, name="wo_h")
            wo_sb_lo = wop.tile([P, NPH, 2, HID_SH], F8, name="wo_l")
            for g4 in range(0, NPH, 4):
                nc.gpsimd.dma_start(wo_sb_hi[:, g4:g4 + 4],
                                    wslice(wo_hi, 0, m=HID_SH)[:, g4:g4 + 4])
                nc.gpsimd.dma_start(wo_sb_lo[:, g4:g4 + 4],
                                    wslice(wo_lo, 0, m=HID_SH)[:, g4:g4 + 4])
            s2b = mpers.tile([P, S], BF16, name="s2b")
            g2pool["p"] = b_es.enter_context(tc.tile_pool(name="g2p", bufs=1, side="right"))
            emit_attn(2)
            emit_o(0)
            emit_o(1)
            emit_ag2(0)
            emit_attn(3)
            emit_s2b(0)
            emit_s2b(1)
            g2[0] = load_g2(0)
            emit_o(2)
            emit_o(3)
            emit_ag2(1)
            g2[1] = load_g2(1)
            emit_s2b(2)
            emit_s2b(3)
            wo_es.close()
            a_es.close()

            mstr = b_es.enter_context(tc.tile_pool(name="mstr", bufs=1, side="right"))
            utp = b_es.enter_context(tc.tile_pool(name="utp", bufs=1, side="right"))
            ut_hi = [utp.tile([P, 2, S], F8, name=f"uth{g}") for g in range(NPI)]
            ut_lo = [utp.tile([P, 2, S], F8, name=f"utl{g}") for g in range(NPI)]

            def emit_up(half, it_range):
                ghi, glo = g2[half]

                def rhs(lst, g, ncs):
                    return lst[g // 2][:, g % 2, :, ncs]

                for it in it_range:
                    wh = mstr.tile([P, NPH, 2, P], F8, tag="wuh", bufs=2,
                                   name=f"wuh{it}_{half}")
                    nc.scalar.dma_start(wh[:], wslice(wu_hi, it))
                    wl = mstr.tile([P, NPH, 2, P], F8, tag="wul", bufs=2,
                                   name=f"wul{it}_{half}")
                    nc.scalar.dma_start(wl[:], wslice(wu_lo, it))
                    pss = [acc.tile([P, SC], F32, tag="acc", name=f"up{it}_{2*half+ci}")
                           for ci in range(2)]
                    for g in range(NPH):
                        for ci in range(2):
                            nc.tensor.matmul(pss[ci][:], wh[:, g],
                                             rhs(ghi, g, slice(ci * SC, (ci + 1) * SC)),
                                             start=(g == 0), stop=False,
                                             perf_mode=DR)
                    for g in range(NPH):
                        for ci in range(2):
                            nc.tensor.matmul(pss[ci][:], wh[:, g],
                                             rhs(glo, g, slice(ci * SC, (ci + 1) * SC)),
                                             start=False, stop=False, perf_mode=DR)
                    for g in range(NPH):
                        for ci in range(2):
                            nc.tensor.matmul(pss[ci][:], wl[:, g],
                                             rhs(ghi, g, slice(ci * SC, (ci + 1) * SC)),
                                             start=False, stop=(g == NPH - 1),
                                             perf_mode=DR)
                    for ci in range(2):
                        sc = 2 * half + ci
                        cs = slice(sc * SC, (sc + 1) * SC)
                        rl = mwork.tile([P, SC], F32, tag="rl", bufs=2,
                                        name=f"rl{it}_{sc}")
                        nc.scalar.activation(rl[:], pss[ci][:], AF.Relu,
                                             scale=ALPHA_UP)
                        nc.vector.tensor_tensor(rl[:], rl[:], rl[:], ALU.mult)
                        nc.vector.tensor_copy(ut_hi[it // 2][:, it % 2, cs], rl[:])
                        nc.vector.tensor_tensor(ut_lo[it // 2][:, it % 2, cs],
                                                rl[:], ut_hi[it // 2][:, it % 2, cs],
                                                ALU.subtract)

            # ---- era B schedule ----
            emit_up(0, range(NT_INT))
            emit_up(1, range(NT_INT))

            # ---- down proj: 3-term DoubleRow over full S, uneven RS ----
            mstart = 0
            for pi, mc in enumerate(PIECES):
                rs2t = rs2_in[pi]
                rs2_fat = rs2t[:].rearrange("(q p) s -> p q s", p=P)
                for mq in range(mc):
                    m = mstart + mq
                    wh = mstr.tile([P, NPI, 2, P], F8, tag="wdh", bufs=3,
                                   name=f"wdh{m}")
                    nc.scalar.dma_start(wh[:], wslice(wd_hi, m))
                    wl = mstr.tile([P, NPI, 2, P], F8, tag="wdl", bufs=3,
                                   name=f"wdl{m}")
                    nc.scalar.dma_start(wl[:], wslice(wd_lo, m))
                    evf = mwork.tile([P, NSC, SC], BF16, tag="dnev", bufs=1,
                                     name=f"dev{m}")
                    for sc in range(NSC):
                        cs = slice(sc * SC, (sc + 1) * SC)
                        ps = acc.tile([P, SC], F32, tag="acc",
                                      name=f"dn{m}_{sc}")
                        for g in range(NPI):
                            nc.tensor.matmul(ps[:], wh[:, g],
                                             ut_hi[g][:, :, cs],
                                             start=(g == 0), stop=False,
                                             perf_mode=DR)
                        for g in range(NPI):
                            nc.tensor.matmul(ps[:], wh[:, g],
                                             ut_lo[g][:, :, cs],
                                             start=False, stop=False,
                                             perf_mode=DR)
                        for g in range(NPI):
                            nc.tensor.matmul(ps[:], wl[:, g],
                                             ut_hi[g][:, :, cs],
                                             start=False,
                                             stop=(g == NPI - 1),
                                             perf_mode=DR)
                        nc.vector.tensor_tensor(evf[:, sc, :], ps[:],
                                                s2b[:, cs], ALU.mult)
                    nc.sync.dma_start(rs2_fat[:, mq], evf[:])
                nc.gpsimd.collective_compute(
                    "ReduceScatter", ALU.add, replica_groups=RG,
                    ins=[rs2t[:].opt()], outs=[rs2_out[pi][:].opt()])
                orow = mstart * P // N_CORES
                nc.gpsimd.dma_start(
                    out_mlp[orow:orow + mc * P // N_CORES, :], rs2_out[pi][:])
                mstart += mc
            b_es.close()

    nc.compile()
    return nc


def _q8_pair(x):
    x32 = np.asarray(x, np.float32)
    hi = np.asarray(np.clip(x32, -240, 240), F8NP)
    lo = np.asarray(np.clip(x32 - hi.astype(np.float32), -240, 240), F8NP)
    return np.ascontiguousarray(hi), np.ascontiguousarray(lo)


def _lay_tiles(w, mt=P):
    """[K, M] fp8 -> [(M//mt)*128, (K//256)*2*mt] in SBUF tile order.

    Row = mtile*128 + p; cols = (kpair, two, m) flattened, so each per-tile
    DMA is one contiguous [128, (K//256)*2*mt] block."""
    K, M = w.shape
    a = w.reshape(K // 256, 2, P, M // mt, mt).transpose(3, 2, 0, 1, 4)
    return np.ascontiguousarray(a.reshape(M // mt * P, (K // 256) * 2 * mt))


def shard_inputs(positions, hidden_states, residual, qkv_w, o_w, up_w, down_w,
                 ln1_w, ln2_w):
    hTf = np.ascontiguousarray(
        np.asarray(hidden_states).reshape(S, HID).T.astype(ml_dtypes.bfloat16))
    rTf = np.ascontiguousarray(
        np.asarray(residual).reshape(S, HID).T.astype(ml_dtypes.bfloat16))
    pos = np.asarray(positions).reshape(S).astype(np.float64)
    inv = 1.0 / (THETA ** (np.arange(0, DHEAD, 2, dtype=np.float64) / DHEAD))
    fr = pos[:, None] * inv                      # [S, 64]
    cost = np.cos(fr).T.astype(np.float32)       # [64, S]
    sint = np.sin(fr).T.astype(np.float32)
    cos2 = np.ascontiguousarray(
        np.concatenate([cost, cost], 0).astype(ml_dtypes.bfloat16))
    sin_neg = np.ascontiguousarray(
        np.concatenate([-sint, sint], 0).astype(ml_dtypes.bfloat16))
    q_size = N_HEADS * DHEAD
    kv = N_KV * DHEAD
    w1 = np.asarray(qkv_w, np.float32) * np.asarray(ln1_w, np.float32)[:, None] * SW1
    wof = np.asarray(o_w, np.float32) * SWO
    wuf = np.asarray(up_w, np.float32) * np.asarray(ln2_w, np.float32)[:, None] * SWU
    wdf = np.asarray(down_w, np.float32) * SWD
    in_maps = []
    for c in range(N_CORES):
        wq_c = np.concatenate([
            w1[:, c * HQ * DHEAD:(c + 1) * HQ * DHEAD],
            w1[:, q_size + c * DHEAD:q_size + (c + 1) * DHEAD],
            w1[:, q_size + kv + c * DHEAD:q_size + kv + (c + 1) * DHEAD],
        ], axis=1)
        wq_h, wq_l = _q8_pair(wq_c)
        wo_h, wo_l = _q8_pair(wof[:, c * HID_SH:(c + 1) * HID_SH])
        wu_h, wu_l = _q8_pair(wuf[:, c * INT_SH:(c + 1) * INT_SH])
        wd_h, wd_l = _q8_pair(wdf[c * INT_SH:(c + 1) * INT_SH, :])
        wq_h, wq_l = _lay_tiles(wq_h), _lay_tiles(wq_l)
        wo_h, wo_l = _lay_tiles(wo_h, mt=HID_SH), _lay_tiles(wo_l, mt=HID_SH)
        wu_h, wu_l = _lay_tiles(wu_h), _lay_tiles(wu_l)
        wd_h, wd_l = _lay_tiles(wd_h), _lay_tiles(wd_l)
        in_maps.append({
            "hT": np.ascontiguousarray(hTf[c * HID_SH:(c + 1) * HID_SH]),
            "rT": np.ascontiguousarray(rTf[c * HID_SH:(c + 1) * HID_SH]),
            "cos2": cos2, "sin_neg": sin_neg,
            "wq_hi": wq_h, "wq_lo": wq_l,
            "wo_hi": wo_h, "wo_lo": wo_l,
            "wu_hi": wu_h, "wu_lo": wu_l,
            "wd_hi": wd_h, "wd_lo": wd_l,
        })
    return in_maps


_CACHE = {}


def kernel(**inputs):
    from concourse.bass_utils import run_bass_kernel_spmd
    if "nc" not in _CACHE:
        _CACHE["nc"] = build_graph()
    nc = _CACHE["nc"]
    in_maps = shard_inputs(**{k: np.asarray(v) for k, v in inputs.items()})
    res = run_bass_kernel_spmd(nc, in_maps, core_ids=list(range(N_CORES)),
                               trace=False)
    res2T = np.concatenate([res.results[c]["res2T"] for c in range(N_CORES)], axis=0)
    mlpT = np.empty((HID, S), np.float32)
    for c in range(N_CORES):
        mt = res.results[c]["mlpT"]
        mstart = 0
        for pi, mc in enumerate(PIECES):
            rows = mc * P // N_CORES          # rows per core for this piece
            orow = mstart * P // N_CORES
            g0 = mstart * P + c * rows        # global hid row start
            mlpT[g0:g0 + rows] = mt[orow:orow + rows]
            mstart += mc
    mlp_out = np.ascontiguousarray(mlpT.T).reshape(1, S, HID)
    residual2 = np.ascontiguousarray(res2T.T).reshape(1, S, HID)
    return mlp_out, residual2
